# revision 1
# baseline (speedup 1.0000x reference)
"""AttentionRNN (BiDAF-style QA reader) Trainium2 kernel.

Per core (pure data-parallel over batch, 4 of 32 rows per core):
  1. Embedding gather via transposing dma_gather (two int16-indexed halves of
     the bf16 table merged with copy_predicated) into (E, token) layout.
  2. xp = ep @ wih.T + bias for the 4 GRU directions (bf16 PE).
  3. GRU scans as chunked-parallel recurrences: chunks of S=16 payload steps
     with W=24 warmup steps re-run from h=0 (the GRU contracts ~0.65/step, so
     warmup error ~3e-5 < bf16 noise).  Chunks whose warmup would cross t=0
     are frozen (z pinned to 1 via +1e4 logit) until their true start — those
     are exact.  Padding steps freeze h the same way.  All directions/chunks/
     batch advance in lockstep: one round = one time step of 72 chains.
  4. Decomposed BiDAF attention, softmax over Q, start/end heads, log-softmax
     over P; padded positions forced to exactly -1e7 as in the reference.
"""

import contextlib
import os

import numpy as np
import ml_dtypes

import concourse.bass as bass
import concourse.mybir as mybir
from concourse import library_config
from concourse.masks import make_identity
from concourse.tile import TileContext
from concourse.bass_utils import run_bass_kernel_spmd

F32 = mybir.dt.float32
BF16 = mybir.dt.bfloat16
I16 = mybir.dt.int16
U8 = mybir.dt.uint8
AX = mybir.AxisListType.X
ALU = mybir.AluOpType
AF = mybir.ActivationFunctionType

B, P, Q, E, H, VOCAB = 32, 512, 64, 300, 256, 50000
HH = 128
EPAD = 384
NC = 8
BC = B // NC
NEG = -1e7
BIGM = 1.0e4

S, W = 16, 20
RND = S + W                   # 40 rounds
NCHP, NCHQ = P // S, Q // S   # 32, 4
FDP, FDQ = NCHP * BC, NCHQ * BC   # 128, 16
OFF_PF, OFF_PB, OFF_QF, OFF_QB = 0, FDP, 2 * FDP, 2 * FDP + FDQ
FDT = 2 * FDP + 2 * FDQ       # 288

NTP, NTQ = BC * P, BC * Q     # 2048, 256
XC = 2 * NTP + 2 * NTQ        # 4608
D_PF, D_PB, D_QF, D_QB = 0, NTP, 2 * NTP, 2 * NTP + NTQ
HALF = 32768

_CACHE = {}


def _round_geom(k):
    e = k - W
    s = e % S
    coff = (e - s) // S                      # -2 | -1 | 0
    cmin = 0 if k >= W else -((k - W) // S) - (1 if (W - k) % S == 0 else 1) + 1
    # cmin = ceil((W-k)/S) for k < W
    if k < W:
        cmin = (W - k + S - 1) // S
    return s, coff, cmin


def _build_nc():
    nc = bass.Bass()

    epTp_d = nc.declare_dram_parameter("epTp_d", [128, 3 * NTP], BF16,
                                       isOutput=False)
    epTq_d = nc.declare_dram_parameter("epTq_d", [128, 3 * NTQ], BF16,
                                       isOutput=False)
    m0 = nc.declare_dram_parameter("m0", [1, XC], F32, isOutput=False)
    m8 = nc.declare_dram_parameter("m8", [2 * BC, P], U8, isOutput=False)
    wihT = nc.declare_dram_parameter("wihT", [128, 4 * 3 * 3 * HH], BF16,
                                     isOutput=False)
    whhT = nc.declare_dram_parameter("whhT", [128, 4 * 3 * HH], BF16,
                                     isOutput=False)
    brzn = nc.declare_dram_parameter("brzn", [128, 12], F32, isOutput=False)
    bhnp = nc.declare_dram_parameter("bhn", [128, 4], F32, isOutput=False)
    outw = nc.declare_dram_parameter("outw", [HH, 8], F32, isOutput=False)
    seww = nc.declare_dram_parameter("sew", [HH, 14], BF16, isOutput=False)
    out = nc.declare_dram_parameter("out", [4 * BC, P], F32, isOutput=True)

    es = contextlib.ExitStack()

    def sb(name, shape, dtype):
        return es.enter_context(nc.sbuf_tensor(name, shape, dtype))

    # raw sbuf: written only in the pre-Tile preamble (or read-only consts)
    epTp = sb("epTp", [128, 3, NTP], BF16)
    epTq = sb("epTq", [128, 3, NTQ], BF16)
    m0b = sb("m0b", [128, XC], F32)
    wih_sb = sb("wih_sb", [128, 4 * 3 * 3 * HH], BF16)
    whh_sb = sb("whh_sb", [128, 4 * 3 * HH], BF16)
    brzn_sb = sb("brzn_sb", [128, 12], F32)
    bhn_sb = sb("bhn_sb", [128, 4], F32)
    outw_sb = sb("outw_sb", [128, 8], F32)
    sew_sb = sb("sew_sb", [128, 14], BF16)
    m8_sb = sb("m8_sb", [2 * BC, P], U8)
    neg_sb = sb("neg_sb", [2 * BC, P], F32)
    ones_sb = sb("ones_sb", [1, 128], BF16)

    # ---------- raw preamble ----------
    pre_sem = es.enter_context(nc.semaphore("pre_sem"))
    g = nc.gpsimd
    nd = 0

    def dma(dst, src):
        nonlocal nd
        g.dma_start(out=dst, in_=src).then_inc(pre_sem, 16)
        nd += 1

    dma(wih_sb[:, :], wihT[:, :])
    dma(whh_sb[:, :], whhT[:, :])
    dma(brzn_sb[:, :], brzn[:, :])
    dma(bhn_sb[:, :], bhnp[:, :])
    dma(outw_sb[:, :], outw[:, :])
    dma(sew_sb[:, :], seww[:, :])
    dma(m8_sb[:, :], m8[:, :])
    dma(m0b[:, :], m0[:, :].broadcast_to([128, XC]))
    dma(epTp[:, :, :], epTp_d[:, :].rearrange("p (c t) -> p c t", c=3))
    dma(epTq[:, :, :], epTq_d[:, :].rearrange("p (c t) -> p c t", c=3))
    nc.vector.wait_ge(pre_sem, nd * 16)
    nc.vector.memset(neg_sb[:, :], NEG)
    nc.vector.memset(ones_sb[:, :], 1.0)
    cmb_sem = es.enter_context(nc.semaphore("cmb_sem"))
    nc.vector.drain()
    nc.vector.sem_inc(cmb_sem, 1)
    for eng in (nc.scalar, nc.tensor, nc.gpsimd, nc.sync):
        eng.wait_ge(cmb_sem, 1)

    # ---------- Tile phases ----------
    with TileContext(nc) as tc:
        with tc.tile_pool(name="psA", bufs=2, space="PSUM") as psp, \
             tc.tile_pool(name="psB", bufs=2, space="PSUM") as psn, \
             tc.tile_pool(name="sbp", bufs=2) as sbp, \
             tc.tile_pool(name="pst", bufs=1) as pst:

            def pt(name, shape, dtype):
                return pst.tile(shape, dtype, name=name, tag=name)

            xr = pt("xr", [128, XC], F32)
            xz = pt("xz", [128, XC], F32)
            xn = pt("xn", [128, XC], F32)
            pencFB = pt("pencFB", [128, 2 * NTP], BF16)
            qencFB = pt("qencFB", [128, 2 * NTQ], BF16)
            hcur = pt("hcur", [128, FDT], BF16)
            rz_sb = pt("rz_sb", [128, 2 * FDT], BF16)
            t1_sb = pt("t1_sb", [128, FDT], F32)
            t2_sb = pt("t2_sb", [128, FDT], F32)
            n_sb = pt("n_sb", [128, FDT], BF16)
            d_sb = pt("d_sb", [128, FDT], BF16)
            e_sb = pt("e_sb", [128, FDT], BF16)
            srz = pt("srz", [128, 2 * FDT], F32)
            ident_sb = pt("ident_sb", [128, 128], BF16)
            qenc3 = pt("qenc3", [128, 2 * NTQ], BF16)
            qwm = pt("qwm", [1, NTQ], BF16)
            qwt = pt("qwt", [1, NTQ], F32)
            probs = pt("probs", [128, 64 * 4 * BC], BF16)
            probsT = pt("probsT", [64, P * BC], BF16)
            qencT = pt("qencT", [64, 2 * HH * BC], BF16)
            attwFB = pt("attwFB", [128, 2 * NTP], BF16)
            pawFB = pt("pawFB", [128, 2 * NTP], BF16)
            se_sb = pt("se_sb", [2, BC * P], F32)
            se8 = pt("se8", [2 * BC, P], F32)
            lsm_sb = pt("lsm_sb", [2 * BC, P], F32)
            lse_sb = pt("lse_sb", [2 * BC, P], F32)
            red_sb = pt("red_sb", [2 * BC, 8], F32)

            make_identity(nc, ident_sb[:, :])
            nc.vector.memset(hcur[:, :], 0)
            nc.vector.memset(t2_sb[:, :], 0)
            nc.vector.memset(srz[:, :], 0)

            # ---- xp projections ----
            for di, (dbase, epT, ntok) in enumerate(
                    ((D_PF, epTp, NTP), (D_PB, epTp, NTP),
                     (D_QF, epTq, NTQ), (D_QB, epTq, NTQ))):
                for gate, xdst in ((0, xr), (1, xz), (2, xn)):
                    for nb in range(0, ntok, 512):
                        nn = min(512, ntok - nb)
                        pp = psp.tile([128, 1024], F32, name="xps", tag="big")
                        for kc in range(3):
                            wcol = ((di * 3 + kc) * 3 + gate) * HH
                            nc.tensor.matmul(
                                pp[:, 0:nn], wih_sb[:, wcol:wcol + HH],
                                epT[:, kc, nb:nb + nn],
                                start=(kc == 0), stop=(kc == 2))
                        nc.scalar.activation(
                            xdst[:, dbase + nb:dbase + nb + nn], pp[:, 0:nn],
                            AF.Identity,
                            bias=brzn_sb[:, di * 3 + gate:di * 3 + gate + 1])

            for s0 in range(0, XC, 2304):
                nc.vector.scalar_tensor_tensor(
                    xz[:, s0:s0 + 2304], m0b[:, s0:s0 + 2304], BIGM,
                    xz[:, s0:s0 + 2304], op0=ALU.mult, op1=ALU.add)

            # round-sliced views (p, c, b, s)
            def view4(x, base, ntok, nch):
                return x[:, base:base + ntok].rearrange(
                    "p (b c s) -> p c b s", b=BC, c=nch, s=S)

            xv = {}
            for nm, x in (("xr", xr), ("xz", xz), ("xn", xn)):
                xv[(nm, "pf")] = view4(x, D_PF, NTP, NCHP)
                xv[(nm, "pb")] = view4(x, D_PB, NTP, NCHP)
                xv[(nm, "qf")] = view4(x, D_QF, NTQ, NCHQ)
                xv[(nm, "qb")] = view4(x, D_QB, NTQ, NCHQ)
            pv_f = view4(pencFB, 0, NTP, NCHP)
            pv_b = view4(pencFB, NTP, NTP, NCHP)
            qv_f = view4(qencFB, 0, NTQ, NCHQ)
            qv_b = view4(qencFB, NTQ, NTQ, NCHQ)

            def xslices(nm, k):
                s, coff, cmin = _round_geom(k)
                res = []
                for nch, off, szd, kf, kb in ((NCHP, OFF_PF, FDP, "pf", "pb"),
                                              (NCHQ, OFF_QF, FDQ, "qf", "qb")):
                    cnt = nch - cmin
                    res.append((off + cmin * BC, cnt * BC,
                                xv[(nm, kf)][:, cmin + coff:cmin + coff + cnt,
                                             :, s].squeeze()))
                    res.append((off + szd, cnt * BC,
                                xv[(nm, kb)][:, -coff:-coff + cnt, :,
                                             S - 1 - s].squeeze()))
                return res

            def cb(apx):
                return apx.rearrange("p (c b) -> p c b", b=BC)

            # ---- the scan ----
            _PH = 3
            DIRS = ((0, OFF_PF, FDP), (1, OFF_PB, FDP),
                    (2, OFF_QF, FDQ), (3, OFF_QB, FDQ))
            for k in range(RND if _PH >= 2 else 0):
                s, coff, cmin = _round_geom(k)
                prz = psp.tile([128, 1024], F32, name="prz", tag="big")
                pn = psn.tile([128, 512], F32, name="pn", tag="pn")
                for gi, goff in ((0, 0), (1, 512)):
                    for j, (d, off, fd) in enumerate(DIRS):
                        nc.tensor.matmul(
                            prz[:, goff + off:goff + off + fd],
                            whh_sb[:, (d * 3 + gi) * HH:(d * 3 + gi + 1) * HH],
                            hcur[:, off:off + fd],
                            start=(j == 0), stop=(j == 3))
                for j, (d, off, fd) in enumerate(DIRS):
                    nc.tensor.matmul(
                        pn[:, off:off + fd],
                        whh_sb[:, (d * 3 + 2) * HH:(d * 3 + 3) * HH],
                        hcur[:, off:off + fd],
                        start=(j == 0), stop=(j == 3))

                for off, wd, ap in xslices("xr", k):
                    nc.vector.tensor_add(cb(srz[:, off:off + wd]),
                                         cb(prz[:, off:off + wd]), ap)
                if cmin > 0:
                    for lo, hi in ((OFF_PF, OFF_PF + cmin * BC),
                                   (OFF_QF, OFF_QF + cmin * BC),
                                   (OFF_PB + FDP - cmin * BC, OFF_PB + FDP),
                                   (OFF_QB + FDQ - cmin * BC, OFF_QB + FDQ)):
                        nc.vector.tensor_scalar_add(srz[:, lo:hi],
                                                    prz[:, lo:hi], 0.0)
                nc.scalar.activation(rz_sb[:, 0:FDT], srz[:, 0:FDT], AF.Sigmoid)

                for off, wd, ap in xslices("xz", k):
                    nc.vector.tensor_add(cb(srz[:, FDT + off:FDT + off + wd]),
                                         cb(prz[:, 512 + off:512 + off + wd]),
                                         ap)
                if cmin > 0:
                    for lo, hi in ((OFF_PF, OFF_PF + cmin * BC),
                                   (OFF_QF, OFF_QF + cmin * BC),
                                   (OFF_PB + FDP - cmin * BC, OFF_PB + FDP),
                                   (OFF_QB + FDQ - cmin * BC, OFF_QB + FDQ)):
                        nc.vector.tensor_scalar_add(srz[:, FDT + lo:FDT + hi],
                                                    prz[:, 512 + lo:512 + hi],
                                                    BIGM)
                nc.scalar.activation(rz_sb[:, FDT:2 * FDT],
                                     srz[:, FDT:2 * FDT], AF.Sigmoid)

                for d, off, fd in DIRS:
                    nc.vector.scalar_tensor_tensor(
                        t1_sb[:, off:off + fd], pn[:, off:off + fd],
                        bhn_sb[:, d:d + 1], rz_sb[:, off:off + fd],
                        op0=ALU.add, op1=ALU.mult)
                for off, wd, ap in xslices("xn", k):
                    nc.gpsimd.tensor_add(cb(t2_sb[:, off:off + wd]),
                                         cb(t1_sb[:, off:off + wd]), ap)
                nc.scalar.activation(n_sb[:, 0:FDT], t2_sb[:, 0:FDT], AF.Tanh)

                nc.vector.tensor_sub(d_sb[:, :], hcur[:, :], n_sb[:, :])
                nc.vector.tensor_mul(e_sb[:, :], rz_sb[:, FDT:2 * FDT],
                                     d_sb[:, :])
                nc.vector.tensor_add(hcur[:, :], n_sb[:, :], e_sb[:, :])

                if k >= W:
                    nc.gpsimd.tensor_copy(pv_f[:, :, :, s],
                                          cb(hcur[:, OFF_PF:OFF_PF + FDP]))
                    nc.gpsimd.tensor_copy(pv_b[:, :, :, S - 1 - s],
                                          cb(hcur[:, OFF_PB:OFF_PB + FDP]))
                    nc.gpsimd.tensor_copy(qv_f[:, :, :, s],
                                          cb(hcur[:, OFF_QF:OFF_QF + FDQ]))
                    nc.gpsimd.tensor_copy(qv_b[:, :, :, S - 1 - s],
                                          cb(hcur[:, OFF_QB:OFF_QB + FDQ]))

            # ---- attention ----
            if _PH < 3:
                nc.sync.dma_start(out[0:2, :], se_sb[0:2, 0:P])
            _dummy_done = None
            if _PH >= 3:
                nc.scalar.activation(qenc3[:, 0:NTQ], qencFB[:, 0:NTQ], AF.Copy,
                                     scale=outw_sb[:, 4:5])
                nc.scalar.activation(qenc3[:, NTQ:2 * NTQ], qencFB[:, NTQ:2 * NTQ],
                                     AF.Copy, scale=outw_sb[:, 5:6])
                pqw = psn.tile([1, 512], F32, name="pqw", tag="pn")
                nc.tensor.matmul(pqw[0:1, 0:NTQ], sew_sb[:, 12:13],
                                 qencFB[:, 0:NTQ], start=True, stop=False)
                nc.tensor.matmul(pqw[0:1, 0:NTQ], sew_sb[:, 13:14],
                                 qencFB[:, NTQ:2 * NTQ], start=False, stop=True)
                nc.scalar.activation(qwt[0:1, :], pqw[0:1, 0:NTQ], AF.Identity,
                                     bias=outw_sb[0:1, 2:3])
                nc.vector.scalar_tensor_tensor(
                    qwm[0:1, :], m0b[0:1, D_QF:D_QF + NTQ], NEG, qwt[0:1, :],
                    op0=ALU.mult, op1=ALU.add)

                for b in range(BC):
                    for tcn in range(4):
                        plg = psp.tile([128, 1024], F32, name="plg", tag="big")
                        t0 = b * P + tcn * 128
                        nc.tensor.matmul(plg[:, 0:64], pencFB[:, t0:t0 + 128],
                                         qenc3[:, b * Q:(b + 1) * Q],
                                         start=True, stop=False)
                        nc.tensor.matmul(plg[:, 0:64],
                                         pencFB[:, NTP + t0:NTP + t0 + 128],
                                         qenc3[:, NTQ + b * Q:NTQ + (b + 1) * Q],
                                         start=False, stop=False)
                        nc.tensor.matmul(plg[:, 0:64], ones_sb[0:1, :],
                                         qwm[0:1, b * Q:(b + 1) * Q],
                                         start=False, stop=True)
                        mxn = sbp.tile([128, 1], F32, name="mxn")
                        nc.vector.tensor_reduce(mxn[:, :], plg[:, 0:64], AX,
                                                ALU.max, negate=True)
                        ex = sbp.tile([128, 64], BF16, name="ex")
                        nc.scalar.activation(ex[:, :], plg[:, 0:64], AF.Exp,
                                             bias=mxn[:, 0:1])
                        sm = sbp.tile([128, 1], F32, name="sm")
                        nc.vector.tensor_reduce(sm[:, :], ex[:, :], AX, ALU.add)
                        rs = sbp.tile([128, 1], F32, name="rs")
                        nc.vector.reciprocal(rs[:, :], sm[:, :])
                        nc.vector.tensor_scalar_mul(
                            probs[:, (b * 4 + tcn) * 64:(b * 4 + tcn + 1) * 64],
                            ex[:, :], rs[:, 0:1])

                for b in range(BC):
                    for tcn in range(4):
                        ptr = psn.tile([128, 512], BF16, name="ptr", tag="pn")
                        nc.tensor.transpose(
                            ptr[0:64, 0:128],
                            probs[:, (b * 4 + tcn) * 64:(b * 4 + tcn + 1) * 64],
                            ident_sb[:, :])
                        nc.scalar.activation(
                            probsT[:, b * P + tcn * 128:b * P + (tcn + 1) * 128],
                            ptr[0:64, 0:128], AF.Copy)
                    for hc in range(2):
                        ptr = psn.tile([128, 512], BF16, name="ptq", tag="pn")
                        nc.tensor.transpose(
                            ptr[0:64, 0:128],
                            qencFB[:, hc * NTQ + b * Q:hc * NTQ + (b + 1) * Q],
                            ident_sb[:, :])
                        nc.scalar.activation(
                            qencT[:, (b * 2 + hc) * 128:(b * 2 + hc + 1) * 128],
                            ptr[0:64, 0:128], AF.Copy)

                for b in range(BC):
                    for hc in range(2):
                        paw = psp.tile([128, 1024], F32, name="paw", tag="big")
                        nc.tensor.matmul(
                            paw[:, 0:P],
                            qencT[0:64, (b * 2 + hc) * 128:(b * 2 + hc + 1) * 128],
                            probsT[0:64, b * P:(b + 1) * P], start=True, stop=True)
                        nc.scalar.activation(
                            attwFB[:, hc * NTP + b * P:hc * NTP + (b + 1) * P],
                            paw[:, 0:P], AF.Copy)
                nc.vector.tensor_mul(pawFB[:, :], pencFB[:, :], attwFB[:, :])

                for b in range(BC):
                    pse = psn.tile([2, 512], F32, name="pse", tag="pn")
                    rhss = (pencFB[:, b * P:(b + 1) * P],
                            pencFB[:, NTP + b * P:NTP + (b + 1) * P],
                            attwFB[:, b * P:(b + 1) * P],
                            attwFB[:, NTP + b * P:NTP + (b + 1) * P],
                            pawFB[:, b * P:(b + 1) * P],
                            pawFB[:, NTP + b * P:NTP + (b + 1) * P])
                    for j, rhs in enumerate(rhss):
                        nc.tensor.matmul(pse[0:2, 0:P], sew_sb[:, 2 * j:2 * j + 2],
                                         rhs, start=(j == 0), stop=(j == 5))
                    nc.scalar.activation(se_sb[0:2, b * P:(b + 1) * P],
                                         pse[0:2, 0:P],
                                         AF.Identity, bias=outw_sb[0:2, 3:4])
                nc.sync.dma_start(se8[0:BC, :], se_sb[0:1, :])
                nc.sync.dma_start(se8[BC:2 * BC, :], se_sb[1:2, :])
                nc.vector.copy_predicated(se8[:, :], m8_sb[:, :], neg_sb[:, :])

                nc.vector.tensor_reduce(red_sb[:, 0:1], se8[:, :], AX, ALU.max)
                nc.vector.tensor_reduce(red_sb[:, 1:2], se8[:, :], AX, ALU.max,
                                        negate=True)
                nc.scalar.activation(lse_sb[:, :], se8[:, :], AF.Exp,
                                     bias=red_sb[:, 1:2])
                nc.vector.tensor_reduce(red_sb[:, 2:3], lse_sb[:, :], AX, ALU.add)
                nc.scalar.activation(red_sb[:, 3:4], red_sb[:, 2:3], AF.Ln)
                nc.vector.tensor_add(red_sb[:, 4:5], red_sb[:, 0:1],
                                     red_sb[:, 3:4])
                nc.vector.tensor_scalar(out=lsm_sb[:, :], in0=se8[:, :],
                                        scalar1=red_sb[:, 4:5], scalar2=None,
                                        op0=ALU.subtract)

                nc.sync.dma_start(out[0:2 * BC, :], se8[:, :])
                nc.sync.dma_start(out[2 * BC:4 * BC, :], lsm_sb[:, :])

    _split_multiwaits(nc)
    return nc, es


def _split_multiwaits(nc):
    """HW instruction encodings hold a single semaphore wait; move extra
    waits emitted by Tile onto same-engine NOPs inserted just before."""
    for b in nc.main_func.blocks:
        il = b.instructions
        newlist = []
        for inst in il:
            if type(inst).__name__ == "InstISA":
                # EVENT_SEMAPHORE_RANGE_CLEAR mis-encodes for this walrus
                # build; NRT clears semaphores per execution anyway.
                continue
            si = inst.sync_info
            if si is not None and len(si.on_wait) > 1:
                waits = list(si.on_wait)
                for wx in waits[:-1]:
                    nop = nc.engines[inst.engine].nop(hint="wsplit").ins
                    # remove from wherever nop() appended it
                    for bb in nc.main_func.blocks:
                        try:
                            bb.instructions.remove(nop)
                            break
                        except ValueError:
                            pass
                    nop.sync_info = mybir.SyncInfo(on_wait=[wx], on_update=[])
                    newlist.append(nop)
                inst.sync_info = mybir.SyncInfo(on_wait=[waits[-1]],
                                                on_update=list(si.on_update))
            newlist.append(inst)
        il[:] = newlist


def _wrap16(idx):
    return np.ascontiguousarray(np.tile(idx.reshape(-1, 16).T, (8, 1)))


def _prep_core(inputs, c):
    bs = slice(c * BC, (c + 1) * BC)
    ptok = np.asarray(inputs["passage"][bs]).astype(np.int64).reshape(-1)
    qtok = np.asarray(inputs["question"][bs]).astype(np.int64).reshape(-1)
    d = {}
    embp = inputs["_embp"]
    d["epTp_d"] = np.ascontiguousarray(
        embp[ptok].T.reshape(3, 128, NTP).transpose(1, 0, 2).reshape(128, -1))
    d["epTq_d"] = np.ascontiguousarray(
        embp[qtok].T.reshape(3, 128, NTQ).transpose(1, 0, 2).reshape(128, -1))
    pm0 = (ptok == 0).astype(np.float32)
    qm0 = (qtok == 0).astype(np.float32)
    d["m0"] = np.ascontiguousarray(
        np.concatenate([pm0, pm0, qm0, qm0])[None, :])
    pm2 = (ptok == 0).reshape(BC, P).astype(np.uint8)
    d["m8"] = np.ascontiguousarray(np.concatenate([pm2, pm2], axis=0))
    return d


def _prep_shared(inputs):
    bf = ml_dtypes.bfloat16

    wihT = np.zeros((4, 3, 128, 3 * HH), bf)      # (d, kc, p, m)
    whhT = np.zeros((4, HH, 3 * HH), bf)          # (d, p, m)
    brzn = np.zeros((4, HH, 3), np.float32)
    bhnv = np.zeros((4, HH, 1), np.float32)
    for di, (pre, dd) in enumerate((("p", "f"), ("p", "b"),
                                    ("q", "f"), ("q", "b"))):
        wih = np.asarray(inputs[f"{pre}_wih_{dd}"], np.float32)
        whh = np.asarray(inputs[f"{pre}_whh_{dd}"], np.float32)
        bih = np.asarray(inputs[f"{pre}_bih_{dd}"], np.float32)
        bhh = np.asarray(inputs[f"{pre}_bhh_{dd}"], np.float32)
        wT = np.zeros((EPAD, 3 * HH), bf)
        wT[:E, :] = wih.T.astype(bf)
        wihT[di] = wT.reshape(3, 128, 3 * HH)
        whhT[di] = whh.T.astype(bf)
        for gg in range(3):
            brzn[di, :, gg] = bih[gg * HH:(gg + 1) * HH] + (
                bhh[gg * HH:(gg + 1) * HH] if gg < 2 else 0)
        bhnv[di, :, 0] = bhh[2 * HH:]
    # flatten to (p, flat) device layouts
    wihT = np.ascontiguousarray(
        wihT.transpose(2, 0, 1, 3).reshape(128, -1))      # (p,(d,kc,m))
    whhT = np.ascontiguousarray(
        whhT.transpose(1, 0, 2).reshape(128, -1))         # (p,(d,m))
    brzn = np.ascontiguousarray(brzn.transpose(1, 0, 2).reshape(128, 12))
    bhnv = np.ascontiguousarray(bhnv.transpose(1, 0, 2).reshape(128, 4))

    aw = np.asarray(inputs["attn_w"], np.float32)
    w1, w2, w3 = aw[:256], aw[256:512], aw[512:]
    outw = np.zeros((HH, 8), np.float32)
    outw[:, 4], outw[:, 5] = w3[:128], w3[128:]
    outw[0, 2] = float(np.asarray(inputs["attn_b"]))
    outw[0, 3] = float(np.asarray(inputs["start_b"]))
    outw[1, 3] = float(np.asarray(inputs["end_b"]))

    sw = np.asarray(inputs["start_w"], np.float32)
    ew = np.asarray(inputs["end_w"], np.float32)
    sew = np.zeros((HH, 14), bf)
    for j in range(6):
        sew[:, 2 * j] = sw[j * 128:(j + 1) * 128].astype(bf)
        sew[:, 2 * j + 1] = ew[j * 128:(j + 1) * 128].astype(bf)
    sew[:, 12] = w2[:128].astype(bf)
    sew[:, 13] = w2[128:].astype(bf)
    return {"wihT": wihT, "whhT": whhT, "brzn": brzn,
            "bhn": bhnv, "outw": outw, "sew": sew}


def kernel(**inputs):
    if "nc" not in _CACHE:
        _CACHE["nc"] = _build_nc()
    nc, _es = _CACHE["nc"]
    shared = _prep_shared(inputs)
    bf = ml_dtypes.bfloat16
    embp = np.zeros((VOCAB, EPAD), bf)
    embp[:, :E] = np.asarray(inputs["emb"], np.float32).astype(bf)
    inputs = dict(inputs)
    inputs["_embp"] = embp
    in_maps = []
    for c in range(NC):
        m = dict(shared)
        m.update(_prep_core(inputs, c))
        in_maps.append(m)
    res = run_bass_kernel_spmd(nc, in_maps, list(range(NC)))
    outs = [np.asarray(res.results[c]["out"]) for c in range(NC)]
    se = np.concatenate([o[0:2 * BC].reshape(2, BC, P) for o in outs], axis=1)
    lsm = np.concatenate([o[2 * BC:].reshape(2, BC, P) for o in outs], axis=1)
    return (np.ascontiguousarray(se[0]), np.ascontiguousarray(se[1]),
            np.ascontiguousarray(lsm[0]), np.ascontiguousarray(lsm[1]))



# revision 3
# speedup vs baseline: 2.1818x; 2.1818x over previous
"""AttentionRNN (BiDAF-style QA reader) Trainium2 kernel.

Per core (pure data-parallel over batch, 4 of 32 rows per core):
  1. Host gathers embeddings in an s-major permuted token order and pads two
     extra embedding rows: row 300 = pad-token indicator (drives a +BIGM into
     the z gate via the weight matrix, freezing h at padded steps), row 301 =
     constant 1.0 (injects the gate biases).  So each scan round's x-gate
     pre-activations are plain contiguous-slice matmuls.
  2. GRU scans as chunked-parallel recurrences: chunks of S=16 payload steps
     with W=12 warmup steps re-run from h=0 (the GRU contracts ~0.6/step).
     Chunks whose warmup would cross t=0 are frozen (z pinned via +BIGM)
     until their true start.  One round = one time step of 72 chains; the
     x-projection matmuls for round k+1 are issued ahead of round k's
     recurrent matmuls so the PE stays busy during the serial chain.
  3. Decomposed BiDAF attention, softmax over Q, start/end heads, log-softmax
     over P; padded positions forced to exactly -1e7 as in the reference.
"""

import contextlib

import numpy as np
import ml_dtypes

import concourse.bass as bass
import concourse.mybir as mybir
from concourse.masks import make_identity
from concourse.tile import TileContext
from concourse.bass_utils import run_bass_kernel_spmd

F32 = mybir.dt.float32
BF16 = mybir.dt.bfloat16
U8 = mybir.dt.uint8
AX = mybir.AxisListType.X
ALU = mybir.AluOpType
AF = mybir.ActivationFunctionType

B, P, Q, E, H, VOCAB = 32, 512, 64, 300, 256, 50000
HH = 128
EPAD = 384
NC = 8
BC = B // NC
NEG = -1e7
BIGM = 1.0e4

S, W = 16, 12
RND = S + W                   # 28 rounds
NCHP, NCHQ = P // S, Q // S   # 32, 4
FDP, FDQ = NCHP * BC, NCHQ * BC   # 128, 16
OFF_PF, OFF_PB, OFF_QF, OFF_QB = 0, FDP, 2 * FDP, 2 * FDP + FDQ
FDT = 2 * FDP + 2 * FDQ       # 288

NTP, NTQ = BC * P, BC * Q     # 2048, 256

_CACHE = {}


def _mk_rank():
    order, seen = [], set()
    for k in range(RND):
        s = (k - W) % S
        for v in (s, S - 1 - s):
            if v not in seen:
                seen.add(v)
                order.append(v)
    rank = [0] * S
    for i, s in enumerate(order):
        rank[s] = i
    return order, rank


SORD, SRANK = _mk_rank()


def _round_geom(k):
    e = k - W
    s = e % S
    coff = (e - s) // S                      # -1 | 0
    cmin = (W - k + S - 1) // S if k < W else 0
    return s, coff, cmin


def _build_nc():
    nc = bass.Bass()

    epTp_d = nc.declare_dram_parameter("epTp_d", [128, 3 * NTP], BF16,
                                       isOutput=False)
    epTq_d = nc.declare_dram_parameter("epTq_d", [128, 3 * NTQ], BF16,
                                       isOutput=False)
    mq_d = nc.declare_dram_parameter("mq", [1, NTQ], F32, isOutput=False)
    m8 = nc.declare_dram_parameter("m8", [2 * BC, P], U8, isOutput=False)
    wihT = nc.declare_dram_parameter("wihT", [128, 4 * 3 * 3 * HH], BF16,
                                     isOutput=False)
    whhT = nc.declare_dram_parameter("whhT", [128, 4 * 3 * HH], BF16,
                                     isOutput=False)
    bhnr_d = nc.declare_dram_parameter("bhnr", [1, 4 * HH], BF16,
                                       isOutput=False)
    outw = nc.declare_dram_parameter("outw", [HH, 8], F32, isOutput=False)
    seww = nc.declare_dram_parameter("sew", [HH, 14], BF16, isOutput=False)
    out = nc.declare_dram_parameter("out", [4 * BC, P], F32, isOutput=True)

    es = contextlib.ExitStack()

    def sb(name, shape, dtype):
        return es.enter_context(nc.sbuf_tensor(name, shape, dtype))

    # constants initialized before the Tile phases
    neg_sb = sb("neg_sb", [2 * BC, P], F32)
    ones_sb = sb("ones_sb", [1, 128], BF16)
    bigm_sb = sb("bigm_sb", [1, 128], BF16)

    pre_sem = es.enter_context(nc.semaphore("pre_sem"))
    nc.vector.memset(neg_sb[:, :], NEG)
    nc.vector.memset(ones_sb[:, :], 1.0)
    nc.vector.memset(bigm_sb[:, :], BIGM)
    nc.vector.drain()
    nc.vector.sem_inc(pre_sem, 1)
    for eng in (nc.scalar, nc.tensor, nc.gpsimd, nc.sync):
        eng.wait_ge(pre_sem, 1)

    # ---------- Tile phases ----------
    with TileContext(nc) as tc:
        with tc.tile_pool(name="psA", bufs=2, space="PSUM") as psA, \
             tc.tile_pool(name="psB", bufs=2, space="PSUM") as psB, \
             tc.tile_pool(name="sbp", bufs=2) as sbp, \
             tc.tile_pool(name="pst", bufs=1) as pst:

            def pt(name, shape, dtype):
                return pst.tile(shape, dtype, name=name, tag=name)

            # input tiles (DMA-streamed)
            epTp = pt("epTp", [128, 3, NTP], BF16)
            epTq = pt("epTq", [128, 3, NTQ], BF16)
            wih_sb = pt("wih_sb", [128, 4 * 3 * 3 * HH], BF16)
            whh_sb = pt("whh_sb", [128, 4 * 3 * HH], BF16)
            bhnr_sb = pt("bhnr_sb", [1, 4 * HH], BF16)
            mq_sb = pt("mq_sb", [1, NTQ], F32)
            m8_sb = pt("m8_sb", [2 * BC, P], U8)
            outw_sb = pt("outw_sb", [128, 8], F32)
            sew_sb = pt("sew_sb", [128, 14], BF16)

            # scan state
            pencFB = pt("pencFB", [128, 2 * NTP], BF16)
            qencFB = pt("qencFB", [128, 2 * NTQ], BF16)
            hcur = pt("hcur", [128, FDT], BF16)
            rz_sb = pt("rz_sb", [128, 2 * FDT], BF16)
            nh_sb = pt("nh_sb", [128, FDT], BF16)
            nx_sb = pt("nx_sb", [128, 2, FDT], BF16)
            t1_sb = pt("t1_sb", [128, FDT], BF16)
            t2_sb = pt("t2_sb", [128, FDT], BF16)
            n_sb = pt("n_sb", [128, FDT], BF16)
            d_sb = pt("d_sb", [128, FDT], BF16)
            e_sb = pt("e_sb", [128, FDT], BF16)
            ident_sb = pt("ident_sb", [128, 128], BF16)
            # attention tiles
            qenc3 = pt("qenc3", [128, 2 * NTQ], BF16)
            qwm = pt("qwm", [1, NTQ], BF16)
            qwt = pt("qwt", [1, NTQ], F32)
            probs = pt("probs", [128, 64 * 4 * BC], BF16)
            probsT = pt("probsT", [64, P * BC], BF16)
            qencT = pt("qencT", [64, 2 * HH * BC], BF16)
            attwFB = pt("attwFB", [128, 2 * NTP], BF16)
            pawFB = pt("pawFB", [128, 2 * NTP], BF16)
            se_sb = pt("se_sb", [2, BC * P], F32)
            se8 = pt("se8", [2 * BC, P], F32)
            lsm_sb = pt("lsm_sb", [2 * BC, P], F32)
            lse_sb = pt("lse_sb", [2 * BC, P], F32)
            red_sb = pt("red_sb", [2 * BC, 8], F32)

            # ---- input DMA (gpsimd queue: cheap issue), priority order ----
            g = nc.gpsimd
            g.dma_start(whh_sb[:, :], whhT[:, :])
            g.dma_start(wih_sb[:, :], wihT[:, :])
            g.dma_start(epTq[:, :, :],
                        epTq_d[:, :].rearrange("p (c t) -> p c t", c=3))
            epv = epTp_d[:, :].rearrange("p (c t) -> p c t", c=3)
            for r0, r1 in ((0, 2), (2, 4), (4, 6), (6, 8), (8, 16)):
                g.dma_start(epTp[:, :, r0 * FDP:r1 * FDP],
                            epv[:, :, r0 * FDP:r1 * FDP])
            g.dma_start(bhnr_sb[:, :], bhnr_d[:, :])
            g.dma_start(mq_sb[:, :], mq_d[:, :])
            g.dma_start(m8_sb[:, :], m8[:, :])
            g.dma_start(outw_sb[0:HH, :], outw[:, :])
            g.dma_start(sew_sb[0:HH, :], seww[:, :])

            make_identity(nc, ident_sb[:, :])
            nc.vector.memset(hcur[:, :], 0)

            # round psum tiles: T1 holds r (bank0) | z (bank1),
            #                   T2 holds nx (bank0) | nh (bank1)
            tiles = [None] * RND

            def alloc_round(j):
                tiles[j] = (psA.tile([128, 1024], F32, name="T1", tag="t1"),
                            psB.tile([128, 1024], F32, name="T2", tag="t2"))

            # x-projection geometry for round j: per dir, the contiguous
            # epT column range and psum dst range.
            def xgeom(j):
                s, coff, cmin = _round_geom(j)
                res = []
                for (nch, fd, offF, offB, epT, blk) in (
                        (NCHP, FDP, OFF_PF, OFF_PB, epTp, FDP),
                        (NCHQ, FDQ, OFF_QF, OFF_QB, epTq, FDQ)):
                    cnt = nch - cmin
                    # forward: chains [cmin, nch) read chunk c+coff at pos s
                    res.append((epT,
                                SRANK[s] * blk + (cmin + coff) * BC,
                                offF + cmin * BC, cnt * BC))
                    # backward: chains [0, cnt) read chunk c-coff at pos S-1-s
                    res.append((epT,
                                SRANK[S - 1 - s] * blk + (-coff) * BC,
                                offB, cnt * BC))
                return res

            def emit_wih(j):
                T1, T2 = tiles[j]
                s, coff, cmin = _round_geom(j)
                geo = xgeom(j)
                for gate, dst, goff in ((0, T1, 0), (1, T1, 512), (2, T2, 0)):
                    first = True
                    for di in range(4):
                        epT, c0, o0, wd = geo[di]
                        for kc in range(3):
                            wcol = ((di * 3 + kc) * 3 + gate) * HH
                            # the nx group (gate 2) has no whh part, so its
                            # last matmul closes the accumulation group
                            last = gate == 2 and di == 3 and kc == 2
                            nc.tensor.matmul(
                                dst[:, goff + o0:goff + o0 + wd],
                                wih_sb[:, wcol:wcol + HH],
                                epT[:, kc, c0:c0 + wd],
                                start=first, stop=last)
                            first = False
                    if gate == 1 and cmin > 0:
                        # freeze warmup-frozen chains: z += BIGM
                        fz = cmin * BC
                        for lo in (OFF_PF, OFF_PB + FDP - fz,
                                   OFF_QF, OFF_QB + FDQ - fz):
                            nc.tensor.matmul(
                                dst[:, 512 + lo:512 + lo + fz],
                                bigm_sb[0:1, :], ones_sb[0:1, 0:fz],
                                start=False, stop=False)
                # nh group: bhh_n broadcast rows
                for di, (off, fd) in enumerate(((OFF_PF, FDP), (OFF_PB, FDP),
                                                (OFF_QF, FDQ), (OFF_QB, FDQ))):
                    nc.tensor.matmul(T2[:, 512 + off:512 + off + fd],
                                     bhnr_sb[0:1, di * HH:(di + 1) * HH],
                                     ones_sb[0:1, 0:fd],
                                     start=(di == 0), stop=False)

            def emit_whh(j):
                T1, T2 = tiles[j]
                DIRS = ((0, OFF_PF, FDP), (1, OFF_PB, FDP),
                        (2, OFF_QF, FDQ), (3, OFF_QB, FDQ))
                for gi, dst, goff in ((0, T1, 0), (1, T1, 512), (2, T2, 512)):
                    for di, (d, off, fd) in enumerate(DIRS):
                        nc.tensor.matmul(
                            dst[:, goff + off:goff + off + fd],
                            whh_sb[:, (d * 3 + gi) * HH:(d * 3 + gi + 1) * HH],
                            hcur[:, off:off + fd],
                            start=False, stop=(di == 3))

            # payload output views (b-major column layout: b*T + c*S + s)
            def view4(x, base, ntok, nch):
                return x[:, base:base + ntok].rearrange(
                    "p (b c s) -> p c b s", b=BC, c=nch, s=S)

            pv_f = view4(pencFB, 0, NTP, NCHP)
            pv_b = view4(pencFB, NTP, NTP, NCHP)
            qv_f = view4(qencFB, 0, NTQ, NCHQ)
            qv_b = view4(qencFB, NTQ, NTQ, NCHQ)

            # ---- the scan ----
            alloc_round(0)
            emit_wih(0)
            alloc_round(1)
            emit_wih(1)
            nc.scalar.activation(nx_sb[:, 0, :], tiles[0][1][:, 0:FDT],
                                 AF.Copy)
            for k in range(RND):
                T1, T2 = tiles[k]
                s, coff, cmin = _round_geom(k)
                emit_whh(k)
                # nh evacuation (psum -> sbuf bf16), overlaps sigmoid
                nc.vector.tensor_scalar_add(nh_sb[:, :], T2[:, 512:512 + FDT],
                                            0.0)
                # r|z sigmoid in one strided pass over both psum banks
                rzv = T1[:, :].rearrange("p (g c) -> p g c", g=2)[:, :, 0:FDT]
                nc.scalar.activation(
                    rz_sb[:, :].rearrange("p (g c) -> p g c", g=2),
                    rzv, AF.Sigmoid)
                nc.vector.tensor_mul(t1_sb[:, :], rz_sb[:, 0:FDT],
                                     nh_sb[:, :])
                nc.vector.tensor_add(t2_sb[:, :], t1_sb[:, :],
                                     nx_sb[:, k % 2, :])
                nc.scalar.activation(n_sb[:, :], t2_sb[:, :], AF.Tanh)
                nc.vector.tensor_sub(d_sb[:, :], hcur[:, :], n_sb[:, :])
                nc.vector.tensor_mul(e_sb[:, :], rz_sb[:, FDT:2 * FDT],
                                     d_sb[:, :])
                nc.vector.tensor_add(hcur[:, :], n_sb[:, :], e_sb[:, :])
                if k + 2 < RND:
                    alloc_round(k + 2)
                    emit_wih(k + 2)
                if k + 1 < RND:
                    nc.scalar.activation(nx_sb[:, (k + 1) % 2, :],
                                         tiles[k + 1][1][:, 0:FDT], AF.Copy)
                if k >= W:
                    nc.gpsimd.tensor_copy(pv_f[:, :, :, s],
                                          hcur[:, OFF_PF:OFF_PF + FDP]
                                          .rearrange("p (c b) -> p c b", b=BC))
                    nc.gpsimd.tensor_copy(pv_b[:, :, :, S - 1 - s],
                                          hcur[:, OFF_PB:OFF_PB + FDP]
                                          .rearrange("p (c b) -> p c b", b=BC))
                    nc.gpsimd.tensor_copy(qv_f[:, :, :, s],
                                          hcur[:, OFF_QF:OFF_QF + FDQ]
                                          .rearrange("p (c b) -> p c b", b=BC))
                    nc.gpsimd.tensor_copy(qv_b[:, :, :, S - 1 - s],
                                          hcur[:, OFF_QB:OFF_QB + FDQ]
                                          .rearrange("p (c b) -> p c b", b=BC))

            # ---- attention ----
            nc.scalar.activation(qenc3[:, 0:NTQ], qencFB[:, 0:NTQ], AF.Copy,
                                 scale=outw_sb[:, 4:5])
            nc.scalar.activation(qenc3[:, NTQ:2 * NTQ], qencFB[:, NTQ:2 * NTQ],
                                 AF.Copy, scale=outw_sb[:, 5:6])
            pqw = psB.tile([1, 512], F32, name="pqw", tag="t2")
            nc.tensor.matmul(pqw[0:1, 0:NTQ], sew_sb[:, 12:13],
                             qencFB[:, 0:NTQ], start=True, stop=False)
            nc.tensor.matmul(pqw[0:1, 0:NTQ], sew_sb[:, 13:14],
                             qencFB[:, NTQ:2 * NTQ], start=False, stop=True)
            nc.scalar.activation(qwt[0:1, :], pqw[0:1, 0:NTQ], AF.Identity,
                                 bias=outw_sb[0:1, 2:3])
            nc.vector.scalar_tensor_tensor(
                qwm[0:1, :], mq_sb[0:1, :], NEG, qwt[0:1, :],
                op0=ALU.mult, op1=ALU.add)

            for b in range(BC):
                for tcn in range(4):
                    plg = psA.tile([128, 1024], F32, name="plg", tag="t1")
                    t0 = b * P + tcn * 128
                    nc.tensor.matmul(plg[:, 0:64], pencFB[:, t0:t0 + 128],
                                     qenc3[:, b * Q:(b + 1) * Q],
                                     start=True, stop=False)
                    nc.tensor.matmul(plg[:, 0:64],
                                     pencFB[:, NTP + t0:NTP + t0 + 128],
                                     qenc3[:, NTQ + b * Q:NTQ + (b + 1) * Q],
                                     start=False, stop=False)
                    nc.tensor.matmul(plg[:, 0:64], ones_sb[0:1, :],
                                     qwm[0:1, b * Q:(b + 1) * Q],
                                     start=False, stop=True)
                    mxn = sbp.tile([128, 1], F32, name="mxn")
                    nc.vector.tensor_reduce(mxn[:, :], plg[:, 0:64], AX,
                                            ALU.max, negate=True)
                    ex = sbp.tile([128, 64], BF16, name="ex")
                    nc.scalar.activation(ex[:, :], plg[:, 0:64], AF.Exp,
                                         bias=mxn[:, 0:1])
                    sm = sbp.tile([128, 1], F32, name="sm")
                    nc.vector.tensor_reduce(sm[:, :], ex[:, :], AX, ALU.add)
                    rs = sbp.tile([128, 1], F32, name="rs")
                    nc.vector.reciprocal(rs[:, :], sm[:, :])
                    nc.vector.tensor_scalar_mul(
                        probs[:, (b * 4 + tcn) * 64:(b * 4 + tcn + 1) * 64],
                        ex[:, :], rs[:, 0:1])

            for b in range(BC):
                for tcn in range(4):
                    ptr = psB.tile([128, 512], BF16, name="ptr", tag="t2")
                    nc.tensor.transpose(
                        ptr[0:64, 0:128],
                        probs[:, (b * 4 + tcn) * 64:(b * 4 + tcn + 1) * 64],
                        ident_sb[:, :])
                    nc.scalar.activation(
                        probsT[:, b * P + tcn * 128:b * P + (tcn + 1) * 128],
                        ptr[0:64, 0:128], AF.Copy)
                for hc in range(2):
                    ptr = psB.tile([128, 512], BF16, name="ptq", tag="t2")
                    nc.tensor.transpose(
                        ptr[0:64, 0:128],
                        qencFB[:, hc * NTQ + b * Q:hc * NTQ + (b + 1) * Q],
                        ident_sb[:, :])
                    nc.scalar.activation(
                        qencT[:, (b * 2 + hc) * 128:(b * 2 + hc + 1) * 128],
                        ptr[0:64, 0:128], AF.Copy)

            for b in range(BC):
                for hc in range(2):
                    paw = psA.tile([128, 1024], F32, name="paw", tag="t1")
                    nc.tensor.matmul(
                        paw[:, 0:P],
                        qencT[0:64, (b * 2 + hc) * 128:(b * 2 + hc + 1) * 128],
                        probsT[0:64, b * P:(b + 1) * P], start=True, stop=True)
                    nc.scalar.activation(
                        attwFB[:, hc * NTP + b * P:hc * NTP + (b + 1) * P],
                        paw[:, 0:P], AF.Copy)
            nc.vector.tensor_mul(pawFB[:, :], pencFB[:, :], attwFB[:, :])

            for b in range(BC):
                pse = psB.tile([2, 512], F32, name="pse", tag="t2")
                rhss = (pencFB[:, b * P:(b + 1) * P],
                        pencFB[:, NTP + b * P:NTP + (b + 1) * P],
                        attwFB[:, b * P:(b + 1) * P],
                        attwFB[:, NTP + b * P:NTP + (b + 1) * P],
                        pawFB[:, b * P:(b + 1) * P],
                        pawFB[:, NTP + b * P:NTP + (b + 1) * P])
                for j, rhs in enumerate(rhss):
                    nc.tensor.matmul(pse[0:2, 0:P], sew_sb[:, 2 * j:2 * j + 2],
                                     rhs, start=(j == 0), stop=(j == 5))
                nc.scalar.activation(se_sb[0:2, b * P:(b + 1) * P],
                                     pse[0:2, 0:P],
                                     AF.Identity, bias=outw_sb[0:2, 3:4])
            nc.sync.dma_start(se8[0:BC, :], se_sb[0:1, :])
            nc.sync.dma_start(se8[BC:2 * BC, :], se_sb[1:2, :])
            nc.vector.copy_predicated(se8[:, :], m8_sb[:, :], neg_sb[:, :])

            nc.vector.tensor_reduce(red_sb[:, 0:1], se8[:, :], AX, ALU.max)
            nc.vector.tensor_reduce(red_sb[:, 1:2], se8[:, :], AX, ALU.max,
                                    negate=True)
            nc.scalar.activation(lse_sb[:, :], se8[:, :], AF.Exp,
                                 bias=red_sb[:, 1:2])
            nc.vector.tensor_reduce(red_sb[:, 2:3], lse_sb[:, :], AX, ALU.add)
            nc.scalar.activation(red_sb[:, 3:4], red_sb[:, 2:3], AF.Ln)
            nc.vector.tensor_add(red_sb[:, 4:5], red_sb[:, 0:1],
                                 red_sb[:, 3:4])
            nc.vector.tensor_scalar(out=lsm_sb[:, :], in0=se8[:, :],
                                    scalar1=red_sb[:, 4:5], scalar2=None,
                                    op0=ALU.subtract)

            nc.sync.dma_start(out[0:2 * BC, :], se8[:, :])
            nc.sync.dma_start(out[2 * BC:4 * BC, :], lsm_sb[:, :])

    _split_multiwaits(nc)
    return nc, es


def _split_multiwaits(nc):
    """HW instruction encodings hold a single semaphore wait; move extra
    waits emitted by Tile onto same-engine NOPs inserted just before."""
    for b in nc.main_func.blocks:
        il = b.instructions
        newlist = []
        for inst in il:
            if type(inst).__name__ == "InstISA":
                # EVENT_SEMAPHORE_RANGE_CLEAR mis-encodes for this walrus
                # build; NRT clears semaphores per execution anyway.
                continue
            si = inst.sync_info
            if si is not None and len(si.on_wait) > 1:
                waits = list(si.on_wait)
                for wx in waits[:-1]:
                    nop = nc.engines[inst.engine].nop(hint="wsplit").ins
                    # remove from wherever nop() appended it
                    for bb in nc.main_func.blocks:
                        try:
                            bb.instructions.remove(nop)
                            break
                        except ValueError:
                            pass
                    nop.sync_info = mybir.SyncInfo(on_wait=[wx], on_update=[])
                    newlist.append(nop)
                inst.sync_info = mybir.SyncInfo(on_wait=[waits[-1]],
                                                on_update=list(si.on_update))
            newlist.append(inst)
        il[:] = newlist


def _perm_tokens(tok2d, nch, blk):
    """Token array (BC, T) -> s-major column order: col = rank(s)*blk + c*BC + b."""
    T = tok2d.shape[1]
    cols = np.empty(BC * T, np.int64)
    for rank in range(S):
        s = SORD[rank]
        blkv = tok2d[:, s::S]          # (BC, nch) tokens at pos s per chunk
        # col index rank*blk + c*BC + b
        cols[rank * blk:(rank + 1) * blk] = blkv.T.reshape(-1)
    return cols


def _prep_core(inputs, c):
    bs = slice(c * BC, (c + 1) * BC)
    ptok = np.asarray(inputs["passage"][bs]).astype(np.int64)
    qtok = np.asarray(inputs["question"][bs]).astype(np.int64)
    embp = inputs["_embp"]
    pcols = _perm_tokens(ptok, NCHP, FDP)
    qcols = _perm_tokens(qtok, NCHQ, FDQ)
    d = {}
    d["epTp_d"] = np.ascontiguousarray(
        embp[pcols].T.reshape(3, 128, NTP).transpose(1, 0, 2).reshape(128, -1))
    d["epTq_d"] = np.ascontiguousarray(
        embp[qcols].T.reshape(3, 128, NTQ).transpose(1, 0, 2).reshape(128, -1))
    qm0 = (qtok.reshape(-1) == 0).astype(np.float32)
    d["mq"] = np.ascontiguousarray(qm0[None, :])
    pm2 = (ptok.reshape(-1) == 0).reshape(BC, P).astype(np.uint8)
    d["m8"] = np.ascontiguousarray(np.concatenate([pm2, pm2], axis=0))
    return d


def _prep_shared(inputs):
    bf = ml_dtypes.bfloat16

    wihT = np.zeros((4, 3, 128, 3 * HH), bf)      # (d, kc, p, m)
    whhT = np.zeros((4, HH, 3 * HH), bf)          # (d, p, m)
    bhnr = np.zeros((4, HH), bf)
    for di, (pre, dd) in enumerate((("p", "f"), ("p", "b"),
                                    ("q", "f"), ("q", "b"))):
        wih = np.asarray(inputs[f"{pre}_wih_{dd}"], np.float32)
        whh = np.asarray(inputs[f"{pre}_whh_{dd}"], np.float32)
        bih = np.asarray(inputs[f"{pre}_bih_{dd}"], np.float32)
        bhh = np.asarray(inputs[f"{pre}_bhh_{dd}"], np.float32)
        wT = np.zeros((EPAD, 3 * HH), np.float32)
        wT[:E, :] = wih.T
        # row 300: pad-token indicator -> +BIGM on the z gate
        wT[300, HH:2 * HH] = BIGM
        # row 301: constant-1 -> gate biases (bih+bhh for r/z, bih for n)
        wT[301, 0:HH] = bih[0:HH] + bhh[0:HH]
        wT[301, HH:2 * HH] = bih[HH:2 * HH] + bhh[HH:2 * HH]
        wT[301, 2 * HH:] = bih[2 * HH:]
        wihT[di] = wT.astype(bf).reshape(3, 128, 3 * HH)
        whhT[di] = whh.T.astype(bf)
        bhnr[di] = bhh[2 * HH:].astype(bf)
    wihT = np.ascontiguousarray(
        wihT.transpose(2, 0, 1, 3).reshape(128, -1))      # (p,(d,kc,m))
    whhT = np.ascontiguousarray(
        whhT.transpose(1, 0, 2).reshape(128, -1))         # (p,(d,m))
    bhnr = np.ascontiguousarray(bhnr.reshape(1, -1))

    aw = np.asarray(inputs["attn_w"], np.float32)
    w1, w2, w3 = aw[:256], aw[256:512], aw[512:]
    outw = np.zeros((HH, 8), np.float32)
    outw[:, 4], outw[:, 5] = w3[:128], w3[128:]
    outw[0, 2] = float(np.asarray(inputs["attn_b"]))
    outw[0, 3] = float(np.asarray(inputs["start_b"]))
    outw[1, 3] = float(np.asarray(inputs["end_b"]))

    sw = np.asarray(inputs["start_w"], np.float32)
    ew = np.asarray(inputs["end_w"], np.float32)
    sew = np.zeros((HH, 14), bf)
    for j in range(6):
        sew[:, 2 * j] = sw[j * 128:(j + 1) * 128].astype(bf)
        sew[:, 2 * j + 1] = ew[j * 128:(j + 1) * 128].astype(bf)
    sew[:, 12] = w2[:128].astype(bf)
    sew[:, 13] = w2[128:].astype(bf)
    return {"wihT": wihT, "whhT": whhT, "bhnr": bhnr,
            "outw": outw, "sew": sew}


def kernel(**inputs):
    if "nc" not in _CACHE:
        _CACHE["nc"] = _build_nc()
    nc, _es = _CACHE["nc"]
    shared = _prep_shared(inputs)
    bf = ml_dtypes.bfloat16
    embp = np.zeros((VOCAB, EPAD), bf)
    embp[:, :E] = np.asarray(inputs["emb"], np.float32).astype(bf)
    embp[0, 300] = 1.0   # pad-token indicator row
    embp[:, 301] = 1.0   # constant-1 bias row
    inputs = dict(inputs)
    inputs["_embp"] = embp
    in_maps = []
    for c in range(NC):
        m = dict(shared)
        m.update(_prep_core(inputs, c))
        in_maps.append(m)
    res = run_bass_kernel_spmd(nc, in_maps, list(range(NC)))
    outs = [np.asarray(res.results[c]["out"]) for c in range(NC)]
    se = np.concatenate([o[0:2 * BC].reshape(2, BC, P) for o in outs], axis=1)
    lsm = np.concatenate([o[2 * BC:].reshape(2, BC, P) for o in outs], axis=1)
    return (np.ascontiguousarray(se[0]), np.ascontiguousarray(se[1]),
            np.ascontiguousarray(lsm[0]), np.ascontiguousarray(lsm[1]))


# revision 10
# speedup vs baseline: 2.5384x; 1.1635x over previous
"""AttentionRNN (BiDAF-style QA reader) Trainium2 kernel.

Per core (pure data-parallel over batch, 4 of 32 rows per core):
  1. Host gathers embeddings in an s-major permuted token order and pads two
     extra embedding rows: row 300 = pad-token indicator (drives a +BIGM into
     the z gate via the weight matrix, freezing h at padded steps), row 301 =
     constant 1.0 (injects the gate biases).  So each scan round's x-gate
     pre-activations are plain contiguous-slice matmuls.
  2. GRU scans as chunked-parallel recurrences: chunks of S=16 payload steps
     with W=12 warmup steps re-run from h=0 (the GRU contracts ~0.6/step).
     Chunks whose warmup would cross t=0 are frozen (z pinned via +BIGM)
     until their true start.  One round = one time step of 72 chains; the
     x-projection matmuls for round k+1 are issued ahead of round k's
     recurrent matmuls so the PE stays busy during the serial chain.
  3. Decomposed BiDAF attention, softmax over Q, start/end heads, log-softmax
     over P; padded positions forced to exactly -1e7 as in the reference.
"""

import contextlib

import numpy as np
import ml_dtypes

import concourse.bass as bass
import concourse.mybir as mybir
from concourse.masks import make_identity
from concourse.tile import TileContext
from concourse.bass_utils import run_bass_kernel_spmd

F32 = mybir.dt.float32
BF16 = mybir.dt.bfloat16
U8 = mybir.dt.uint8
AX = mybir.AxisListType.X
ALU = mybir.AluOpType
AF = mybir.ActivationFunctionType

B, P, Q, E, H, VOCAB = 32, 512, 64, 300, 256, 50000
HH = 128
EPAD = 384
NC = 8
BC = B // NC
NEG = -1e7
BIGM = 1.0e4

S, W = 16, 12
RND = S + W                   # 28 rounds
NCHP, NCHQ = P // S, Q // S   # 32, 4
FDP, FDQ = NCHP * BC, NCHQ * BC   # 128, 16
OFF_PF, OFF_PB, OFF_QF, OFF_QB = 0, FDP, 2 * FDP, 2 * FDP + FDQ
FDT = 2 * FDP + 2 * FDQ       # 288

NTP, NTQ = BC * P, BC * Q     # 2048, 256

_CACHE = {}


def _mk_rank():
    order, seen = [], set()
    for k in range(RND):
        s = (k - W) % S
        for v in (s, S - 1 - s):
            if v not in seen:
                seen.add(v)
                order.append(v)
    rank = [0] * S
    for i, s in enumerate(order):
        rank[s] = i
    return order, rank


SORD, SRANK = _mk_rank()


def _round_geom(k):
    e = k - W
    s = e % S
    coff = (e - s) // S                      # -1 | 0
    cmin = (W - k + S - 1) // S if k < W else 0
    return s, coff, cmin


def _build_nc():
    nc = bass.Bass()

    epTp_d = nc.declare_dram_parameter("epTp_d", [128, 3 * NTP], BF16,
                                       isOutput=False)
    epTq_d = nc.declare_dram_parameter("epTq_d", [128, 3 * NTQ], BF16,
                                       isOutput=False)
    mq_d = nc.declare_dram_parameter("mq", [1, NTQ], F32, isOutput=False)
    m8 = nc.declare_dram_parameter("m8", [2 * BC, P], U8, isOutput=False)
    wihT = nc.declare_dram_parameter("wihT", [128, 4 * 3 * 3 * HH], BF16,
                                     isOutput=False)
    whhT = nc.declare_dram_parameter("whhT", [128, 4 * 3 * HH], BF16,
                                     isOutput=False)
    bhnr_d = nc.declare_dram_parameter("bhnr", [1, 4 * HH], BF16,
                                       isOutput=False)
    outw = nc.declare_dram_parameter("outw", [HH, 8], F32, isOutput=False)
    seww = nc.declare_dram_parameter("sew", [HH, 14], BF16, isOutput=False)
    out = nc.declare_dram_parameter("out", [4 * BC, P], F32, isOutput=True)

    es = contextlib.ExitStack()

    # ---------- Tile phases ----------
    with TileContext(nc) as tc:
        with tc.tile_pool(name="psA", bufs=2, space="PSUM") as psA, \
             tc.tile_pool(name="psB", bufs=2, space="PSUM") as psB, \
             tc.tile_pool(name="sbp", bufs=2) as sbp, \
             tc.tile_pool(name="pst", bufs=1) as pst:

            def pt(name, shape, dtype):
                return pst.tile(shape, dtype, name=name, tag=name)

            neg_sb = pt("neg_sb", [2 * BC, P], F32)
            ones_sb = pt("ones_sb", [1, 128], BF16)
            bigm_sb = pt("bigm_sb", [1, 128], BF16)

            # input tiles (DMA-streamed)
            epTp = pt("epTp", [128, 3, NTP], BF16)
            epTq = pt("epTq", [128, 3, NTQ], BF16)
            wih_sb = pt("wih_sb", [128, 4 * 3 * 3 * HH], BF16)
            whh_sb = pt("whh_sb", [128, 4 * 3 * HH], BF16)
            bhnr_sb = pt("bhnr_sb", [1, 4 * HH], BF16)
            mq_sb = pt("mq_sb", [1, NTQ], F32)
            m8_sb = pt("m8_sb", [2 * BC, P], U8)
            outw_sb = pt("outw_sb", [128, 8], F32)
            sew_sb = pt("sew_sb", [128, 14], BF16)

            # scan state
            pencFB = pt("pencFB", [128, 2 * NTP], BF16)
            qencFB = pt("qencFB", [128, 2 * NTQ], BF16)
            hcur = pt("hcur", [128, FDT], BF16)
            rz_sb = pt("rz_sb", [128, 2 * FDT], BF16)
            nh_sb = pt("nh_sb", [128, FDT], BF16)
            nx_sb = pt("nx_sb", [128, 2, FDT], BF16)
            t1_sb = pt("t1_sb", [128, FDT], BF16)
            t2_sb = pt("t2_sb", [128, FDT], BF16)
            n_sb = pt("n_sb", [128, FDT], BF16)
            d_sb = pt("d_sb", [128, FDT], BF16)
            e_sb = pt("e_sb", [128, FDT], BF16)
            ident_sb = pt("ident_sb", [128, 128], BF16)
            # attention tiles
            qenc3 = pt("qenc3", [128, 2 * NTQ], BF16)
            qwm = pt("qwm", [1, NTQ], BF16)
            qwt = pt("qwt", [1, NTQ], F32)
            probs = pt("probs", [128, 64 * 4 * BC], BF16)
            probsT = pt("probsT", [64, P * BC], BF16)
            qencT = pt("qencT", [64, 2 * HH * BC], BF16)
            attwFB = pt("attwFB", [128, 2 * NTP], BF16)
            pawFB = pt("pawFB", [128, 2 * NTP], BF16)
            se_sb = pt("se_sb", [2, BC * P], F32)
            se8 = pt("se8", [2 * BC, P], F32)
            lsm_sb = pt("lsm_sb", [2 * BC, P], F32)
            lse_sb = pt("lse_sb", [2 * BC, P], F32)
            red_sb = pt("red_sb", [2 * BC, 8], F32)

            # ---- input DMA (gpsimd queue: cheap issue), priority order ----
            g = nc.gpsimd
            HW12 = 4 * 3 * 3 * HH // 2
            g.dma_start(wih_sb[:, 0:HW12], wihT[:, 0:HW12])       # p dirs
            epv = epTp_d[:, :].rearrange("p (c t) -> p c t", c=3)
            g.dma_start(epTp[:, :, 0:2 * FDP], epv[:, :, 0:2 * FDP])
            g.dma_start(wih_sb[:, HW12:], wihT[:, HW12:])         # q dirs
            g.dma_start(epTq[:, :, :],
                        epTq_d[:, :].rearrange("p (c t) -> p c t", c=3))
            g.dma_start(whh_sb[:, :], whhT[:, :])
            g.dma_start(bhnr_sb[:, :], bhnr_d[:, :])
            for r0, r1 in ((2, 4), (4, 6), (6, 8), (8, 16)):
                g.dma_start(epTp[:, :, r0 * FDP:r1 * FDP],
                            epv[:, :, r0 * FDP:r1 * FDP])
            g.dma_start(mq_sb[:, :], mq_d[:, :])
            g.dma_start(m8_sb[:, :], m8[:, :])
            g.dma_start(outw_sb[0:HH, :], outw[:, :])
            g.dma_start(sew_sb[0:HH, :], seww[:, :])

            nc.vector.memset(ones_sb[:, :], 1.0)
            nc.vector.memset(bigm_sb[:, :], BIGM)
            nc.vector.memset(hcur[:, :], 0)
            nc.vector.memset(neg_sb[:, :], NEG)
            make_identity(nc, ident_sb[:, :])

            # round psum tiles: T1 holds r (bank0) | z (bank1),
            #                   T2 holds nx (bank0) | nh (bank1)
            tiles = [None] * RND

            def alloc_round(j):
                tiles[j] = (psA.tile([128, 1024], F32, name="T1", tag="t1"),
                            psB.tile([128, 1024], F32, name="T2", tag="t2"))

            # x-projection geometry for round j: per dir, the contiguous
            # epT column range and psum dst range.
            def xgeom(j):
                s, coff, cmin = _round_geom(j)
                res = []
                for (nch, fd, offF, offB, epT, blk) in (
                        (NCHP, FDP, OFF_PF, OFF_PB, epTp, FDP),
                        (NCHQ, FDQ, OFF_QF, OFF_QB, epTq, FDQ)):
                    cnt = nch - cmin
                    # forward: chains [cmin, nch) read chunk c+coff at pos s
                    res.append((epT,
                                SRANK[s] * blk + (cmin + coff) * BC,
                                offF + cmin * BC, cnt * BC))
                    # backward: chains [0, cnt) read chunk c-coff at pos S-1-s
                    res.append((epT,
                                SRANK[S - 1 - s] * blk + (-coff) * BC,
                                offB, cnt * BC))
                return res

            def emit_wih(j):
                T1, T2 = tiles[j]
                s, coff, cmin = _round_geom(j)
                geo = xgeom(j)
                for gate, dst, goff in ((0, T1, 0), (1, T1, 512), (2, T2, 0)):
                    first = True
                    for di in range(4):
                        epT, c0, o0, wd = geo[di]
                        for kc in range(3):
                            wcol = ((di * 3 + kc) * 3 + gate) * HH
                            # the nx group (gate 2) has no whh part, so its
                            # last matmul closes the accumulation group
                            last = gate == 2 and di == 3 and kc == 2
                            nc.tensor.matmul(
                                dst[:, goff + o0:goff + o0 + wd],
                                wih_sb[:, wcol:wcol + HH],
                                epT[:, kc, c0:c0 + wd],
                                start=first, stop=last)
                            first = False
                    if gate == 1 and cmin > 0:
                        # freeze warmup-frozen chains: z += BIGM
                        fz = cmin * BC
                        for lo in (OFF_PF, OFF_PB + FDP - fz,
                                   OFF_QF, OFF_QB + FDQ - fz):
                            nc.tensor.matmul(
                                dst[:, 512 + lo:512 + lo + fz],
                                bigm_sb[0:1, :], ones_sb[0:1, 0:fz],
                                start=False, stop=False)
                # nh group: bhh_n broadcast rows
                for di, (off, fd) in enumerate(((OFF_PF, FDP), (OFF_PB, FDP),
                                                (OFF_QF, FDQ), (OFF_QB, FDQ))):
                    nc.tensor.matmul(T2[:, 512 + off:512 + off + fd],
                                     bhnr_sb[0:1, di * HH:(di + 1) * HH],
                                     ones_sb[0:1, 0:fd],
                                     start=(di == 0), stop=False)

            def emit_whh(j):
                T1, T2 = tiles[j]
                DIRS = ((0, OFF_PF, FDP), (1, OFF_PB, FDP),
                        (2, OFF_QF, FDQ), (3, OFF_QB, FDQ))
                for gi, dst, goff in ((0, T1, 0), (1, T1, 512), (2, T2, 512)):
                    for di, (d, off, fd) in enumerate(DIRS):
                        nc.tensor.matmul(
                            dst[:, goff + off:goff + off + fd],
                            whh_sb[:, (d * 3 + gi) * HH:(d * 3 + gi + 1) * HH],
                            hcur[:, off:off + fd],
                            start=False, stop=(di == 3))

            # payload output views (b-major column layout: b*T + c*S + s)
            def view4(x, base, ntok, nch):
                return x[:, base:base + ntok].rearrange(
                    "p (b c s) -> p c b s", b=BC, c=nch, s=S)

            pv_f = view4(pencFB, 0, NTP, NCHP)
            pv_b = view4(pencFB, NTP, NTP, NCHP)
            qv_f = view4(qencFB, 0, NTQ, NCHQ)
            qv_b = view4(qencFB, NTQ, NTQ, NCHQ)

            # ---- the scan ----
            alloc_round(0)
            emit_wih(0)
            alloc_round(1)
            emit_wih(1)
            nc.scalar.activation(nx_sb[:, 0, :], tiles[0][1][:, 0:FDT],
                                 AF.Copy)
            for k in range(RND):
                T1, T2 = tiles[k]
                s, coff, cmin = _round_geom(k)
                emit_whh(k)
                # nh evacuation (psum -> sbuf bf16), overlaps sigmoid
                nc.vector.tensor_scalar_add(nh_sb[:, :], T2[:, 512:512 + FDT],
                                            0.0)
                # r sigmoid on the critical path; z sigmoid off it
                nc.scalar.activation(rz_sb[:, 0:FDT], T1[:, 0:FDT],
                                     AF.Sigmoid)
                nc.scalar.activation(rz_sb[:, FDT:2 * FDT],
                                     T1[:, 512:512 + FDT], AF.Sigmoid)
                nc.vector.tensor_mul(t1_sb[:, :], rz_sb[:, 0:FDT],
                                     nh_sb[:, :])
                nc.vector.tensor_add(t2_sb[:, :], t1_sb[:, :],
                                     nx_sb[:, k % 2, :])
                nc.scalar.activation(n_sb[:, :], t2_sb[:, :], AF.Tanh)
                nc.vector.tensor_sub(d_sb[:, :], hcur[:, :], n_sb[:, :])
                nc.vector.tensor_mul(e_sb[:, :], rz_sb[:, FDT:2 * FDT],
                                     d_sb[:, :])
                nc.vector.tensor_add(hcur[:, :], n_sb[:, :], e_sb[:, :])
                if k + 2 < RND:
                    alloc_round(k + 2)
                    emit_wih(k + 2)
                if k + 1 < RND:
                    nc.scalar.activation(nx_sb[:, (k + 1) % 2, :],
                                         tiles[k + 1][1][:, 0:FDT], AF.Copy)
                if k >= W:
                    nc.gpsimd.tensor_copy(pv_f[:, :, :, s],
                                          hcur[:, OFF_PF:OFF_PF + FDP]
                                          .rearrange("p (c b) -> p c b", b=BC))
                    nc.gpsimd.tensor_copy(pv_b[:, :, :, S - 1 - s],
                                          hcur[:, OFF_PB:OFF_PB + FDP]
                                          .rearrange("p (c b) -> p c b", b=BC))
                    nc.gpsimd.tensor_copy(qv_f[:, :, :, s],
                                          hcur[:, OFF_QF:OFF_QF + FDQ]
                                          .rearrange("p (c b) -> p c b", b=BC))
                    nc.gpsimd.tensor_copy(qv_b[:, :, :, S - 1 - s],
                                          hcur[:, OFF_QB:OFF_QB + FDQ]
                                          .rearrange("p (c b) -> p c b", b=BC))

            # ---- attention ----
            nc.scalar.activation(qenc3[:, 0:NTQ], qencFB[:, 0:NTQ], AF.Copy,
                                 scale=outw_sb[:, 4:5])
            nc.scalar.activation(qenc3[:, NTQ:2 * NTQ], qencFB[:, NTQ:2 * NTQ],
                                 AF.Copy, scale=outw_sb[:, 5:6])
            pqw = psB.tile([1, 512], F32, name="pqw", tag="t2")
            nc.tensor.matmul(pqw[0:1, 0:NTQ], sew_sb[:, 12:13],
                             qencFB[:, 0:NTQ], start=True, stop=False)
            nc.tensor.matmul(pqw[0:1, 0:NTQ], sew_sb[:, 13:14],
                             qencFB[:, NTQ:2 * NTQ], start=False, stop=True)
            nc.scalar.activation(qwt[0:1, :], pqw[0:1, 0:NTQ], AF.Identity,
                                 bias=outw_sb[0:1, 2:3])
            nc.vector.scalar_tensor_tensor(
                qwm[0:1, :], mq_sb[0:1, :], NEG, qwt[0:1, :],
                op0=ALU.mult, op1=ALU.add)

            # logits for all 16 (b, tcn) blocks into one psum tile; the
            # qwm mask is -1e7 at padded q so exp underflows to exactly 0 --
            # no max-subtraction needed (logits are O(10) bounded).
            plg = psA.tile([128, 1024], F32, name="plg", tag="t1")
            for b in range(BC):
                for tcn in range(4):
                    t0 = b * P + tcn * 128
                    o = (b * 4 + tcn) * 64
                    nc.tensor.matmul(plg[:, o:o + 64],
                                     pencFB[:, t0:t0 + 128],
                                     qenc3[:, b * Q:(b + 1) * Q],
                                     start=True, stop=False)
                    nc.tensor.matmul(plg[:, o:o + 64],
                                     pencFB[:, NTP + t0:NTP + t0 + 128],
                                     qenc3[:, NTQ + b * Q:NTQ + (b + 1) * Q],
                                     start=False, stop=False)
                    nc.tensor.matmul(plg[:, o:o + 64], ones_sb[0:1, :],
                                     qwm[0:1, b * Q:(b + 1) * Q],
                                     start=False, stop=True)
            exu = pt("exu", [128, 1024], BF16)
            nc.scalar.activation(exu[:, :], plg[:, :], AF.Exp)
            sm16 = pt("sm16", [128, 16], F32)
            nc.vector.tensor_reduce(
                sm16[:, :], exu[:, :].rearrange("p (n q) -> p n q", n=16),
                AX, ALU.add)
            rs16 = pt("rs16", [128, 16], F32)
            nc.vector.reciprocal(rs16[:, :], sm16[:, :])
            for j in range(16):
                nc.vector.tensor_scalar_mul(
                    probs[:, j * 64:(j + 1) * 64],
                    exu[:, j * 64:(j + 1) * 64], rs16[:, j:j + 1])

            for b in range(BC):
                ptb = psB.tile([128, 512], BF16, name="ptb", tag="t2")
                for tcn in range(4):
                    nc.tensor.transpose(
                        ptb[0:64, tcn * 128:(tcn + 1) * 128],
                        probs[:, (b * 4 + tcn) * 64:(b * 4 + tcn + 1) * 64],
                        ident_sb[:, :])
                nc.scalar.activation(probsT[:, b * P:(b + 1) * P],
                                     ptb[0:64, :], AF.Copy)
            for hc2 in range(2):
                ptq = psB.tile([128, 512], BF16, name="ptq", tag="t2")
                for i in range(4):
                    b, hc = (hc2 * 4 + i) // 2, (hc2 * 4 + i) % 2
                    nc.tensor.transpose(
                        ptq[0:64, i * 128:(i + 1) * 128],
                        qencFB[:, hc * NTQ + b * Q:hc * NTQ + (b + 1) * Q],
                        ident_sb[:, :])
                nc.vector.tensor_scalar_add(
                    qencT[:, hc2 * 512:(hc2 + 1) * 512], ptq[0:64, :], 0.0)

            for b in range(BC):
                for hc in range(2):
                    paw = psA.tile([128, 1024], F32, name="paw", tag="t1")
                    nc.tensor.matmul(
                        paw[:, 0:P],
                        qencT[0:64, (b * 2 + hc) * 128:(b * 2 + hc + 1) * 128],
                        probsT[0:64, b * P:(b + 1) * P], start=True, stop=True)
                    dst = attwFB[:, hc * NTP + b * P:hc * NTP + (b + 1) * P]
                    if (b + hc) % 2 == 0:
                        nc.scalar.activation(dst, paw[:, 0:P], AF.Copy)
                    else:
                        nc.vector.tensor_scalar_add(dst, paw[:, 0:P], 0.0)
            nc.vector.tensor_mul(pawFB[:, 0:NTP], pencFB[:, 0:NTP],
                                 attwFB[:, 0:NTP])
            nc.vector.tensor_mul(pawFB[:, NTP:], pencFB[:, NTP:],
                                 attwFB[:, NTP:])

            for bp in range(2):
                pse = psB.tile([2, 1024], F32, name="pse", tag="t2")
                for bi in range(2):
                    b = bp * 2 + bi
                    rhss = (pencFB[:, b * P:(b + 1) * P],
                            pencFB[:, NTP + b * P:NTP + (b + 1) * P],
                            attwFB[:, b * P:(b + 1) * P],
                            attwFB[:, NTP + b * P:NTP + (b + 1) * P],
                            pawFB[:, b * P:(b + 1) * P],
                            pawFB[:, NTP + b * P:NTP + (b + 1) * P])
                    for j, rhs in enumerate(rhss):
                        nc.tensor.matmul(
                            pse[0:2, bi * P:(bi + 1) * P],
                            sew_sb[:, 2 * j:2 * j + 2],
                            rhs, start=(j == 0), stop=(j == 5))
                nc.scalar.activation(se_sb[0:2, bp * 2 * P:(bp * 2 + 2) * P],
                                     pse[0:2, :],
                                     AF.Identity, bias=outw_sb[0:2, 3:4])
            nc.sync.dma_start(se8[0:BC, :], se_sb[0:1, :])
            nc.sync.dma_start(se8[BC:2 * BC, :], se_sb[1:2, :])
            nc.vector.copy_predicated(se8[:, :], m8_sb[:, :], neg_sb[:, :])

            # log-softmax without max-subtraction: valid entries are O(10),
            # -1e7 pads underflow exp to 0
            nc.scalar.activation(lse_sb[:, :], se8[:, :], AF.Exp)
            nc.vector.tensor_reduce(red_sb[:, 2:3], lse_sb[:, :], AX, ALU.add)
            nc.scalar.activation(red_sb[:, 3:4], red_sb[:, 2:3], AF.Ln)
            nc.vector.tensor_scalar(out=lsm_sb[:, :], in0=se8[:, :],
                                    scalar1=red_sb[:, 3:4], scalar2=None,
                                    op0=ALU.subtract)

            nc.sync.dma_start(out[0:2 * BC, :], se8[:, :])
            nc.sync.dma_start(out[2 * BC:4 * BC, :], lsm_sb[:, :])

    _split_multiwaits(nc)
    return nc, es


def _split_multiwaits(nc):
    """HW instruction encodings hold a single semaphore wait; move extra
    waits emitted by Tile onto same-engine NOPs inserted just before."""
    for b in nc.main_func.blocks:
        il = b.instructions
        newlist = []
        for inst in il:
            if type(inst).__name__ == "InstISA":
                # EVENT_SEMAPHORE_RANGE_CLEAR mis-encodes for this walrus
                # build; NRT clears semaphores per execution anyway.
                continue
            si = inst.sync_info
            if si is not None and len(si.on_wait) > 1:
                waits = list(si.on_wait)
                for wx in waits[:-1]:
                    nop = nc.engines[inst.engine].nop(hint="wsplit").ins
                    # remove from wherever nop() appended it
                    for bb in nc.main_func.blocks:
                        try:
                            bb.instructions.remove(nop)
                            break
                        except ValueError:
                            pass
                    nop.sync_info = mybir.SyncInfo(on_wait=[wx], on_update=[])
                    newlist.append(nop)
                inst.sync_info = mybir.SyncInfo(on_wait=[waits[-1]],
                                                on_update=list(si.on_update))
            newlist.append(inst)
        il[:] = newlist


def _perm_tokens(tok2d, nch, blk):
    """Token array (BC, T) -> s-major column order: col = rank(s)*blk + c*BC + b."""
    T = tok2d.shape[1]
    cols = np.empty(BC * T, np.int64)
    for rank in range(S):
        s = SORD[rank]
        blkv = tok2d[:, s::S]          # (BC, nch) tokens at pos s per chunk
        # col index rank*blk + c*BC + b
        cols[rank * blk:(rank + 1) * blk] = blkv.T.reshape(-1)
    return cols


def _prep_core(inputs, c):
    bs = slice(c * BC, (c + 1) * BC)
    ptok = np.asarray(inputs["passage"][bs]).astype(np.int64)
    qtok = np.asarray(inputs["question"][bs]).astype(np.int64)
    embp = inputs["_embp"]
    pcols = _perm_tokens(ptok, NCHP, FDP)
    qcols = _perm_tokens(qtok, NCHQ, FDQ)
    d = {}
    d["epTp_d"] = np.ascontiguousarray(
        embp[pcols].T.reshape(3, 128, NTP).transpose(1, 0, 2).reshape(128, -1))
    d["epTq_d"] = np.ascontiguousarray(
        embp[qcols].T.reshape(3, 128, NTQ).transpose(1, 0, 2).reshape(128, -1))
    qm0 = (qtok.reshape(-1) == 0).astype(np.float32)
    d["mq"] = np.ascontiguousarray(qm0[None, :])
    pm2 = (ptok.reshape(-1) == 0).reshape(BC, P).astype(np.uint8)
    d["m8"] = np.ascontiguousarray(np.concatenate([pm2, pm2], axis=0))
    return d


def _prep_shared(inputs):
    bf = ml_dtypes.bfloat16

    wihT = np.zeros((4, 3, 128, 3 * HH), bf)      # (d, kc, p, m)
    whhT = np.zeros((4, HH, 3 * HH), bf)          # (d, p, m)
    bhnr = np.zeros((4, HH), bf)
    for di, (pre, dd) in enumerate((("p", "f"), ("p", "b"),
                                    ("q", "f"), ("q", "b"))):
        wih = np.asarray(inputs[f"{pre}_wih_{dd}"], np.float32)
        whh = np.asarray(inputs[f"{pre}_whh_{dd}"], np.float32)
        bih = np.asarray(inputs[f"{pre}_bih_{dd}"], np.float32)
        bhh = np.asarray(inputs[f"{pre}_bhh_{dd}"], np.float32)
        wT = np.zeros((EPAD, 3 * HH), np.float32)
        wT[:E, :] = wih.T
        # row 300: pad-token indicator -> +BIGM on the z gate
        wT[300, HH:2 * HH] = BIGM
        # row 301: constant-1 -> gate biases (bih+bhh for r/z, bih for n)
        wT[301, 0:HH] = bih[0:HH] + bhh[0:HH]
        wT[301, HH:2 * HH] = bih[HH:2 * HH] + bhh[HH:2 * HH]
        wT[301, 2 * HH:] = bih[2 * HH:]
        wihT[di] = wT.astype(bf).reshape(3, 128, 3 * HH)
        whhT[di] = whh.T.astype(bf)
        bhnr[di] = bhh[2 * HH:].astype(bf)
    wihT = np.ascontiguousarray(
        wihT.transpose(2, 0, 1, 3).reshape(128, -1))      # (p,(d,kc,m))
    whhT = np.ascontiguousarray(
        whhT.transpose(1, 0, 2).reshape(128, -1))         # (p,(d,m))
    bhnr = np.ascontiguousarray(bhnr.reshape(1, -1))

    aw = np.asarray(inputs["attn_w"], np.float32)
    w1, w2, w3 = aw[:256], aw[256:512], aw[512:]
    outw = np.zeros((HH, 8), np.float32)
    outw[:, 4], outw[:, 5] = w3[:128], w3[128:]
    outw[0, 2] = float(np.asarray(inputs["attn_b"]))
    outw[0, 3] = float(np.asarray(inputs["start_b"]))
    outw[1, 3] = float(np.asarray(inputs["end_b"]))

    sw = np.asarray(inputs["start_w"], np.float32)
    ew = np.asarray(inputs["end_w"], np.float32)
    sew = np.zeros((HH, 14), bf)
    for j in range(6):
        sew[:, 2 * j] = sw[j * 128:(j + 1) * 128].astype(bf)
        sew[:, 2 * j + 1] = ew[j * 128:(j + 1) * 128].astype(bf)
    sew[:, 12] = w2[:128].astype(bf)
    sew[:, 13] = w2[128:].astype(bf)
    return {"wihT": wihT, "whhT": whhT, "bhnr": bhnr,
            "outw": outw, "sew": sew}


def kernel(**inputs):
    if "nc" not in _CACHE:
        _CACHE["nc"] = _build_nc()
    nc, _es = _CACHE["nc"]
    shared = _prep_shared(inputs)
    bf = ml_dtypes.bfloat16
    embp = np.zeros((VOCAB, EPAD), bf)
    embp[:, :E] = np.asarray(inputs["emb"], np.float32).astype(bf)
    embp[0, 300] = 1.0   # pad-token indicator row
    embp[:, 301] = 1.0   # constant-1 bias row
    inputs = dict(inputs)
    inputs["_embp"] = embp
    in_maps = []
    for c in range(NC):
        m = dict(shared)
        m.update(_prep_core(inputs, c))
        in_maps.append(m)
    res = run_bass_kernel_spmd(nc, in_maps, list(range(NC)))
    outs = [np.asarray(res.results[c]["out"]) for c in range(NC)]
    se = np.concatenate([o[0:2 * BC].reshape(2, BC, P) for o in outs], axis=1)
    lsm = np.concatenate([o[2 * BC:].reshape(2, BC, P) for o in outs], axis=1)
    return (np.ascontiguousarray(se[0]), np.ascontiguousarray(se[1]),
            np.ascontiguousarray(lsm[0]), np.ascontiguousarray(lsm[1]))


# revision 15
# speedup vs baseline: 2.6406x; 1.0403x over previous
"""AttentionRNN (BiDAF-style QA reader) Trainium2 kernel.

Per core (pure data-parallel over batch, 4 of 32 rows per core):
  1. Host gathers embeddings in an s-major permuted token order and pads two
     extra embedding rows: row 300 = pad-token indicator (drives a +BIGM into
     the z gate via the weight matrix, freezing h at padded steps), row 301 =
     constant 1.0 (injects the gate biases).  So each scan round's x-gate
     pre-activations are plain contiguous-slice matmuls.
  2. GRU scans as chunked-parallel recurrences: chunks of S=16 payload steps
     with W=12 warmup steps re-run from h=0 (the GRU contracts ~0.6/step).
     Chunks whose warmup would cross t=0 are frozen (z pinned via +BIGM)
     until their true start.  One round = one time step of 72 chains; the
     x-projection matmuls for round k+1 are issued ahead of round k's
     recurrent matmuls so the PE stays busy during the serial chain.
  3. Decomposed BiDAF attention, softmax over Q, start/end heads, log-softmax
     over P; padded positions forced to exactly -1e7 as in the reference.
"""

import contextlib

import numpy as np
import ml_dtypes

import concourse.bass as bass
import concourse.mybir as mybir
from concourse.masks import make_identity
from concourse.tile import TileContext
from concourse.bass_utils import run_bass_kernel_spmd

F32 = mybir.dt.float32
BF16 = mybir.dt.bfloat16
U8 = mybir.dt.uint8
AX = mybir.AxisListType.X
ALU = mybir.AluOpType
AF = mybir.ActivationFunctionType

B, P, Q, E, H, VOCAB = 32, 512, 64, 300, 256, 50000
HH = 128
EPAD = 384
NC = 8
BC = B // NC
NEG = -1e7
BIGM = 1.0e4

S, W = 16, 12
RND = S + W                   # 28 rounds
NCHP, NCHQ = P // S, Q // S   # 32, 4
FDP, FDQ = NCHP * BC, NCHQ * BC   # 128, 16
OFF_PF, OFF_PB, OFF_QF, OFF_QB = 0, FDP, 2 * FDP, 2 * FDP + FDQ
FDT = 2 * FDP + 2 * FDQ       # 288

NTP, NTQ = BC * P, BC * Q     # 2048, 256

_CACHE = {}


def _mk_rank():
    order, seen = [], set()
    for k in range(RND):
        s = (k - W) % S
        for v in (s, S - 1 - s):
            if v not in seen:
                seen.add(v)
                order.append(v)
    rank = [0] * S
    for i, s in enumerate(order):
        rank[s] = i
    return order, rank


SORD, SRANK = _mk_rank()


def _round_geom(k):
    e = k - W
    s = e % S
    coff = (e - s) // S                      # -1 | 0
    cmin = (W - k + S - 1) // S if k < W else 0
    return s, coff, cmin


def _build_nc():
    nc = bass.Bass()

    epTp_d = nc.declare_dram_parameter("epTp_d", [128, 3 * NTP], BF16,
                                       isOutput=False)
    epTq_d = nc.declare_dram_parameter("epTq_d", [128, 3 * NTQ], BF16,
                                       isOutput=False)
    mq_d = nc.declare_dram_parameter("mq", [1, NTQ], F32, isOutput=False)
    m8 = nc.declare_dram_parameter("m8", [2 * BC, P], U8, isOutput=False)
    wihT = nc.declare_dram_parameter("wihT", [128, 4 * 3 * 3 * HH], BF16,
                                     isOutput=False)
    whhT = nc.declare_dram_parameter("whhT", [128, 4 * 3 * HH], BF16,
                                     isOutput=False)
    bhnr_d = nc.declare_dram_parameter("bhnr", [1, 4 * HH], BF16,
                                       isOutput=False)
    outw = nc.declare_dram_parameter("outw", [HH, 8], F32, isOutput=False)
    seww = nc.declare_dram_parameter("sew", [HH, 14], BF16, isOutput=False)
    out = nc.declare_dram_parameter("out", [4 * BC, P], F32, isOutput=True)

    es = contextlib.ExitStack()

    # ---------- Tile phases ----------
    with TileContext(nc) as tc:
        with tc.tile_pool(name="psA", bufs=2, space="PSUM") as psA, \
             tc.tile_pool(name="psB", bufs=2, space="PSUM") as psB, \
             tc.tile_pool(name="sbp", bufs=2) as sbp, \
             tc.tile_pool(name="pst", bufs=1) as pst:

            def pt(name, shape, dtype):
                return pst.tile(shape, dtype, name=name, tag=name)

            neg_sb = pt("neg_sb", [2 * BC, P], F32)
            ones_sb = pt("ones_sb", [1, 128], BF16)
            bigm_sb = pt("bigm_sb", [1, 128], BF16)

            # input tiles (DMA-streamed)
            epTp = pt("epTp", [128, 3, NTP], BF16)
            epTq = pt("epTq", [128, 3, NTQ], BF16)
            wih_sb = pt("wih_sb", [128, 4 * 3 * 3 * HH], BF16)
            whh_sb = pt("whh_sb", [128, 4 * 3 * HH], BF16)
            bhnr_sb = pt("bhnr_sb", [1, 4 * HH], BF16)
            mq_sb = pt("mq_sb", [1, NTQ], F32)
            m8_sb = pt("m8_sb", [2 * BC, P], U8)
            outw_sb = pt("outw_sb", [128, 8], F32)
            sew_sb = pt("sew_sb", [128, 14], BF16)

            # scan state
            pencFB = pt("pencFB", [128, 2 * NTP], BF16)
            qencFB = pt("qencFB", [128, 2 * NTQ], BF16)
            hcur = pt("hcur", [128, FDT], BF16)
            rz_sb = pt("rz_sb", [128, 2 * FDT], BF16)
            nh_sb = pt("nh_sb", [128, FDT], BF16)
            nx_sb = pt("nx_sb", [128, 2, FDT], BF16)
            t1_sb = pt("t1_sb", [128, FDT], BF16)
            t2_sb = pt("t2_sb", [128, FDT], BF16)
            n_sb = pt("n_sb", [128, FDT], BF16)
            d_sb = pt("d_sb", [128, FDT], BF16)
            e_sb = pt("e_sb", [128, FDT], BF16)
            ident_sb = pt("ident_sb", [128, 128], BF16)
            # attention tiles
            qenc3 = pt("qenc3", [128, 2 * NTQ], BF16)
            qwm = pt("qwm", [1, NTQ], BF16)
            qwt = pt("qwt", [1, NTQ], F32)
            probs = pt("probs", [128, 64 * 4 * BC], BF16)
            probsT = pt("probsT", [64, P * BC], BF16)
            qencT = pt("qencT", [64, 2 * HH * BC], BF16)
            attwFB = pt("attwFB", [128, 2 * NTP], BF16)
            pawFB = pt("pawFB", [128, 2 * NTP], BF16)
            se_sb = pt("se_sb", [2, BC * P], F32)
            se8 = pt("se8", [2 * BC, P], F32)
            lsm_sb = pt("lsm_sb", [2 * BC, P], F32)
            lse_sb = pt("lse_sb", [2 * BC, P], F32)
            red_sb = pt("red_sb", [2 * BC, 8], F32)

            # ---- input DMA (gpsimd queue: cheap issue), priority order ----
            g = nc.gpsimd
            HW12 = 4 * 3 * 3 * HH // 2
            g.dma_start(wih_sb[:, 0:HW12], wihT[:, 0:HW12])       # p dirs
            epv = epTp_d[:, :].rearrange("p (c t) -> p c t", c=3)
            g.dma_start(epTp[:, :, 0:4 * FDP], epv[:, :, 0:4 * FDP])
            g.dma_start(wih_sb[:, HW12:], wihT[:, HW12:])         # q dirs
            g.dma_start(epTq[:, :, :],
                        epTq_d[:, :].rearrange("p (c t) -> p c t", c=3))
            g.dma_start(whh_sb[:, :], whhT[:, :])
            g.dma_start(bhnr_sb[:, :], bhnr_d[:, :])
            for r0, r1 in ((4, 6), (6, 8), (8, 16)):
                g.dma_start(epTp[:, :, r0 * FDP:r1 * FDP],
                            epv[:, :, r0 * FDP:r1 * FDP])
            g.dma_start(mq_sb[:, :], mq_d[:, :])
            g.dma_start(m8_sb[:, :], m8[:, :])
            g.dma_start(outw_sb[0:HH, :], outw[:, :])
            g.dma_start(sew_sb[0:HH, :], seww[:, :])

            nc.vector.memset(ones_sb[:, :], 1.0)
            nc.vector.memset(bigm_sb[:, :], BIGM)
            nc.vector.memset(hcur[:, :], 0)
            nc.vector.memset(neg_sb[:, :], NEG)
            make_identity(nc, ident_sb[:, :])

            # round psum tiles: T1 holds r (bank0) | z (bank1),
            #                   T2 holds nx (bank0) | nh (bank1)
            tiles = [None] * RND

            def alloc_round(j):
                tiles[j] = (psA.tile([128, 1024], F32, name="T1", tag="t1"),
                            psB.tile([128, 1024], F32, name="T2", tag="t2"))

            # x-projection geometry for round j: per dir, the contiguous
            # epT column range and psum dst range.
            def xgeom(j):
                s, coff, cmin = _round_geom(j)
                res = []
                for (nch, fd, offF, offB, epT, blk) in (
                        (NCHP, FDP, OFF_PF, OFF_PB, epTp, FDP),
                        (NCHQ, FDQ, OFF_QF, OFF_QB, epTq, FDQ)):
                    cnt = nch - cmin
                    # forward: chains [cmin, nch) read chunk c+coff at pos s
                    res.append((epT,
                                SRANK[s] * blk + (cmin + coff) * BC,
                                offF + cmin * BC, cnt * BC))
                    # backward: chains [0, cnt) read chunk c-coff at pos S-1-s
                    res.append((epT,
                                SRANK[S - 1 - s] * blk + (-coff) * BC,
                                offB, cnt * BC))
                return res

            def emit_wih(j):
                T1, T2 = tiles[j]
                s, coff, cmin = _round_geom(j)
                geo = xgeom(j)
                for gate, dst, goff in ((0, T1, 0), (1, T1, 512), (2, T2, 0)):
                    first = True
                    for di in range(4):
                        epT, c0, o0, wd = geo[di]
                        for kc in range(3):
                            wcol = ((di * 3 + kc) * 3 + gate) * HH
                            # the nx group (gate 2) has no whh part, so its
                            # last matmul closes the accumulation group
                            last = gate == 2 and di == 3 and kc == 2
                            nc.tensor.matmul(
                                dst[:, goff + o0:goff + o0 + wd],
                                wih_sb[:, wcol:wcol + HH],
                                epT[:, kc, c0:c0 + wd],
                                start=first, stop=last)
                            first = False
                    if gate == 1 and cmin > 0:
                        # freeze warmup-frozen chains: z += BIGM
                        fz = cmin * BC
                        for lo in (OFF_PF, OFF_PB + FDP - fz,
                                   OFF_QF, OFF_QB + FDQ - fz):
                            nc.tensor.matmul(
                                dst[:, 512 + lo:512 + lo + fz],
                                bigm_sb[0:1, :], ones_sb[0:1, 0:fz],
                                start=False, stop=False)
                # nh group: bhh_n broadcast rows
                for di, (off, fd) in enumerate(((OFF_PF, FDP), (OFF_PB, FDP),
                                                (OFF_QF, FDQ), (OFF_QB, FDQ))):
                    nc.tensor.matmul(T2[:, 512 + off:512 + off + fd],
                                     bhnr_sb[0:1, di * HH:(di + 1) * HH],
                                     ones_sb[0:1, 0:fd],
                                     start=(di == 0), stop=False)

            def emit_whh(j):
                T1, T2 = tiles[j]
                DIRS = ((0, OFF_PF, FDP), (1, OFF_PB, FDP),
                        (2, OFF_QF, FDQ), (3, OFF_QB, FDQ))
                for gi, dst, goff in ((0, T1, 0), (1, T1, 512), (2, T2, 512)):
                    for di, (d, off, fd) in enumerate(DIRS):
                        nc.tensor.matmul(
                            dst[:, goff + off:goff + off + fd],
                            whh_sb[:, (d * 3 + gi) * HH:(d * 3 + gi + 1) * HH],
                            hcur[:, off:off + fd],
                            start=False, stop=(di == 3))

            # payload output views (b-major column layout: b*T + c*S + s)
            def view4(x, base, ntok, nch):
                return x[:, base:base + ntok].rearrange(
                    "p (b c s) -> p c b s", b=BC, c=nch, s=S)

            pv_f = view4(pencFB, 0, NTP, NCHP)
            pv_b = view4(pencFB, NTP, NTP, NCHP)
            qv_f = view4(qencFB, 0, NTQ, NCHQ)
            qv_b = view4(qencFB, NTQ, NTQ, NCHQ)

            # ---- the scan ----
            alloc_round(0)
            emit_wih(0)
            alloc_round(1)
            emit_wih(1)
            nc.scalar.activation(nx_sb[:, 0, :], tiles[0][1][:, 0:FDT],
                                 AF.Copy)
            for k in range(RND):
                T1, T2 = tiles[k]
                s, coff, cmin = _round_geom(k)
                emit_whh(k)
                # nh evacuation (psum -> sbuf bf16), overlaps sigmoid
                nc.vector.tensor_scalar_add(nh_sb[:, :], T2[:, 512:512 + FDT],
                                            0.0)
                # r sigmoid on the critical path; z sigmoid off it
                nc.scalar.activation(rz_sb[:, 0:FDT], T1[:, 0:FDT],
                                     AF.Sigmoid)
                nc.scalar.activation(rz_sb[:, FDT:2 * FDT],
                                     T1[:, 512:512 + FDT], AF.Sigmoid)
                nc.vector.tensor_mul(t1_sb[:, :], rz_sb[:, 0:FDT],
                                     nh_sb[:, :])
                nc.vector.tensor_add(t2_sb[:, :], t1_sb[:, :],
                                     nx_sb[:, k % 2, :])
                nc.scalar.activation(n_sb[:, :], t2_sb[:, :], AF.Tanh)
                # h' = n*(1-z) + z*h with (1-z) and z*h computed during the
                # tanh window so only two ops remain after it
                nc.vector.tensor_scalar(out=d_sb[:, :],
                                        in0=rz_sb[:, FDT:2 * FDT],
                                        scalar1=-1.0, scalar2=1.0,
                                        op0=ALU.mult, op1=ALU.add)
                nc.vector.tensor_mul(e_sb[:, :], rz_sb[:, FDT:2 * FDT],
                                     hcur[:, :])
                nc.vector.tensor_mul(t1_sb[:, :], n_sb[:, :], d_sb[:, :])
                nc.vector.tensor_add(hcur[:, :], t1_sb[:, :], e_sb[:, :])
                if k + 2 < RND:
                    alloc_round(k + 2)
                    emit_wih(k + 2)
                if k + 1 < RND:
                    nc.scalar.activation(nx_sb[:, (k + 1) % 2, :],
                                         tiles[k + 1][1][:, 0:FDT], AF.Copy)
                if k >= W:
                    nc.gpsimd.tensor_copy(pv_f[:, :, :, s],
                                          hcur[:, OFF_PF:OFF_PF + FDP]
                                          .rearrange("p (c b) -> p c b", b=BC))
                    nc.gpsimd.tensor_copy(pv_b[:, :, :, S - 1 - s],
                                          hcur[:, OFF_PB:OFF_PB + FDP]
                                          .rearrange("p (c b) -> p c b", b=BC))
                    nc.gpsimd.tensor_copy(qv_f[:, :, :, s],
                                          hcur[:, OFF_QF:OFF_QF + FDQ]
                                          .rearrange("p (c b) -> p c b", b=BC))
                    nc.gpsimd.tensor_copy(qv_b[:, :, :, S - 1 - s],
                                          hcur[:, OFF_QB:OFF_QB + FDQ]
                                          .rearrange("p (c b) -> p c b", b=BC))

            # ---- attention ----
            nc.vector.tensor_scalar_mul(qenc3[:, 0:NTQ], qencFB[:, 0:NTQ],
                                        outw_sb[:, 4:5])
            nc.vector.tensor_scalar_mul(qenc3[:, NTQ:2 * NTQ],
                                        qencFB[:, NTQ:2 * NTQ],
                                        outw_sb[:, 5:6])
            pqw = psB.tile([1, 512], F32, name="pqw", tag="t2")
            nc.tensor.matmul(pqw[0:1, 0:NTQ], sew_sb[:, 12:13],
                             qencFB[:, 0:NTQ], start=True, stop=False)
            nc.tensor.matmul(pqw[0:1, 0:NTQ], sew_sb[:, 13:14],
                             qencFB[:, NTQ:2 * NTQ], start=False, stop=True)
            nc.vector.tensor_scalar_add(qwt[0:1, :], pqw[0:1, 0:NTQ],
                                        outw_sb[0:1, 2:3])
            nc.vector.scalar_tensor_tensor(
                qwm[0:1, :], mq_sb[0:1, :], NEG, qwt[0:1, :],
                op0=ALU.mult, op1=ALU.add)

            # logits for all 16 (b, tcn) blocks into one psum tile; the
            # qwm mask is -1e7 at padded q so exp underflows to exactly 0 --
            # no max-subtraction needed (logits are O(10) bounded).
            plg = psA.tile([128, 1024], F32, name="plg", tag="t1")
            for b in range(BC):
                for tcn in range(4):
                    t0 = b * P + tcn * 128
                    o = (b * 4 + tcn) * 64
                    nc.tensor.matmul(plg[:, o:o + 64],
                                     pencFB[:, t0:t0 + 128],
                                     qenc3[:, b * Q:(b + 1) * Q],
                                     start=True, stop=False)
                    nc.tensor.matmul(plg[:, o:o + 64],
                                     pencFB[:, NTP + t0:NTP + t0 + 128],
                                     qenc3[:, NTQ + b * Q:NTQ + (b + 1) * Q],
                                     start=False, stop=False)
                    nc.tensor.matmul(plg[:, o:o + 64], ones_sb[0:1, :],
                                     qwm[0:1, b * Q:(b + 1) * Q],
                                     start=False, stop=True)
            exu = pt("exu", [128, 1024], BF16)
            nc.scalar.activation(exu[:, :], plg[:, :], AF.Exp)
            sm16 = pt("sm16", [128, 16], F32)
            nc.vector.tensor_reduce(
                sm16[:, :], exu[:, :].rearrange("p (n q) -> p n q", n=16),
                AX, ALU.add)
            rs16 = pt("rs16", [128, 16], F32)
            nc.vector.reciprocal(rs16[:, :], sm16[:, :])
            for j in range(16):
                nc.vector.tensor_scalar_mul(
                    probs[:, j * 64:(j + 1) * 64],
                    exu[:, j * 64:(j + 1) * 64], rs16[:, j:j + 1])

            for b in range(BC):
                ptb = psB.tile([128, 512], BF16, name="ptb", tag="t2")
                for tcn in range(4):
                    nc.tensor.transpose(
                        ptb[0:64, tcn * 128:(tcn + 1) * 128],
                        probs[:, (b * 4 + tcn) * 64:(b * 4 + tcn + 1) * 64],
                        ident_sb[:, :])
                nc.scalar.activation(probsT[:, b * P:(b + 1) * P],
                                     ptb[0:64, :], AF.Copy)
            for hc2 in range(2):
                ptq = psB.tile([128, 512], BF16, name="ptq", tag="t2")
                for i in range(4):
                    b, hc = (hc2 * 4 + i) // 2, (hc2 * 4 + i) % 2
                    nc.tensor.transpose(
                        ptq[0:64, i * 128:(i + 1) * 128],
                        qencFB[:, hc * NTQ + b * Q:hc * NTQ + (b + 1) * Q],
                        ident_sb[:, :])
                nc.vector.tensor_scalar_add(
                    qencT[:, hc2 * 512:(hc2 + 1) * 512], ptq[0:64, :], 0.0)

            for b in range(BC):
                for hc in range(2):
                    paw = psA.tile([128, 1024], F32, name="paw", tag="t1")
                    nc.tensor.matmul(
                        paw[:, 0:P],
                        qencT[0:64, (b * 2 + hc) * 128:(b * 2 + hc + 1) * 128],
                        probsT[0:64, b * P:(b + 1) * P], start=True, stop=True)
                    dst = attwFB[:, hc * NTP + b * P:hc * NTP + (b + 1) * P]
                    if (b + hc) % 2 == 0:
                        nc.scalar.activation(dst, paw[:, 0:P], AF.Copy)
                    else:
                        nc.vector.tensor_scalar_add(dst, paw[:, 0:P], 0.0)
            nc.vector.tensor_mul(pawFB[:, 0:NTP], pencFB[:, 0:NTP],
                                 attwFB[:, 0:NTP])
            nc.vector.tensor_mul(pawFB[:, NTP:], pencFB[:, NTP:],
                                 attwFB[:, NTP:])

            for bp in range(2):
                pse = psB.tile([2, 1024], F32, name="pse", tag="t2")
                for bi in range(2):
                    b = bp * 2 + bi
                    rhss = (pencFB[:, b * P:(b + 1) * P],
                            pencFB[:, NTP + b * P:NTP + (b + 1) * P],
                            attwFB[:, b * P:(b + 1) * P],
                            attwFB[:, NTP + b * P:NTP + (b + 1) * P],
                            pawFB[:, b * P:(b + 1) * P],
                            pawFB[:, NTP + b * P:NTP + (b + 1) * P])
                    for j, rhs in enumerate(rhss):
                        nc.tensor.matmul(
                            pse[0:2, bi * P:(bi + 1) * P],
                            sew_sb[:, 2 * j:2 * j + 2],
                            rhs, start=(j == 0), stop=(j == 5))
                nc.scalar.activation(se_sb[0:2, bp * 2 * P:(bp * 2 + 2) * P],
                                     pse[0:2, :],
                                     AF.Identity, bias=outw_sb[0:2, 3:4])
            nc.gpsimd.dma_start(se8[0:BC, :], se_sb[0:1, :])
            nc.gpsimd.dma_start(se8[BC:2 * BC, :], se_sb[1:2, :])
            nc.vector.copy_predicated(se8[:, :], m8_sb[:, :], neg_sb[:, :])

            # log-softmax without max-subtraction: valid entries are O(10),
            # -1e7 pads underflow exp to 0
            nc.scalar.activation(lse_sb[:, :], se8[:, :], AF.Exp)
            nc.vector.tensor_reduce(red_sb[:, 2:3], lse_sb[:, :], AX, ALU.add)
            nc.scalar.activation(red_sb[:, 3:4], red_sb[:, 2:3], AF.Ln)
            nc.vector.tensor_scalar(out=lsm_sb[:, :], in0=se8[:, :],
                                    scalar1=red_sb[:, 3:4], scalar2=None,
                                    op0=ALU.subtract)

            nc.sync.dma_start(out[0:2 * BC, :], se8[:, :])
            nc.sync.dma_start(out[2 * BC:4 * BC, :], lsm_sb[:, :])

    _split_multiwaits(nc)
    return nc, es


def _split_multiwaits(nc):
    """HW instruction encodings hold a single semaphore wait; move extra
    waits emitted by Tile onto same-engine NOPs inserted just before."""
    for b in nc.main_func.blocks:
        il = b.instructions
        newlist = []
        for inst in il:
            if type(inst).__name__ == "InstISA":
                # EVENT_SEMAPHORE_RANGE_CLEAR mis-encodes for this walrus
                # build; NRT clears semaphores per execution anyway.
                continue
            si = inst.sync_info
            if si is not None and len(si.on_wait) > 1:
                waits = list(si.on_wait)
                for wx in waits[:-1]:
                    nop = nc.engines[inst.engine].nop(hint="wsplit").ins
                    # remove from wherever nop() appended it
                    for bb in nc.main_func.blocks:
                        try:
                            bb.instructions.remove(nop)
                            break
                        except ValueError:
                            pass
                    nop.sync_info = mybir.SyncInfo(on_wait=[wx], on_update=[])
                    newlist.append(nop)
                inst.sync_info = mybir.SyncInfo(on_wait=[waits[-1]],
                                                on_update=list(si.on_update))
            newlist.append(inst)
        il[:] = newlist


def _perm_tokens(tok2d, nch, blk):
    """Token array (BC, T) -> s-major column order: col = rank(s)*blk + c*BC + b."""
    T = tok2d.shape[1]
    cols = np.empty(BC * T, np.int64)
    for rank in range(S):
        s = SORD[rank]
        blkv = tok2d[:, s::S]          # (BC, nch) tokens at pos s per chunk
        # col index rank*blk + c*BC + b
        cols[rank * blk:(rank + 1) * blk] = blkv.T.reshape(-1)
    return cols


def _prep_core(inputs, c):
    bs = slice(c * BC, (c + 1) * BC)
    ptok = np.asarray(inputs["passage"][bs]).astype(np.int64)
    qtok = np.asarray(inputs["question"][bs]).astype(np.int64)
    embp = inputs["_embp"]
    pcols = _perm_tokens(ptok, NCHP, FDP)
    qcols = _perm_tokens(qtok, NCHQ, FDQ)
    d = {}
    d["epTp_d"] = np.ascontiguousarray(
        embp[pcols].T.reshape(3, 128, NTP).transpose(1, 0, 2).reshape(128, -1))
    d["epTq_d"] = np.ascontiguousarray(
        embp[qcols].T.reshape(3, 128, NTQ).transpose(1, 0, 2).reshape(128, -1))
    qm0 = (qtok.reshape(-1) == 0).astype(np.float32)
    d["mq"] = np.ascontiguousarray(qm0[None, :])
    pm2 = (ptok.reshape(-1) == 0).reshape(BC, P).astype(np.uint8)
    d["m8"] = np.ascontiguousarray(np.concatenate([pm2, pm2], axis=0))
    return d


def _prep_shared(inputs):
    bf = ml_dtypes.bfloat16

    wihT = np.zeros((4, 3, 128, 3 * HH), bf)      # (d, kc, p, m)
    whhT = np.zeros((4, HH, 3 * HH), bf)          # (d, p, m)
    bhnr = np.zeros((4, HH), bf)
    for di, (pre, dd) in enumerate((("p", "f"), ("p", "b"),
                                    ("q", "f"), ("q", "b"))):
        wih = np.asarray(inputs[f"{pre}_wih_{dd}"], np.float32)
        whh = np.asarray(inputs[f"{pre}_whh_{dd}"], np.float32)
        bih = np.asarray(inputs[f"{pre}_bih_{dd}"], np.float32)
        bhh = np.asarray(inputs[f"{pre}_bhh_{dd}"], np.float32)
        wT = np.zeros((EPAD, 3 * HH), np.float32)
        wT[:E, :] = wih.T
        # row 300: pad-token indicator -> +BIGM on the z gate
        wT[300, HH:2 * HH] = BIGM
        # row 301: constant-1 -> gate biases (bih+bhh for r/z, bih for n)
        wT[301, 0:HH] = bih[0:HH] + bhh[0:HH]
        wT[301, HH:2 * HH] = bih[HH:2 * HH] + bhh[HH:2 * HH]
        wT[301, 2 * HH:] = bih[2 * HH:]
        wihT[di] = wT.astype(bf).reshape(3, 128, 3 * HH)
        whhT[di] = whh.T.astype(bf)
        bhnr[di] = bhh[2 * HH:].astype(bf)
    wihT = np.ascontiguousarray(
        wihT.transpose(2, 0, 1, 3).reshape(128, -1))      # (p,(d,kc,m))
    whhT = np.ascontiguousarray(
        whhT.transpose(1, 0, 2).reshape(128, -1))         # (p,(d,m))
    bhnr = np.ascontiguousarray(bhnr.reshape(1, -1))

    aw = np.asarray(inputs["attn_w"], np.float32)
    w1, w2, w3 = aw[:256], aw[256:512], aw[512:]
    outw = np.zeros((HH, 8), np.float32)
    outw[:, 4], outw[:, 5] = w3[:128], w3[128:]
    outw[0, 2] = float(np.asarray(inputs["attn_b"]))
    outw[0, 3] = float(np.asarray(inputs["start_b"]))
    outw[1, 3] = float(np.asarray(inputs["end_b"]))

    sw = np.asarray(inputs["start_w"], np.float32)
    ew = np.asarray(inputs["end_w"], np.float32)
    sew = np.zeros((HH, 14), bf)
    for j in range(6):
        sew[:, 2 * j] = sw[j * 128:(j + 1) * 128].astype(bf)
        sew[:, 2 * j + 1] = ew[j * 128:(j + 1) * 128].astype(bf)
    sew[:, 12] = w2[:128].astype(bf)
    sew[:, 13] = w2[128:].astype(bf)
    return {"wihT": wihT, "whhT": whhT, "bhnr": bhnr,
            "outw": outw, "sew": sew}


def kernel(**inputs):
    if "nc" not in _CACHE:
        _CACHE["nc"] = _build_nc()
    nc, _es = _CACHE["nc"]
    shared = _prep_shared(inputs)
    bf = ml_dtypes.bfloat16
    embp = np.zeros((VOCAB, EPAD), bf)
    embp[:, :E] = np.asarray(inputs["emb"], np.float32).astype(bf)
    embp[0, 300] = 1.0   # pad-token indicator row
    embp[:, 301] = 1.0   # constant-1 bias row
    inputs = dict(inputs)
    inputs["_embp"] = embp
    in_maps = []
    for c in range(NC):
        m = dict(shared)
        m.update(_prep_core(inputs, c))
        in_maps.append(m)
    res = run_bass_kernel_spmd(nc, in_maps, list(range(NC)))
    outs = [np.asarray(res.results[c]["out"]) for c in range(NC)]
    se = np.concatenate([o[0:2 * BC].reshape(2, BC, P) for o in outs], axis=1)
    lsm = np.concatenate([o[2 * BC:].reshape(2, BC, P) for o in outs], axis=1)
    return (np.ascontiguousarray(se[0]), np.ascontiguousarray(se[1]),
            np.ascontiguousarray(lsm[0]), np.ascontiguousarray(lsm[1]))


# revision 17
# speedup vs baseline: 2.6845x; 1.0166x over previous
"""AttentionRNN (BiDAF-style QA reader) Trainium2 kernel.

Per core (pure data-parallel over batch, 4 of 32 rows per core):
  1. Host gathers embeddings in an s-major permuted token order and pads two
     extra embedding rows: row 300 = pad-token indicator (drives a +BIGM into
     the z gate via the weight matrix, freezing h at padded steps), row 301 =
     constant 1.0 (injects the gate biases).  So each scan round's x-gate
     pre-activations are plain contiguous-slice matmuls.
  2. GRU scans as chunked-parallel recurrences: chunks of S=16 payload steps
     with W=12 warmup steps re-run from h=0 (the GRU contracts ~0.6/step).
     Chunks whose warmup would cross t=0 are frozen (z pinned via +BIGM)
     until their true start.  One round = one time step of 72 chains; the
     x-projection matmuls for round k+1 are issued ahead of round k's
     recurrent matmuls so the PE stays busy during the serial chain.
  3. Decomposed BiDAF attention, softmax over Q, start/end heads, log-softmax
     over P; padded positions forced to exactly -1e7 as in the reference.
"""

import contextlib

import numpy as np
import ml_dtypes

import concourse.bass as bass
import concourse.mybir as mybir
from concourse.masks import make_identity
from concourse.tile import TileContext
from concourse.bass_utils import run_bass_kernel_spmd

F32 = mybir.dt.float32
BF16 = mybir.dt.bfloat16
U8 = mybir.dt.uint8
AX = mybir.AxisListType.X
ALU = mybir.AluOpType
AF = mybir.ActivationFunctionType

B, P, Q, E, H, VOCAB = 32, 512, 64, 300, 256, 50000
HH = 128
EPAD = 384
NC = 8
BC = B // NC
NEG = -1e7
BIGM = 1.0e4

S, W = 16, 12
RND = S + W                   # 28 rounds
NCHP, NCHQ = P // S, Q // S   # 32, 4
FDP, FDQ = NCHP * BC, NCHQ * BC   # 128, 16
OFF_PF, OFF_PB, OFF_QF, OFF_QB = 0, FDP, 2 * FDP, 2 * FDP + FDQ
FDT = 2 * FDP + 2 * FDQ       # 288

NTP, NTQ = BC * P, BC * Q     # 2048, 256

_CACHE = {}


def _mk_rank():
    order, seen = [], set()
    for k in range(RND):
        s = (k - W) % S
        for v in (s, S - 1 - s):
            if v not in seen:
                seen.add(v)
                order.append(v)
    rank = [0] * S
    for i, s in enumerate(order):
        rank[s] = i
    return order, rank


SORD, SRANK = _mk_rank()


def _round_geom(k):
    e = k - W
    s = e % S
    coff = (e - s) // S                      # -1 | 0
    cmin = (W - k + S - 1) // S if k < W else 0
    return s, coff, cmin


def _build_nc():
    nc = bass.Bass()

    epTp_d = nc.declare_dram_parameter("epTp_d", [128, 3 * NTP], BF16,
                                       isOutput=False)
    epTq_d = nc.declare_dram_parameter("epTq_d", [128, 3 * NTQ], BF16,
                                       isOutput=False)
    mq_d = nc.declare_dram_parameter("mq", [1, NTQ], F32, isOutput=False)
    m8 = nc.declare_dram_parameter("m8", [2 * BC, P], U8, isOutput=False)
    wihT = nc.declare_dram_parameter("wihT", [128, 4 * 3 * 3 * HH], BF16,
                                     isOutput=False)
    whhT = nc.declare_dram_parameter("whhT", [128, 4 * 3 * HH], BF16,
                                     isOutput=False)
    bhnr_d = nc.declare_dram_parameter("bhnr", [1, 4 * HH], BF16,
                                       isOutput=False)
    outw = nc.declare_dram_parameter("outw", [HH, 8], F32, isOutput=False)
    seww = nc.declare_dram_parameter("sew", [HH, 14], BF16, isOutput=False)
    out = nc.declare_dram_parameter("out", [4 * BC, P], F32, isOutput=True)

    es = contextlib.ExitStack()

    # ---------- Tile phases ----------
    with TileContext(nc) as tc:
        with tc.tile_pool(name="psA", bufs=2, space="PSUM") as psA, \
             tc.tile_pool(name="psB", bufs=2, space="PSUM") as psB, \
             tc.tile_pool(name="sbp", bufs=2) as sbp, \
             tc.tile_pool(name="pst", bufs=1) as pst:

            def pt(name, shape, dtype):
                return pst.tile(shape, dtype, name=name, tag=name)

            neg_sb = pt("neg_sb", [2 * BC, P], F32)
            ones_sb = pt("ones_sb", [1, 128], BF16)
            bigm_sb = pt("bigm_sb", [1, 128], BF16)

            # input tiles (DMA-streamed)
            epTp = pt("epTp", [128, 3, NTP], BF16)
            epTq = pt("epTq", [128, 3, NTQ], BF16)
            wih_sb = pt("wih_sb", [128, 4 * 3 * 3 * HH], BF16)
            whh_sb = pt("whh_sb", [128, 4 * 3 * HH], BF16)
            bhnr_sb = pt("bhnr_sb", [1, 4 * HH], BF16)
            mq_sb = pt("mq_sb", [1, NTQ], F32)
            m8_sb = pt("m8_sb", [2 * BC, P], U8)
            outw_sb = pt("outw_sb", [128, 8], F32)
            sew_sb = pt("sew_sb", [128, 14], BF16)

            # scan state
            pencFB = pt("pencFB", [128, 2 * NTP], BF16)
            qencFB = pt("qencFB", [128, 2 * NTQ], BF16)
            hcur = pt("hcur", [128, FDT], BF16)
            rz_sb = pt("rz_sb", [128, 2 * FDT], BF16)
            nh_sb = pt("nh_sb", [128, FDT], BF16)
            nx_sb = pt("nx_sb", [128, 2, FDT], BF16)
            t1_sb = pt("t1_sb", [128, FDT], BF16)
            t2_sb = pt("t2_sb", [128, FDT], BF16)
            n_sb = pt("n_sb", [128, FDT], BF16)
            d_sb = pt("d_sb", [128, FDT], BF16)
            e_sb = pt("e_sb", [128, FDT], BF16)
            ident_sb = pt("ident_sb", [128, 128], BF16)
            # attention tiles
            qenc3 = pt("qenc3", [128, 2 * NTQ], BF16)
            qwm = pt("qwm", [1, NTQ], BF16)
            qwt = pt("qwt", [1, NTQ], F32)
            probs = pt("probs", [128, 64 * 4 * BC], BF16)
            probsT = pt("probsT", [64, P * BC], BF16)
            qencT = pt("qencT", [64, 2 * HH * BC], BF16)
            attwFB = pt("attwFB", [128, 2 * NTP], BF16)
            pawFB = pt("pawFB", [128, 2 * NTP], BF16)
            se_sb = pt("se_sb", [2, BC * P], F32)
            se8 = pt("se8", [2 * BC, P], F32)
            lsm_sb = pt("lsm_sb", [2 * BC, P], F32)
            lse_sb = pt("lse_sb", [2 * BC, P], F32)
            red_sb = pt("red_sb", [2 * BC, 8], F32)

            # ---- input DMA (gpsimd queue: cheap issue), priority order ----
            g = nc.gpsimd
            HW12 = 4 * 3 * 3 * HH // 2
            g.dma_start(wih_sb[:, 0:HW12], wihT[:, 0:HW12])       # p dirs
            epv = epTp_d[:, :].rearrange("p (c t) -> p c t", c=3)
            g.dma_start(epTp[:, :, 0:4 * FDP], epv[:, :, 0:4 * FDP])
            g.dma_start(wih_sb[:, HW12:], wihT[:, HW12:])         # q dirs
            g.dma_start(epTq[:, :, :],
                        epTq_d[:, :].rearrange("p (c t) -> p c t", c=3))
            g.dma_start(whh_sb[:, :], whhT[:, :])
            g.dma_start(bhnr_sb[:, :], bhnr_d[:, :])
            for r0, r1 in ((4, 6), (6, 8), (8, 16)):
                g.dma_start(epTp[:, :, r0 * FDP:r1 * FDP],
                            epv[:, :, r0 * FDP:r1 * FDP])
            g.dma_start(mq_sb[:, :], mq_d[:, :])
            g.dma_start(m8_sb[:, :], m8[:, :])
            g.dma_start(outw_sb[0:HH, :], outw[:, :])
            g.dma_start(sew_sb[0:HH, :], seww[:, :])

            nc.vector.memset(ones_sb[:, :], 1.0)
            nc.vector.memset(bigm_sb[:, :], BIGM)
            nc.vector.memset(hcur[:, :], 0)
            nc.vector.memset(neg_sb[:, :], NEG)
            make_identity(nc, ident_sb[:, :])

            # round psum tiles: T1 holds r (bank0) | z (bank1),
            #                   T2 holds nx (bank0) | nh (bank1)
            tiles = [None] * RND

            def alloc_round(j):
                tiles[j] = (psA.tile([128, 1024], F32, name="T1", tag="t1"),
                            psB.tile([128, 1024], F32, name="T2", tag="t2"))

            # x-projection geometry for round j: per dir, the contiguous
            # epT column range and psum dst range.
            def xgeom(j):
                s, coff, cmin = _round_geom(j)
                res = []
                for (nch, fd, offF, offB, epT, blk) in (
                        (NCHP, FDP, OFF_PF, OFF_PB, epTp, FDP),
                        (NCHQ, FDQ, OFF_QF, OFF_QB, epTq, FDQ)):
                    cnt = nch - cmin
                    # forward: chains [cmin, nch) read chunk c+coff at pos s
                    res.append((epT,
                                SRANK[s] * blk + (cmin + coff) * BC,
                                offF + cmin * BC, cnt * BC))
                    # backward: chains [0, cnt) read chunk c-coff at pos S-1-s
                    res.append((epT,
                                SRANK[S - 1 - s] * blk + (-coff) * BC,
                                offB, cnt * BC))
                return res

            def emit_wih(j):
                T1, T2 = tiles[j]
                s, coff, cmin = _round_geom(j)
                geo = xgeom(j)
                for gate, dst, goff in ((0, T1, 0), (1, T1, 512), (2, T2, 0)):
                    first = True
                    for di in range(4):
                        epT, c0, o0, wd = geo[di]
                        for kc in range(3):
                            wcol = ((di * 3 + kc) * 3 + gate) * HH
                            # the nx group (gate 2) has no whh part, so its
                            # last matmul closes the accumulation group
                            last = gate == 2 and di == 3 and kc == 2
                            nc.tensor.matmul(
                                dst[:, goff + o0:goff + o0 + wd],
                                wih_sb[:, wcol:wcol + HH],
                                epT[:, kc, c0:c0 + wd],
                                start=first, stop=last)
                            first = False
                    if gate == 1 and cmin > 0:
                        # freeze warmup-frozen chains: z += BIGM
                        fz = cmin * BC
                        for lo in (OFF_PF, OFF_PB + FDP - fz,
                                   OFF_QF, OFF_QB + FDQ - fz):
                            nc.tensor.matmul(
                                dst[:, 512 + lo:512 + lo + fz],
                                bigm_sb[0:1, :], ones_sb[0:1, 0:fz],
                                start=False, stop=False)
                # nh group: bhh_n broadcast rows
                for di, (off, fd) in enumerate(((OFF_PF, FDP), (OFF_PB, FDP),
                                                (OFF_QF, FDQ), (OFF_QB, FDQ))):
                    nc.tensor.matmul(T2[:, 512 + off:512 + off + fd],
                                     bhnr_sb[0:1, di * HH:(di + 1) * HH],
                                     ones_sb[0:1, 0:fd],
                                     start=(di == 0), stop=False)

            def emit_whh(j):
                T1, T2 = tiles[j]
                DIRS = ((0, OFF_PF, FDP), (1, OFF_PB, FDP),
                        (2, OFF_QF, FDQ), (3, OFF_QB, FDQ))
                for gi, dst, goff in ((0, T1, 0), (1, T1, 512), (2, T2, 512)):
                    for di, (d, off, fd) in enumerate(DIRS):
                        nc.tensor.matmul(
                            dst[:, goff + off:goff + off + fd],
                            whh_sb[:, (d * 3 + gi) * HH:(d * 3 + gi + 1) * HH],
                            hcur[:, off:off + fd],
                            start=False, stop=(di == 3))

            # payload output views (b-major column layout: b*T + c*S + s)
            def view4(x, base, ntok, nch):
                return x[:, base:base + ntok].rearrange(
                    "p (b c s) -> p c b s", b=BC, c=nch, s=S)

            pv_f = view4(pencFB, 0, NTP, NCHP)
            pv_b = view4(pencFB, NTP, NTP, NCHP)
            qv_f = view4(qencFB, 0, NTQ, NCHQ)
            qv_b = view4(qencFB, NTQ, NTQ, NCHQ)

            # ---- the scan ----
            alloc_round(0)
            emit_wih(0)
            alloc_round(1)
            emit_wih(1)
            nc.scalar.activation(nx_sb[:, 0, :], tiles[0][1][:, 0:FDT],
                                 AF.Copy)
            for k in range(RND):
                T1, T2 = tiles[k]
                s, coff, cmin = _round_geom(k)
                emit_whh(k)
                # nh evacuation (psum -> sbuf bf16), overlaps sigmoid
                nc.vector.tensor_scalar_add(nh_sb[:, :], T2[:, 512:512 + FDT],
                                            0.0)
                # r sigmoid on the critical path; z sigmoid off it
                nc.scalar.activation(rz_sb[:, 0:FDT], T1[:, 0:FDT],
                                     AF.Sigmoid)
                nc.scalar.activation(rz_sb[:, FDT:2 * FDT],
                                     T1[:, 512:512 + FDT], AF.Sigmoid)
                nc.vector.tensor_mul(t1_sb[:, :], rz_sb[:, 0:FDT],
                                     nh_sb[:, :])
                nc.vector.tensor_add(t2_sb[:, :], t1_sb[:, :],
                                     nx_sb[:, k % 2, :])
                nc.scalar.activation(n_sb[:, :], t2_sb[:, :], AF.Tanh)
                # h' = n*(1-z) + z*h with (1-z) and z*h computed during the
                # tanh window so only two ops remain after it
                nc.vector.tensor_scalar(out=d_sb[:, :],
                                        in0=rz_sb[:, FDT:2 * FDT],
                                        scalar1=-1.0, scalar2=1.0,
                                        op0=ALU.mult, op1=ALU.add)
                nc.vector.tensor_mul(e_sb[:, :], rz_sb[:, FDT:2 * FDT],
                                     hcur[:, :])
                nc.vector.tensor_mul(t1_sb[:, :], n_sb[:, :], d_sb[:, :])
                nc.vector.tensor_add(hcur[:, :], t1_sb[:, :], e_sb[:, :])
                if k + 2 < RND:
                    alloc_round(k + 2)
                    emit_wih(k + 2)
                if k + 1 < RND:
                    nc.scalar.activation(nx_sb[:, (k + 1) % 2, :],
                                         tiles[k + 1][1][:, 0:FDT], AF.Copy)
                if k >= W:
                    nc.gpsimd.tensor_copy(pv_f[:, :, :, s],
                                          hcur[:, OFF_PF:OFF_PF + FDP]
                                          .rearrange("p (c b) -> p c b", b=BC))
                    nc.gpsimd.tensor_copy(pv_b[:, :, :, S - 1 - s],
                                          hcur[:, OFF_PB:OFF_PB + FDP]
                                          .rearrange("p (c b) -> p c b", b=BC))
                    nc.gpsimd.tensor_copy(qv_f[:, :, :, s],
                                          hcur[:, OFF_QF:OFF_QF + FDQ]
                                          .rearrange("p (c b) -> p c b", b=BC))
                    nc.gpsimd.tensor_copy(qv_b[:, :, :, S - 1 - s],
                                          hcur[:, OFF_QB:OFF_QB + FDQ]
                                          .rearrange("p (c b) -> p c b", b=BC))

            # ---- attention ----
            # the w1.p and attn_b logit terms are constant across q, so they
            # cancel in the softmax and are never computed
            pqw = psB.tile([1, 512], F32, name="pqw", tag="t2")
            nc.tensor.matmul(pqw[0:1, 0:NTQ], sew_sb[:, 12:13],
                             qencFB[:, 0:NTQ], start=True, stop=False)
            nc.tensor.matmul(pqw[0:1, 0:NTQ], sew_sb[:, 13:14],
                             qencFB[:, NTQ:2 * NTQ], start=False, stop=True)
            nc.vector.scalar_tensor_tensor(
                qwm[0:1, :], mq_sb[0:1, :], NEG, pqw[0:1, 0:NTQ],
                op0=ALU.mult, op1=ALU.add)
            nc.vector.tensor_scalar_mul(qenc3[:, 0:NTQ], qencFB[:, 0:NTQ],
                                        outw_sb[:, 4:5])
            nc.vector.tensor_scalar_mul(qenc3[:, NTQ:2 * NTQ],
                                        qencFB[:, NTQ:2 * NTQ],
                                        outw_sb[:, 5:6])

            # logits for all 16 (b, tcn) blocks into one psum tile; the
            # qwm mask is -1e7 at padded q so exp underflows to exactly 0 --
            # no max-subtraction needed (logits are O(10) bounded).  The
            # tiny qwm broadcast matmul opens each group so the big penc
            # matmuls never sit decoded-but-blocked in the PE queue.
            plg = psA.tile([128, 1024], F32, name="plg", tag="t1")
            for b in range(BC):
                for tcn in range(4):
                    t0 = b * P + tcn * 128
                    o = (b * 4 + tcn) * 64
                    nc.tensor.matmul(plg[:, o:o + 64], ones_sb[0:1, :],
                                     qwm[0:1, b * Q:(b + 1) * Q],
                                     start=True, stop=False)
                    nc.tensor.matmul(plg[:, o:o + 64],
                                     pencFB[:, t0:t0 + 128],
                                     qenc3[:, b * Q:(b + 1) * Q],
                                     start=False, stop=False)
                    nc.tensor.matmul(plg[:, o:o + 64],
                                     pencFB[:, NTP + t0:NTP + t0 + 128],
                                     qenc3[:, NTQ + b * Q:NTQ + (b + 1) * Q],
                                     start=False, stop=True)
            exu = pt("exu", [128, 1024], BF16)
            sm16 = pt("sm16", [128, 16], F32)
            rs16 = pt("rs16", [128, 16], F32)
            for hf in range(2):
                nc.scalar.activation(exu[:, hf * 512:(hf + 1) * 512],
                                     plg[:, hf * 512:(hf + 1) * 512], AF.Exp)
                nc.vector.tensor_reduce(
                    sm16[:, hf * 8:(hf + 1) * 8],
                    exu[:, hf * 512:(hf + 1) * 512]
                    .rearrange("p (n q) -> p n q", n=8),
                    AX, ALU.add)
                nc.vector.reciprocal(rs16[:, hf * 8:(hf + 1) * 8],
                                     sm16[:, hf * 8:(hf + 1) * 8])
                for j in range(hf * 8, hf * 8 + 8):
                    nc.vector.tensor_scalar_mul(
                        probs[:, j * 64:(j + 1) * 64],
                        exu[:, j * 64:(j + 1) * 64], rs16[:, j:j + 1])

            for b in range(BC):
                ptb = psB.tile([128, 512], BF16, name="ptb", tag="t2")
                for tcn in range(4):
                    nc.tensor.transpose(
                        ptb[0:64, tcn * 128:(tcn + 1) * 128],
                        probs[:, (b * 4 + tcn) * 64:(b * 4 + tcn + 1) * 64],
                        ident_sb[:, :])
                if b % 2 == 0:
                    nc.scalar.activation(probsT[:, b * P:(b + 1) * P],
                                         ptb[0:64, :], AF.Copy)
                else:
                    nc.vector.tensor_scalar_add(probsT[:, b * P:(b + 1) * P],
                                                ptb[0:64, :], 0.0)
            for hc2 in range(2):
                ptq = psB.tile([128, 512], BF16, name="ptq", tag="t2")
                for i in range(4):
                    b, hc = (hc2 * 4 + i) // 2, (hc2 * 4 + i) % 2
                    nc.tensor.transpose(
                        ptq[0:64, i * 128:(i + 1) * 128],
                        qencFB[:, hc * NTQ + b * Q:hc * NTQ + (b + 1) * Q],
                        ident_sb[:, :])
                nc.vector.tensor_scalar_add(
                    qencT[:, hc2 * 512:(hc2 + 1) * 512], ptq[0:64, :], 0.0)

            for b in range(BC):
                for hc in range(2):
                    paw = psA.tile([128, 1024], F32, name="paw", tag="t1")
                    nc.tensor.matmul(
                        paw[:, 0:P],
                        qencT[0:64, (b * 2 + hc) * 128:(b * 2 + hc + 1) * 128],
                        probsT[0:64, b * P:(b + 1) * P], start=True, stop=True)
                    dst = attwFB[:, hc * NTP + b * P:hc * NTP + (b + 1) * P]
                    if (b + hc) % 2 == 0:
                        nc.scalar.activation(dst, paw[:, 0:P], AF.Copy)
                    else:
                        nc.vector.tensor_scalar_add(dst, paw[:, 0:P], 0.0)
            for b in range(BC):
                for hc in range(2):
                    o = hc * NTP + b * P
                    nc.vector.tensor_mul(pawFB[:, o:o + P],
                                         pencFB[:, o:o + P],
                                         attwFB[:, o:o + P])

            for bp in range(2):
                pse = psB.tile([2, 1024], F32, name="pse", tag="t2")
                for bi in range(2):
                    b = bp * 2 + bi
                    rhss = (pencFB[:, b * P:(b + 1) * P],
                            pencFB[:, NTP + b * P:NTP + (b + 1) * P],
                            attwFB[:, b * P:(b + 1) * P],
                            attwFB[:, NTP + b * P:NTP + (b + 1) * P],
                            pawFB[:, b * P:(b + 1) * P],
                            pawFB[:, NTP + b * P:NTP + (b + 1) * P])
                    for j, rhs in enumerate(rhss):
                        nc.tensor.matmul(
                            pse[0:2, bi * P:(bi + 1) * P],
                            sew_sb[:, 2 * j:2 * j + 2],
                            rhs, start=(j == 0), stop=(j == 5))
                nc.scalar.activation(se_sb[0:2, bp * 2 * P:(bp * 2 + 2) * P],
                                     pse[0:2, :],
                                     AF.Identity, bias=outw_sb[0:2, 3:4])
            nc.gpsimd.dma_start(se8[0:BC, :], se_sb[0:1, :])
            nc.gpsimd.dma_start(se8[BC:2 * BC, :], se_sb[1:2, :])
            nc.vector.copy_predicated(se8[:, :], m8_sb[:, :], neg_sb[:, :])

            # log-softmax without max-subtraction: valid entries are O(10),
            # -1e7 pads underflow exp to 0
            nc.scalar.activation(lse_sb[:, :], se8[:, :], AF.Exp)
            nc.vector.tensor_reduce(red_sb[:, 2:3], lse_sb[:, :], AX, ALU.add)
            nc.scalar.activation(red_sb[:, 3:4], red_sb[:, 2:3], AF.Ln)
            nc.vector.tensor_scalar(out=lsm_sb[:, :], in0=se8[:, :],
                                    scalar1=red_sb[:, 3:4], scalar2=None,
                                    op0=ALU.subtract)

            nc.sync.dma_start(out[0:2 * BC, :], se8[:, :])
            nc.sync.dma_start(out[2 * BC:4 * BC, :], lsm_sb[:, :])

    _split_multiwaits(nc)
    return nc, es


def _split_multiwaits(nc):
    """HW instruction encodings hold a single semaphore wait; move extra
    waits emitted by Tile onto same-engine NOPs inserted just before."""
    for b in nc.main_func.blocks:
        il = b.instructions
        newlist = []
        for inst in il:
            if type(inst).__name__ == "InstISA":
                # EVENT_SEMAPHORE_RANGE_CLEAR mis-encodes for this walrus
                # build; NRT clears semaphores per execution anyway.
                continue
            si = inst.sync_info
            if si is not None and len(si.on_wait) > 1:
                waits = list(si.on_wait)
                for wx in waits[:-1]:
                    nop = nc.engines[inst.engine].nop(hint="wsplit").ins
                    # remove from wherever nop() appended it
                    for bb in nc.main_func.blocks:
                        try:
                            bb.instructions.remove(nop)
                            break
                        except ValueError:
                            pass
                    nop.sync_info = mybir.SyncInfo(on_wait=[wx], on_update=[])
                    newlist.append(nop)
                inst.sync_info = mybir.SyncInfo(on_wait=[waits[-1]],
                                                on_update=list(si.on_update))
            newlist.append(inst)
        il[:] = newlist


def _perm_tokens(tok2d, nch, blk):
    """Token array (BC, T) -> s-major column order: col = rank(s)*blk + c*BC + b."""
    T = tok2d.shape[1]
    cols = np.empty(BC * T, np.int64)
    for rank in range(S):
        s = SORD[rank]
        blkv = tok2d[:, s::S]          # (BC, nch) tokens at pos s per chunk
        # col index rank*blk + c*BC + b
        cols[rank * blk:(rank + 1) * blk] = blkv.T.reshape(-1)
    return cols


def _prep_core(inputs, c):
    bs = slice(c * BC, (c + 1) * BC)
    ptok = np.asarray(inputs["passage"][bs]).astype(np.int64)
    qtok = np.asarray(inputs["question"][bs]).astype(np.int64)
    embp = inputs["_embp"]
    pcols = _perm_tokens(ptok, NCHP, FDP)
    qcols = _perm_tokens(qtok, NCHQ, FDQ)
    d = {}
    d["epTp_d"] = np.ascontiguousarray(
        embp[pcols].T.reshape(3, 128, NTP).transpose(1, 0, 2).reshape(128, -1))
    d["epTq_d"] = np.ascontiguousarray(
        embp[qcols].T.reshape(3, 128, NTQ).transpose(1, 0, 2).reshape(128, -1))
    qm0 = (qtok.reshape(-1) == 0).astype(np.float32)
    d["mq"] = np.ascontiguousarray(qm0[None, :])
    pm2 = (ptok.reshape(-1) == 0).reshape(BC, P).astype(np.uint8)
    d["m8"] = np.ascontiguousarray(np.concatenate([pm2, pm2], axis=0))
    return d


def _prep_shared(inputs):
    bf = ml_dtypes.bfloat16

    wihT = np.zeros((4, 3, 128, 3 * HH), bf)      # (d, kc, p, m)
    whhT = np.zeros((4, HH, 3 * HH), bf)          # (d, p, m)
    bhnr = np.zeros((4, HH), bf)
    for di, (pre, dd) in enumerate((("p", "f"), ("p", "b"),
                                    ("q", "f"), ("q", "b"))):
        wih = np.asarray(inputs[f"{pre}_wih_{dd}"], np.float32)
        whh = np.asarray(inputs[f"{pre}_whh_{dd}"], np.float32)
        bih = np.asarray(inputs[f"{pre}_bih_{dd}"], np.float32)
        bhh = np.asarray(inputs[f"{pre}_bhh_{dd}"], np.float32)
        wT = np.zeros((EPAD, 3 * HH), np.float32)
        wT[:E, :] = wih.T
        # row 300: pad-token indicator -> +BIGM on the z gate
        wT[300, HH:2 * HH] = BIGM
        # row 301: constant-1 -> gate biases (bih+bhh for r/z, bih for n)
        wT[301, 0:HH] = bih[0:HH] + bhh[0:HH]
        wT[301, HH:2 * HH] = bih[HH:2 * HH] + bhh[HH:2 * HH]
        wT[301, 2 * HH:] = bih[2 * HH:]
        wihT[di] = wT.astype(bf).reshape(3, 128, 3 * HH)
        whhT[di] = whh.T.astype(bf)
        bhnr[di] = bhh[2 * HH:].astype(bf)
    wihT = np.ascontiguousarray(
        wihT.transpose(2, 0, 1, 3).reshape(128, -1))      # (p,(d,kc,m))
    whhT = np.ascontiguousarray(
        whhT.transpose(1, 0, 2).reshape(128, -1))         # (p,(d,m))
    bhnr = np.ascontiguousarray(bhnr.reshape(1, -1))

    aw = np.asarray(inputs["attn_w"], np.float32)
    w1, w2, w3 = aw[:256], aw[256:512], aw[512:]
    outw = np.zeros((HH, 8), np.float32)
    outw[:, 4], outw[:, 5] = w3[:128], w3[128:]
    outw[0, 2] = float(np.asarray(inputs["attn_b"]))
    outw[0, 3] = float(np.asarray(inputs["start_b"]))
    outw[1, 3] = float(np.asarray(inputs["end_b"]))

    sw = np.asarray(inputs["start_w"], np.float32)
    ew = np.asarray(inputs["end_w"], np.float32)
    sew = np.zeros((HH, 14), bf)
    for j in range(6):
        sew[:, 2 * j] = sw[j * 128:(j + 1) * 128].astype(bf)
        sew[:, 2 * j + 1] = ew[j * 128:(j + 1) * 128].astype(bf)
    sew[:, 12] = w2[:128].astype(bf)
    sew[:, 13] = w2[128:].astype(bf)
    return {"wihT": wihT, "whhT": whhT, "bhnr": bhnr,
            "outw": outw, "sew": sew}


def kernel(**inputs):
    if "nc" not in _CACHE:
        _CACHE["nc"] = _build_nc()
    nc, _es = _CACHE["nc"]
    shared = _prep_shared(inputs)
    bf = ml_dtypes.bfloat16
    embp = np.zeros((VOCAB, EPAD), bf)
    embp[:, :E] = np.asarray(inputs["emb"], np.float32).astype(bf)
    embp[0, 300] = 1.0   # pad-token indicator row
    embp[:, 301] = 1.0   # constant-1 bias row
    inputs = dict(inputs)
    inputs["_embp"] = embp
    in_maps = []
    for c in range(NC):
        m = dict(shared)
        m.update(_prep_core(inputs, c))
        in_maps.append(m)
    res = run_bass_kernel_spmd(nc, in_maps, list(range(NC)))
    outs = [np.asarray(res.results[c]["out"]) for c in range(NC)]
    se = np.concatenate([o[0:2 * BC].reshape(2, BC, P) for o in outs], axis=1)
    lsm = np.concatenate([o[2 * BC:].reshape(2, BC, P) for o in outs], axis=1)
    return (np.ascontiguousarray(se[0]), np.ascontiguousarray(se[1]),
            np.ascontiguousarray(lsm[0]), np.ascontiguousarray(lsm[1]))


# revision 20
# speedup vs baseline: 2.6879x; 1.0013x over previous
"""AttentionRNN (BiDAF-style QA reader) Trainium2 kernel.

Per core (pure data-parallel over batch, 4 of 32 rows per core):
  1. Host gathers embeddings in an s-major permuted token order and pads two
     extra embedding rows: row 300 = pad-token indicator (drives a +BIGM into
     the z gate via the weight matrix, freezing h at padded steps), row 301 =
     constant 1.0 (injects the gate biases).  So each scan round's x-gate
     pre-activations are plain contiguous-slice matmuls.
  2. GRU scans as chunked-parallel recurrences: chunks of S=16 payload steps
     with W=12 warmup steps re-run from h=0 (the GRU contracts ~0.6/step).
     Chunks whose warmup would cross t=0 are frozen (z pinned via +BIGM)
     until their true start.  One round = one time step of 72 chains; the
     x-projection matmuls for round k+1 are issued ahead of round k's
     recurrent matmuls so the PE stays busy during the serial chain.
  3. Decomposed BiDAF attention, softmax over Q, start/end heads, log-softmax
     over P; padded positions forced to exactly -1e7 as in the reference.
"""

import contextlib

import numpy as np
import ml_dtypes

import concourse.bass as bass
import concourse.mybir as mybir
from concourse.masks import make_identity
from concourse.tile import TileContext
from concourse.bass_utils import run_bass_kernel_spmd

F32 = mybir.dt.float32
BF16 = mybir.dt.bfloat16
U8 = mybir.dt.uint8
AX = mybir.AxisListType.X
ALU = mybir.AluOpType
AF = mybir.ActivationFunctionType

B, P, Q, E, H, VOCAB = 32, 512, 64, 300, 256, 50000
HH = 128
EPAD = 384
NC = 8
BC = B // NC
NEG = -1e7
BIGM = 1.0e4

S, W = 16, 12
RND = S + W                   # 28 rounds
NCHP, NCHQ = P // S, Q // S   # 32, 4
FDP, FDQ = NCHP * BC, NCHQ * BC   # 128, 16
OFF_PF, OFF_PB, OFF_QF, OFF_QB = 0, FDP, 2 * FDP, 2 * FDP + FDQ
FDT = 2 * FDP + 2 * FDQ       # 288

NTP, NTQ = BC * P, BC * Q     # 2048, 256

_CACHE = {}


def _mk_rank():
    order, seen = [], set()
    for k in range(RND):
        s = (k - W) % S
        for v in (s, S - 1 - s):
            if v not in seen:
                seen.add(v)
                order.append(v)
    rank = [0] * S
    for i, s in enumerate(order):
        rank[s] = i
    return order, rank


SORD, SRANK = _mk_rank()


def _round_geom(k):
    e = k - W
    s = e % S
    coff = (e - s) // S                      # -1 | 0
    cmin = (W - k + S - 1) // S if k < W else 0
    return s, coff, cmin


def _build_nc():
    nc = bass.Bass()

    epTp_d = nc.declare_dram_parameter("epTp_d", [128, 3 * NTP], BF16,
                                       isOutput=False)
    epTq_d = nc.declare_dram_parameter("epTq_d", [128, 3 * NTQ], BF16,
                                       isOutput=False)
    mq_d = nc.declare_dram_parameter("mq", [1, NTQ], F32, isOutput=False)
    m8 = nc.declare_dram_parameter("m8", [2 * BC, P], U8, isOutput=False)
    wihT = nc.declare_dram_parameter("wihT", [128, 4 * 3 * 3 * HH], BF16,
                                     isOutput=False)
    whhT = nc.declare_dram_parameter("whhT", [128, 4 * 3 * HH], BF16,
                                     isOutput=False)
    bhnr_d = nc.declare_dram_parameter("bhnr", [1, 4 * HH], BF16,
                                       isOutput=False)
    outw = nc.declare_dram_parameter("outw", [HH, 8], F32, isOutput=False)
    seww = nc.declare_dram_parameter("sew", [HH, 14], BF16, isOutput=False)
    out = nc.declare_dram_parameter("out", [4 * BC, P], F32, isOutput=True)

    es = contextlib.ExitStack()

    # ---------- Tile phases ----------
    with TileContext(nc) as tc:
        with tc.tile_pool(name="psA", bufs=2, space="PSUM") as psA, \
             tc.tile_pool(name="psB", bufs=2, space="PSUM") as psB, \
             tc.tile_pool(name="sbp", bufs=2) as sbp, \
             tc.tile_pool(name="pst", bufs=1) as pst:

            def pt(name, shape, dtype):
                return pst.tile(shape, dtype, name=name, tag=name)

            neg_sb = pt("neg_sb", [2 * BC, P], F32)
            ones_sb = pt("ones_sb", [1, 128], BF16)
            bigm_sb = pt("bigm_sb", [1, 128], BF16)

            # input tiles (DMA-streamed)
            epTp = pt("epTp", [128, 3, NTP], BF16)
            epTq = pt("epTq", [128, 3, NTQ], BF16)
            wih_sb = pt("wih_sb", [128, 4 * 3 * 3 * HH], BF16)
            whh_sb = pt("whh_sb", [128, 4 * 3 * HH], BF16)
            bhnr_sb = pt("bhnr_sb", [1, 4 * HH], BF16)
            mq_sb = pt("mq_sb", [1, NTQ], F32)
            m8_sb = pt("m8_sb", [2 * BC, P], U8)
            outw_sb = pt("outw_sb", [128, 8], F32)
            sew_sb = pt("sew_sb", [128, 14], BF16)

            # scan state
            pencFB = pt("pencFB", [128, 2 * NTP], BF16)
            qencFB = pt("qencFB", [128, 2 * NTQ], BF16)
            hcur = pt("hcur", [128, FDT], BF16)
            rz_sb = pt("rz_sb", [128, 2 * FDT], BF16)
            nh_sb = pt("nh_sb", [128, FDT], BF16)
            nx_sb = pt("nx_sb", [128, 2, FDT], BF16)
            t1_sb = pt("t1_sb", [128, FDT], BF16)
            t2_sb = pt("t2_sb", [128, FDT], BF16)
            n_sb = pt("n_sb", [128, FDT], BF16)
            d_sb = pt("d_sb", [128, FDT], BF16)
            e_sb = pt("e_sb", [128, FDT], BF16)
            ident_sb = pt("ident_sb", [128, 128], BF16)
            # attention tiles
            qenc3 = pt("qenc3", [128, 2 * NTQ], BF16)
            qwm = pt("qwm", [1, NTQ], BF16)
            qwt = pt("qwt", [1, NTQ], F32)
            probs = pt("probs", [128, 64 * 4 * BC], BF16)
            probsT = pt("probsT", [64, P * BC], BF16)
            qencT = pt("qencT", [64, 2 * HH * BC], BF16)
            attwFB = pt("attwFB", [128, 2 * NTP], BF16)
            pawFB = pt("pawFB", [128, 2 * NTP], BF16)
            se_sb = pt("se_sb", [2, BC * P], F32)
            se8 = pt("se8", [2 * BC, P], F32)
            lsm_sb = pt("lsm_sb", [2 * BC, P], F32)
            lse_sb = pt("lse_sb", [2 * BC, P], F32)
            red_sb = pt("red_sb", [2 * BC, 8], F32)

            # ---- input DMA (gpsimd queue: cheap issue), priority order ----
            g = nc.gpsimd
            HW12 = 4 * 3 * 3 * HH // 2
            g.dma_start(wih_sb[:, 0:HW12], wihT[:, 0:HW12])       # p dirs
            epv = epTp_d[:, :].rearrange("p (c t) -> p c t", c=3)
            g.dma_start(epTp[:, :, 0:4 * FDP], epv[:, :, 0:4 * FDP])
            g.dma_start(wih_sb[:, HW12:], wihT[:, HW12:])         # q dirs
            g.dma_start(epTq[:, :, :],
                        epTq_d[:, :].rearrange("p (c t) -> p c t", c=3))
            g.dma_start(whh_sb[:, :], whhT[:, :])
            g.dma_start(bhnr_sb[:, :], bhnr_d[:, :])
            for r0, r1 in ((4, 6), (6, 8), (8, 16)):
                g.dma_start(epTp[:, :, r0 * FDP:r1 * FDP],
                            epv[:, :, r0 * FDP:r1 * FDP])
            g.dma_start(mq_sb[:, :], mq_d[:, :])
            g.dma_start(m8_sb[:, :], m8[:, :])
            g.dma_start(outw_sb[0:HH, :], outw[:, :])
            g.dma_start(sew_sb[0:HH, :], seww[:, :])

            nc.vector.memset(ones_sb[:, :], 1.0)
            nc.vector.memset(bigm_sb[:, :], BIGM)
            nc.vector.memset(hcur[:, :], 0)
            nc.vector.memset(neg_sb[:, :], NEG)
            make_identity(nc, ident_sb[:, :])

            # round psum tiles: T1 holds r (bank0) | z (bank1),
            #                   T2 holds nx (bank0) | nh (bank1)
            tiles = [None] * RND

            def alloc_round(j):
                tiles[j] = (psA.tile([128, 1024], F32, name="T1", tag="t1"),
                            psB.tile([128, 1024], F32, name="T2", tag="t2"))

            # x-projection geometry for round j: per dir, the contiguous
            # epT column range and psum dst range.
            def xgeom(j):
                s, coff, cmin = _round_geom(j)
                res = []
                for (nch, fd, offF, offB, epT, blk) in (
                        (NCHP, FDP, OFF_PF, OFF_PB, epTp, FDP),
                        (NCHQ, FDQ, OFF_QF, OFF_QB, epTq, FDQ)):
                    cnt = nch - cmin
                    # forward: chains [cmin, nch) read chunk c+coff at pos s
                    res.append((epT,
                                SRANK[s] * blk + (cmin + coff) * BC,
                                offF + cmin * BC, cnt * BC))
                    # backward: chains [0, cnt) read chunk c-coff at pos S-1-s
                    res.append((epT,
                                SRANK[S - 1 - s] * blk + (-coff) * BC,
                                offB, cnt * BC))
                return res

            def emit_wih(j):
                T1, T2 = tiles[j]
                s, coff, cmin = _round_geom(j)
                geo = xgeom(j)
                for gate, dst, goff in ((0, T1, 0), (1, T1, 512), (2, T2, 0)):
                    first = True
                    for di in range(4):
                        epT, c0, o0, wd = geo[di]
                        for kc in range(3):
                            wcol = ((di * 3 + kc) * 3 + gate) * HH
                            # the nx group (gate 2) has no whh part, so its
                            # last matmul closes the accumulation group
                            last = gate == 2 and di == 3 and kc == 2
                            nc.tensor.matmul(
                                dst[:, goff + o0:goff + o0 + wd],
                                wih_sb[:, wcol:wcol + HH],
                                epT[:, kc, c0:c0 + wd],
                                start=first, stop=last)
                            first = False
                    if gate == 1 and cmin > 0:
                        # freeze warmup-frozen chains: z += BIGM
                        fz = cmin * BC
                        for lo in (OFF_PF, OFF_PB + FDP - fz,
                                   OFF_QF, OFF_QB + FDQ - fz):
                            nc.tensor.matmul(
                                dst[:, 512 + lo:512 + lo + fz],
                                bigm_sb[0:1, :], ones_sb[0:1, 0:fz],
                                start=False, stop=False)
                # nh group: bhh_n broadcast rows
                for di, (off, fd) in enumerate(((OFF_PF, FDP), (OFF_PB, FDP),
                                                (OFF_QF, FDQ), (OFF_QB, FDQ))):
                    nc.tensor.matmul(T2[:, 512 + off:512 + off + fd],
                                     bhnr_sb[0:1, di * HH:(di + 1) * HH],
                                     ones_sb[0:1, 0:fd],
                                     start=(di == 0), stop=False)

            def emit_whh(j):
                T1, T2 = tiles[j]
                DIRS = ((0, OFF_PF, FDP), (1, OFF_PB, FDP),
                        (2, OFF_QF, FDQ), (3, OFF_QB, FDQ))
                for gi, dst, goff in ((0, T1, 0), (1, T1, 512), (2, T2, 512)):
                    for di, (d, off, fd) in enumerate(DIRS):
                        nc.tensor.matmul(
                            dst[:, goff + off:goff + off + fd],
                            whh_sb[:, (d * 3 + gi) * HH:(d * 3 + gi + 1) * HH],
                            hcur[:, off:off + fd],
                            start=False, stop=(di == 3))

            # payload output views (b-major column layout: b*T + c*S + s)
            def view4(x, base, ntok, nch):
                return x[:, base:base + ntok].rearrange(
                    "p (b c s) -> p c b s", b=BC, c=nch, s=S)

            pv_f = view4(pencFB, 0, NTP, NCHP)
            pv_b = view4(pencFB, NTP, NTP, NCHP)
            qv_f = view4(qencFB, 0, NTQ, NCHQ)
            qv_b = view4(qencFB, NTQ, NTQ, NCHQ)

            # ---- the scan ----
            alloc_round(0)
            emit_wih(0)
            alloc_round(1)
            emit_wih(1)
            nc.scalar.activation(nx_sb[:, 0, :], tiles[0][1][:, 0:FDT],
                                 AF.Copy)
            for k in range(RND):
                T1, T2 = tiles[k]
                s, coff, cmin = _round_geom(k)
                emit_whh(k)
                # nh evacuation (psum -> sbuf bf16), overlaps sigmoid
                nc.vector.tensor_scalar_add(nh_sb[:, :], T2[:, 512:512 + FDT],
                                            0.0)
                # r sigmoid on the critical path; z sigmoid off it
                nc.scalar.activation(rz_sb[:, 0:FDT], T1[:, 0:FDT],
                                     AF.Sigmoid)
                nc.scalar.activation(rz_sb[:, FDT:2 * FDT],
                                     T1[:, 512:512 + FDT], AF.Sigmoid)
                nc.vector.tensor_mul(t1_sb[:, :], rz_sb[:, 0:FDT],
                                     nh_sb[:, :])
                nc.vector.tensor_add(t2_sb[:, :], t1_sb[:, :],
                                     nx_sb[:, k % 2, :])
                nc.scalar.activation(n_sb[:, :], t2_sb[:, :], AF.Tanh)
                # h' = n*(1-z) + z*h with (1-z) and z*h computed during the
                # tanh window so only two ops remain after it
                nc.vector.tensor_scalar(out=d_sb[:, :],
                                        in0=rz_sb[:, FDT:2 * FDT],
                                        scalar1=-1.0, scalar2=1.0,
                                        op0=ALU.mult, op1=ALU.add)
                nc.vector.tensor_mul(e_sb[:, :], rz_sb[:, FDT:2 * FDT],
                                     hcur[:, :])
                nc.vector.tensor_mul(t1_sb[:, :], n_sb[:, :], d_sb[:, :])
                nc.vector.tensor_add(hcur[:, :], t1_sb[:, :], e_sb[:, :])
                if k + 2 < RND:
                    alloc_round(k + 2)
                    emit_wih(k + 2)
                if k + 1 < RND:
                    nc.scalar.activation(nx_sb[:, (k + 1) % 2, :],
                                         tiles[k + 1][1][:, 0:FDT], AF.Copy)
                if k >= W:
                    nc.gpsimd.tensor_copy(pv_f[:, :, :, s],
                                          hcur[:, OFF_PF:OFF_PF + FDP]
                                          .rearrange("p (c b) -> p c b", b=BC))
                    nc.gpsimd.tensor_copy(pv_b[:, :, :, S - 1 - s],
                                          hcur[:, OFF_PB:OFF_PB + FDP]
                                          .rearrange("p (c b) -> p c b", b=BC))
                    nc.gpsimd.tensor_copy(qv_f[:, :, :, s],
                                          hcur[:, OFF_QF:OFF_QF + FDQ]
                                          .rearrange("p (c b) -> p c b", b=BC))
                    nc.gpsimd.tensor_copy(qv_b[:, :, :, S - 1 - s],
                                          hcur[:, OFF_QB:OFF_QB + FDQ]
                                          .rearrange("p (c b) -> p c b", b=BC))

            # ---- attention ----
            # keep the PE busy across the scan->attention transition so the
            # p-state stays high; these writes land in the plg tile before
            # its first accumulation group starts, which discards them
            fill = psA.tile([128, 1024], F32, name="plgf", tag="t1")
            for _ in range(35):
                nc.tensor.matmul(fill[:, 0:512], ident_sb[:, :],
                                 epTp[:, 0, 0:512], start=False, stop=False,
                                 skip_group_check=True)

            # qenc transposes first: they only need qencFB
            for hc2 in range(2):
                ptq = psB.tile([128, 512], BF16, name="ptq", tag="t2")
                for i in range(4):
                    b, hc = (hc2 * 4 + i) // 2, (hc2 * 4 + i) % 2
                    nc.tensor.transpose(
                        ptq[0:64, i * 128:(i + 1) * 128],
                        qencFB[:, hc * NTQ + b * Q:hc * NTQ + (b + 1) * Q],
                        ident_sb[:, :])
                nc.vector.tensor_scalar_add(
                    qencT[:, hc2 * 512:(hc2 + 1) * 512], ptq[0:64, :], 0.0)

            # the w1.p and attn_b logit terms are constant across q, so they
            # cancel in the softmax and are never computed
            pqw = psB.tile([1, 512], F32, name="pqw", tag="t2")
            nc.tensor.matmul(pqw[0:1, 0:NTQ], sew_sb[:, 12:13],
                             qencFB[:, 0:NTQ], start=True, stop=False)
            nc.tensor.matmul(pqw[0:1, 0:NTQ], sew_sb[:, 13:14],
                             qencFB[:, NTQ:2 * NTQ], start=False, stop=True)
            nc.vector.scalar_tensor_tensor(
                qwm[0:1, :], mq_sb[0:1, :], NEG, pqw[0:1, 0:NTQ],
                op0=ALU.mult, op1=ALU.add)
            nc.vector.tensor_scalar_mul(qenc3[:, 0:NTQ], qencFB[:, 0:NTQ],
                                        outw_sb[:, 4:5])
            nc.vector.tensor_scalar_mul(qenc3[:, NTQ:2 * NTQ],
                                        qencFB[:, NTQ:2 * NTQ],
                                        outw_sb[:, 5:6])

            # logits for all 16 (b, tcn) blocks into one psum tile; the
            # qwm mask is -1e7 at padded q so exp underflows to exactly 0 --
            # no max-subtraction needed (logits are O(10) bounded).  The
            # tiny qwm broadcast matmul opens each group so the big penc
            # matmuls never sit decoded-but-blocked in the PE queue.
            plg = psA.tile([128, 1024], F32, name="plg", tag="t1")
            for b in range(BC):
                for tcn in range(4):
                    t0 = b * P + tcn * 128
                    o = (b * 4 + tcn) * 64
                    nc.tensor.matmul(plg[:, o:o + 64], ones_sb[0:1, :],
                                     qwm[0:1, b * Q:(b + 1) * Q],
                                     start=True, stop=False)
                    nc.tensor.matmul(plg[:, o:o + 64],
                                     pencFB[:, t0:t0 + 128],
                                     qenc3[:, b * Q:(b + 1) * Q],
                                     start=False, stop=False)
                    nc.tensor.matmul(plg[:, o:o + 64],
                                     pencFB[:, NTP + t0:NTP + t0 + 128],
                                     qenc3[:, NTQ + b * Q:NTQ + (b + 1) * Q],
                                     start=False, stop=True)
            exu = pt("exu", [128, 1024], BF16)
            sm16 = pt("sm16", [128, 16], F32)
            rs16 = pt("rs16", [128, 16], F32)
            for hf in range(2):
                nc.scalar.activation(exu[:, hf * 512:(hf + 1) * 512],
                                     plg[:, hf * 512:(hf + 1) * 512], AF.Exp)
                nc.vector.tensor_reduce(
                    sm16[:, hf * 8:(hf + 1) * 8],
                    exu[:, hf * 512:(hf + 1) * 512]
                    .rearrange("p (n q) -> p n q", n=8),
                    AX, ALU.add)
                nc.vector.reciprocal(rs16[:, hf * 8:(hf + 1) * 8],
                                     sm16[:, hf * 8:(hf + 1) * 8])
                for j in range(hf * 8, hf * 8 + 8):
                    nc.vector.tensor_scalar_mul(
                        probs[:, j * 64:(j + 1) * 64],
                        exu[:, j * 64:(j + 1) * 64], rs16[:, j:j + 1])

            for b in range(BC):
                ptb = psB.tile([128, 512], BF16, name="ptb", tag="t2")
                for tcn in range(4):
                    nc.tensor.transpose(
                        ptb[0:64, tcn * 128:(tcn + 1) * 128],
                        probs[:, (b * 4 + tcn) * 64:(b * 4 + tcn + 1) * 64],
                        ident_sb[:, :])
                if b % 2 == 0:
                    nc.scalar.activation(probsT[:, b * P:(b + 1) * P],
                                         ptb[0:64, :], AF.Copy)
                else:
                    nc.vector.tensor_scalar_add(probsT[:, b * P:(b + 1) * P],
                                                ptb[0:64, :], 0.0)

            for b in range(BC):
                for hc in range(2):
                    paw = psA.tile([128, 1024], F32, name="paw", tag="t1")
                    nc.tensor.matmul(
                        paw[:, 0:P],
                        qencT[0:64, (b * 2 + hc) * 128:(b * 2 + hc + 1) * 128],
                        probsT[0:64, b * P:(b + 1) * P], start=True, stop=True)
                    dst = attwFB[:, hc * NTP + b * P:hc * NTP + (b + 1) * P]
                    if (b + hc) % 2 == 0:
                        nc.scalar.activation(dst, paw[:, 0:P], AF.Copy)
                    else:
                        nc.vector.tensor_scalar_add(dst, paw[:, 0:P], 0.0)
            for b in range(BC):
                for hc in range(2):
                    o = hc * NTP + b * P
                    nc.vector.tensor_mul(pawFB[:, o:o + P],
                                         pencFB[:, o:o + P],
                                         attwFB[:, o:o + P])

            for bp in range(2):
                pse = psB.tile([2, 1024], F32, name="pse", tag="t2")
                for bi in range(2):
                    b = bp * 2 + bi
                    rhss = (pencFB[:, b * P:(b + 1) * P],
                            pencFB[:, NTP + b * P:NTP + (b + 1) * P],
                            attwFB[:, b * P:(b + 1) * P],
                            attwFB[:, NTP + b * P:NTP + (b + 1) * P],
                            pawFB[:, b * P:(b + 1) * P],
                            pawFB[:, NTP + b * P:NTP + (b + 1) * P])
                    for j, rhs in enumerate(rhss):
                        nc.tensor.matmul(
                            pse[0:2, bi * P:(bi + 1) * P],
                            sew_sb[:, 2 * j:2 * j + 2],
                            rhs, start=(j == 0), stop=(j == 5))
                nc.scalar.activation(se_sb[0:2, bp * 2 * P:(bp * 2 + 2) * P],
                                     pse[0:2, :],
                                     AF.Identity, bias=outw_sb[0:2, 3:4])
                # fan this half out to se8 rows on two different DMA queues
                c0 = bp * 2 * P
                nc.gpsimd.dma_start(se8[bp * 2:bp * 2 + 2, :],
                                    se_sb[0:1, c0:c0 + 2 * P])
                nc.sync.dma_start(se8[BC + bp * 2:BC + bp * 2 + 2, :],
                                  se_sb[1:2, c0:c0 + 2 * P])
            nc.vector.copy_predicated(se8[:, :], m8_sb[:, :], neg_sb[:, :])

            # log-softmax without max-subtraction: valid entries are O(10),
            # -1e7 pads underflow exp to 0
            nc.scalar.activation(lse_sb[:, :], se8[:, :], AF.Exp)
            nc.vector.tensor_reduce(red_sb[:, 2:3], lse_sb[:, :], AX, ALU.add)
            nc.scalar.activation(red_sb[:, 3:4], red_sb[:, 2:3], AF.Ln)
            nc.vector.tensor_scalar(out=lsm_sb[:, :], in0=se8[:, :],
                                    scalar1=red_sb[:, 3:4], scalar2=None,
                                    op0=ALU.subtract)

            nc.sync.dma_start(out[0:2 * BC, :], se8[:, :])
            nc.sync.dma_start(out[2 * BC:4 * BC, :], lsm_sb[:, :])

    _split_multiwaits(nc)
    return nc, es


def _split_multiwaits(nc):
    """HW instruction encodings hold a single semaphore wait; move extra
    waits emitted by Tile onto same-engine NOPs inserted just before."""
    for b in nc.main_func.blocks:
        il = b.instructions
        newlist = []
        for inst in il:
            if type(inst).__name__ == "InstISA":
                # EVENT_SEMAPHORE_RANGE_CLEAR mis-encodes for this walrus
                # build; NRT clears semaphores per execution anyway.
                continue
            si = inst.sync_info
            if si is not None and len(si.on_wait) > 1:
                waits = list(si.on_wait)
                for wx in waits[:-1]:
                    nop = nc.engines[inst.engine].nop(hint="wsplit").ins
                    # remove from wherever nop() appended it
                    for bb in nc.main_func.blocks:
                        try:
                            bb.instructions.remove(nop)
                            break
                        except ValueError:
                            pass
                    nop.sync_info = mybir.SyncInfo(on_wait=[wx], on_update=[])
                    newlist.append(nop)
                inst.sync_info = mybir.SyncInfo(on_wait=[waits[-1]],
                                                on_update=list(si.on_update))
            newlist.append(inst)
        il[:] = newlist


def _perm_tokens(tok2d, nch, blk):
    """Token array (BC, T) -> s-major column order: col = rank(s)*blk + c*BC + b."""
    T = tok2d.shape[1]
    cols = np.empty(BC * T, np.int64)
    for rank in range(S):
        s = SORD[rank]
        blkv = tok2d[:, s::S]          # (BC, nch) tokens at pos s per chunk
        # col index rank*blk + c*BC + b
        cols[rank * blk:(rank + 1) * blk] = blkv.T.reshape(-1)
    return cols


def _prep_core(inputs, c):
    bs = slice(c * BC, (c + 1) * BC)
    ptok = np.asarray(inputs["passage"][bs]).astype(np.int64)
    qtok = np.asarray(inputs["question"][bs]).astype(np.int64)
    embp = inputs["_embp"]
    pcols = _perm_tokens(ptok, NCHP, FDP)
    qcols = _perm_tokens(qtok, NCHQ, FDQ)
    d = {}
    d["epTp_d"] = np.ascontiguousarray(
        embp[pcols].T.reshape(3, 128, NTP).transpose(1, 0, 2).reshape(128, -1))
    d["epTq_d"] = np.ascontiguousarray(
        embp[qcols].T.reshape(3, 128, NTQ).transpose(1, 0, 2).reshape(128, -1))
    qm0 = (qtok.reshape(-1) == 0).astype(np.float32)
    d["mq"] = np.ascontiguousarray(qm0[None, :])
    pm2 = (ptok.reshape(-1) == 0).reshape(BC, P).astype(np.uint8)
    d["m8"] = np.ascontiguousarray(np.concatenate([pm2, pm2], axis=0))
    return d


def _prep_shared(inputs):
    bf = ml_dtypes.bfloat16

    wihT = np.zeros((4, 3, 128, 3 * HH), bf)      # (d, kc, p, m)
    whhT = np.zeros((4, HH, 3 * HH), bf)          # (d, p, m)
    bhnr = np.zeros((4, HH), bf)
    for di, (pre, dd) in enumerate((("p", "f"), ("p", "b"),
                                    ("q", "f"), ("q", "b"))):
        wih = np.asarray(inputs[f"{pre}_wih_{dd}"], np.float32)
        whh = np.asarray(inputs[f"{pre}_whh_{dd}"], np.float32)
        bih = np.asarray(inputs[f"{pre}_bih_{dd}"], np.float32)
        bhh = np.asarray(inputs[f"{pre}_bhh_{dd}"], np.float32)
        wT = np.zeros((EPAD, 3 * HH), np.float32)
        wT[:E, :] = wih.T
        # row 300: pad-token indicator -> +BIGM on the z gate
        wT[300, HH:2 * HH] = BIGM
        # row 301: constant-1 -> gate biases (bih+bhh for r/z, bih for n)
        wT[301, 0:HH] = bih[0:HH] + bhh[0:HH]
        wT[301, HH:2 * HH] = bih[HH:2 * HH] + bhh[HH:2 * HH]
        wT[301, 2 * HH:] = bih[2 * HH:]
        wihT[di] = wT.astype(bf).reshape(3, 128, 3 * HH)
        whhT[di] = whh.T.astype(bf)
        bhnr[di] = bhh[2 * HH:].astype(bf)
    wihT = np.ascontiguousarray(
        wihT.transpose(2, 0, 1, 3).reshape(128, -1))      # (p,(d,kc,m))
    whhT = np.ascontiguousarray(
        whhT.transpose(1, 0, 2).reshape(128, -1))         # (p,(d,m))
    bhnr = np.ascontiguousarray(bhnr.reshape(1, -1))

    aw = np.asarray(inputs["attn_w"], np.float32)
    w1, w2, w3 = aw[:256], aw[256:512], aw[512:]
    outw = np.zeros((HH, 8), np.float32)
    outw[:, 4], outw[:, 5] = w3[:128], w3[128:]
    outw[0, 2] = float(np.asarray(inputs["attn_b"]))
    outw[0, 3] = float(np.asarray(inputs["start_b"]))
    outw[1, 3] = float(np.asarray(inputs["end_b"]))

    sw = np.asarray(inputs["start_w"], np.float32)
    ew = np.asarray(inputs["end_w"], np.float32)
    sew = np.zeros((HH, 14), bf)
    for j in range(6):
        sew[:, 2 * j] = sw[j * 128:(j + 1) * 128].astype(bf)
        sew[:, 2 * j + 1] = ew[j * 128:(j + 1) * 128].astype(bf)
    sew[:, 12] = w2[:128].astype(bf)
    sew[:, 13] = w2[128:].astype(bf)
    return {"wihT": wihT, "whhT": whhT, "bhnr": bhnr,
            "outw": outw, "sew": sew}


def kernel(**inputs):
    if "nc" not in _CACHE:
        _CACHE["nc"] = _build_nc()
    nc, _es = _CACHE["nc"]
    shared = _prep_shared(inputs)
    bf = ml_dtypes.bfloat16
    embp = np.zeros((VOCAB, EPAD), bf)
    embp[:, :E] = np.asarray(inputs["emb"], np.float32).astype(bf)
    embp[0, 300] = 1.0   # pad-token indicator row
    embp[:, 301] = 1.0   # constant-1 bias row
    inputs = dict(inputs)
    inputs["_embp"] = embp
    in_maps = []
    for c in range(NC):
        m = dict(shared)
        m.update(_prep_core(inputs, c))
        in_maps.append(m)
    res = run_bass_kernel_spmd(nc, in_maps, list(range(NC)))
    outs = [np.asarray(res.results[c]["out"]) for c in range(NC)]
    se = np.concatenate([o[0:2 * BC].reshape(2, BC, P) for o in outs], axis=1)
    lsm = np.concatenate([o[2 * BC:].reshape(2, BC, P) for o in outs], axis=1)
    return (np.ascontiguousarray(se[0]), np.ascontiguousarray(se[1]),
            np.ascontiguousarray(lsm[0]), np.ascontiguousarray(lsm[1]))


# revision 24
# speedup vs baseline: 2.7188x; 1.0115x over previous
"""AttentionRNN (BiDAF-style QA reader) Trainium2 kernel.

Per core (pure data-parallel over batch, 4 of 32 rows per core):
  1. Host gathers embeddings in an s-major permuted token order and pads two
     extra embedding rows: row 300 = pad-token indicator (drives a +BIGM into
     the z gate via the weight matrix, freezing h at padded steps), row 301 =
     constant 1.0 (injects the gate biases).  So each scan round's x-gate
     pre-activations are plain contiguous-slice matmuls.
  2. GRU scans as chunked-parallel recurrences: chunks of S=16 payload steps
     with W=12 warmup steps re-run from h=0 (the GRU contracts ~0.6/step).
     Chunks whose warmup would cross t=0 are frozen (z pinned via +BIGM)
     until their true start.  One round = one time step of 72 chains; the
     x-projection matmuls for round k+1 are issued ahead of round k's
     recurrent matmuls so the PE stays busy during the serial chain.
  3. Decomposed BiDAF attention, softmax over Q, start/end heads, log-softmax
     over P; padded positions forced to exactly -1e7 as in the reference.
"""

import contextlib

import numpy as np
import ml_dtypes

import concourse.bass as bass
import concourse.mybir as mybir
from concourse.masks import make_identity
from concourse.tile import TileContext
from concourse.bass_utils import run_bass_kernel_spmd

F32 = mybir.dt.float32
BF16 = mybir.dt.bfloat16
U8 = mybir.dt.uint8
AX = mybir.AxisListType.X
ALU = mybir.AluOpType
AF = mybir.ActivationFunctionType

B, P, Q, E, H, VOCAB = 32, 512, 64, 300, 256, 50000
HH = 128
EPAD = 384
NC = 8
BC = B // NC
NEG = -1e7
BIGM = 1.0e4

S, W = 16, 12
RND = S + W                   # 28 rounds
NCHP, NCHQ = P // S, Q // S   # 32, 4
FDP, FDQ = NCHP * BC, NCHQ * BC   # 128, 16
OFF_PF, OFF_PB, OFF_QF, OFF_QB = 0, FDP, 2 * FDP, 2 * FDP + FDQ
FDT = 2 * FDP + 2 * FDQ       # 288

NTP, NTQ = BC * P, BC * Q     # 2048, 256

_CACHE = {}


def _mk_rank():
    order, seen = [], set()
    for k in range(RND):
        s = (k - W) % S
        for v in (s, S - 1 - s):
            if v not in seen:
                seen.add(v)
                order.append(v)
    rank = [0] * S
    for i, s in enumerate(order):
        rank[s] = i
    return order, rank


SORD, SRANK = _mk_rank()


def _round_geom(k):
    e = k - W
    s = e % S
    coff = (e - s) // S                      # -1 | 0
    cmin = (W - k + S - 1) // S if k < W else 0
    return s, coff, cmin


def _build_nc():
    nc = bass.Bass()

    epTp_d = nc.declare_dram_parameter("epTp_d", [128, 3 * NTP], BF16,
                                       isOutput=False)
    epTq_d = nc.declare_dram_parameter("epTq_d", [128, 3 * NTQ], BF16,
                                       isOutput=False)
    mq_d = nc.declare_dram_parameter("mq", [1, NTQ], F32, isOutput=False)
    m8 = nc.declare_dram_parameter("m8", [2 * BC, P], U8, isOutput=False)
    wihT = nc.declare_dram_parameter("wihT", [128, 4 * 3 * 3 * HH], BF16,
                                     isOutput=False)
    whhT = nc.declare_dram_parameter("whhT", [128, 4 * 3 * HH], BF16,
                                     isOutput=False)
    bhnr_d = nc.declare_dram_parameter("bhnr", [1, 4 * HH], BF16,
                                       isOutput=False)
    outw = nc.declare_dram_parameter("outw", [HH, 8], F32, isOutput=False)
    seww = nc.declare_dram_parameter("sew", [HH, 14], BF16, isOutput=False)
    out = nc.declare_dram_parameter("out", [4 * BC, P], F32, isOutput=True)

    es = contextlib.ExitStack()

    # ---------- Tile phases ----------
    with TileContext(nc) as tc:
        with tc.tile_pool(name="psA", bufs=2, space="PSUM") as psA, \
             tc.tile_pool(name="psB", bufs=2, space="PSUM") as psB, \
             tc.tile_pool(name="sbp", bufs=2) as sbp, \
             tc.tile_pool(name="pst", bufs=1) as pst:

            def pt(name, shape, dtype):
                return pst.tile(shape, dtype, name=name, tag=name)

            neg_sb = pt("neg_sb", [2 * BC, P], F32)
            ones_sb = pt("ones_sb", [1, 128], BF16)
            bigm_sb = pt("bigm_sb", [1, 128], BF16)

            # input tiles (DMA-streamed)
            epTp = pt("epTp", [128, 3, NTP], BF16)
            epTq = pt("epTq", [128, 3, NTQ], BF16)
            wih_sb = pt("wih_sb", [128, 4 * 3 * 3 * HH], BF16)
            whh_sb = pt("whh_sb", [128, 4 * 3 * HH], BF16)
            bhnr_sb = pt("bhnr_sb", [1, 4 * HH], BF16)
            mq_sb = pt("mq_sb", [1, NTQ], F32)
            m8_sb = pt("m8_sb", [2 * BC, P], U8)
            outw_sb = pt("outw_sb", [128, 8], F32)
            sew_sb = pt("sew_sb", [128, 14], BF16)

            # scan state
            pencFB = pt("pencFB", [128, 2 * NTP], BF16)
            qencFB = pt("qencFB", [128, 2 * NTQ], BF16)
            hcur = pt("hcur", [128, FDT], BF16)
            rz_sb = pt("rz_sb", [128, 2 * FDT], BF16)
            nh_sb = pt("nh_sb", [128, FDT], BF16)
            nx_sb = pt("nx_sb", [128, 2, FDT], BF16)
            t1_sb = pt("t1_sb", [128, FDT], BF16)
            t2_sb = pt("t2_sb", [128, FDT], BF16)
            n_sb = pt("n_sb", [128, FDT], BF16)
            d_sb = pt("d_sb", [128, FDT], BF16)
            e_sb = pt("e_sb", [128, FDT], BF16)
            ident_sb = pt("ident_sb", [128, 128], BF16)
            # attention tiles
            qenc3 = pt("qenc3", [128, 2 * NTQ], BF16)
            qwm = pt("qwm", [1, NTQ], BF16)
            qwt = pt("qwt", [1, NTQ], F32)
            probs = pt("probs", [128, 64 * 4 * BC], BF16)
            probsT = pt("probsT", [64, P * BC], BF16)
            qencT = pt("qencT", [64, 2 * HH * BC], BF16)
            attwFB = pt("attwFB", [128, 2 * NTP], BF16)
            pawFB = pt("pawFB", [128, 2 * NTP], BF16)
            se_sb = pt("se_sb", [2, BC * P], F32)
            se8 = pt("se8", [2 * BC, P], F32)
            lsm_sb = pt("lsm_sb", [2 * BC, P], F32)
            lse_sb = pt("lse_sb", [2 * BC, P], F32)
            red_sb = pt("red_sb", [2 * BC, 8], F32)

            # ---- input DMA (gpsimd queue: cheap issue), priority order ----
            g = nc.gpsimd
            HW12 = 4 * 3 * 3 * HH // 2
            g.dma_start(wih_sb[:, 0:HW12], wihT[:, 0:HW12])       # p dirs
            epv = epTp_d[:, :].rearrange("p (c t) -> p c t", c=3)
            g.dma_start(epTp[:, :, 0:4 * FDP], epv[:, :, 0:4 * FDP])
            g.dma_start(wih_sb[:, HW12:], wihT[:, HW12:])         # q dirs
            g.dma_start(epTq[:, :, :],
                        epTq_d[:, :].rearrange("p (c t) -> p c t", c=3))
            g.dma_start(whh_sb[:, :], whhT[:, :])
            g.dma_start(bhnr_sb[:, :], bhnr_d[:, :])
            for r0, r1 in ((4, 6), (6, 8), (8, 16)):
                g.dma_start(epTp[:, :, r0 * FDP:r1 * FDP],
                            epv[:, :, r0 * FDP:r1 * FDP])
            g.dma_start(mq_sb[:, :], mq_d[:, :])
            g.dma_start(m8_sb[:, :], m8[:, :])
            g.dma_start(outw_sb[0:HH, :], outw[:, :])
            g.dma_start(sew_sb[0:HH, :], seww[:, :])

            nc.vector.memset(ones_sb[:, :], 1.0)
            nc.vector.memset(bigm_sb[:, :], BIGM)
            nc.vector.memset(hcur[:, :], 0)
            nc.vector.memset(neg_sb[:, :], NEG)
            make_identity(nc, ident_sb[:, :])

            # round psum tiles: T1 holds r (bank0) | z (bank1),
            #                   T2 holds nx (bank0) | nh (bank1)
            tiles = [None] * RND

            def alloc_round(j):
                tiles[j] = (psA.tile([128, 1024], F32, name="T1", tag="t1"),
                            psB.tile([128, 1024], F32, name="T2", tag="t2"))

            # x-projection geometry for round j: per dir, the contiguous
            # epT column range and psum dst range.
            def xgeom(j):
                s, coff, cmin = _round_geom(j)
                res = []
                for (nch, fd, offF, offB, epT, blk) in (
                        (NCHP, FDP, OFF_PF, OFF_PB, epTp, FDP),
                        (NCHQ, FDQ, OFF_QF, OFF_QB, epTq, FDQ)):
                    cnt = nch - cmin
                    # forward: chains [cmin, nch) read chunk c+coff at pos s
                    res.append((epT,
                                SRANK[s] * blk + (cmin + coff) * BC,
                                offF + cmin * BC, cnt * BC))
                    # backward: chains [0, cnt) read chunk c-coff at pos S-1-s
                    res.append((epT,
                                SRANK[S - 1 - s] * blk + (-coff) * BC,
                                offB, cnt * BC))
                return res

            def emit_wih(j):
                T1, T2 = tiles[j]
                s, coff, cmin = _round_geom(j)
                geo = xgeom(j)
                for gate, dst, goff in ((0, T1, 0), (1, T1, 512), (2, T2, 0)):
                    first = True
                    for di in range(4):
                        epT, c0, o0, wd = geo[di]
                        for kc in range(3):
                            wcol = ((di * 3 + kc) * 3 + gate) * HH
                            # the nx group (gate 2) has no whh part, so its
                            # last matmul closes the accumulation group
                            last = gate == 2 and di == 3 and kc == 2
                            nc.tensor.matmul(
                                dst[:, goff + o0:goff + o0 + wd],
                                wih_sb[:, wcol:wcol + HH],
                                epT[:, kc, c0:c0 + wd],
                                start=first, stop=last)
                            first = False
                    if gate == 1 and cmin > 0:
                        # freeze warmup-frozen chains: z += BIGM
                        fz = cmin * BC
                        for lo in (OFF_PF, OFF_PB + FDP - fz,
                                   OFF_QF, OFF_QB + FDQ - fz):
                            nc.tensor.matmul(
                                dst[:, 512 + lo:512 + lo + fz],
                                bigm_sb[0:1, :], ones_sb[0:1, 0:fz],
                                start=False, stop=False)
                # nh group: bhh_n broadcast rows
                for di, (off, fd) in enumerate(((OFF_PF, FDP), (OFF_PB, FDP),
                                                (OFF_QF, FDQ), (OFF_QB, FDQ))):
                    nc.tensor.matmul(T2[:, 512 + off:512 + off + fd],
                                     bhnr_sb[0:1, di * HH:(di + 1) * HH],
                                     ones_sb[0:1, 0:fd],
                                     start=(di == 0), stop=False)

            def emit_whh(j):
                T1, T2 = tiles[j]
                DIRS = ((0, OFF_PF, FDP), (1, OFF_PB, FDP),
                        (2, OFF_QF, FDQ), (3, OFF_QB, FDQ))
                for gi, dst, goff in ((0, T1, 0), (1, T1, 512), (2, T2, 512)):
                    for di, (d, off, fd) in enumerate(DIRS):
                        nc.tensor.matmul(
                            dst[:, goff + off:goff + off + fd],
                            whh_sb[:, (d * 3 + gi) * HH:(d * 3 + gi + 1) * HH],
                            hcur[:, off:off + fd],
                            start=False, stop=(di == 3))

            # payload output views (b-major column layout: b*T + c*S + s)
            def view4(x, base, ntok, nch):
                return x[:, base:base + ntok].rearrange(
                    "p (b c s) -> p c b s", b=BC, c=nch, s=S)

            pv_f = view4(pencFB, 0, NTP, NCHP)
            pv_b = view4(pencFB, NTP, NTP, NCHP)
            qv_f = view4(qencFB, 0, NTQ, NCHQ)
            qv_b = view4(qencFB, NTQ, NTQ, NCHQ)

            # ---- the scan ----
            alloc_round(0)
            emit_wih(0)
            alloc_round(1)
            emit_wih(1)
            nc.scalar.activation(nx_sb[:, 0, :], tiles[0][1][:, 0:FDT],
                                 AF.Copy)
            for k in range(RND):
                T1, T2 = tiles[k]
                s, coff, cmin = _round_geom(k)
                emit_whh(k)
                # nh evacuation (psum -> sbuf bf16), overlaps sigmoid
                nc.vector.tensor_scalar_add(nh_sb[:, :], T2[:, 512:512 + FDT],
                                            0.0)
                # r sigmoid on the critical path; z sigmoid off it
                nc.scalar.activation(rz_sb[:, 0:FDT], T1[:, 0:FDT],
                                     AF.Sigmoid)
                nc.scalar.activation(rz_sb[:, FDT:2 * FDT],
                                     T1[:, 512:512 + FDT], AF.Sigmoid)
                nc.vector.tensor_mul(t1_sb[:, :], rz_sb[:, 0:FDT],
                                     nh_sb[:, :])
                nc.vector.tensor_add(t2_sb[:, :], t1_sb[:, :],
                                     nx_sb[:, k % 2, :])
                nc.scalar.activation(n_sb[:, :], t2_sb[:, :], AF.Tanh)
                # h' = n*(1-z) + z*h with (1-z) and z*h computed during the
                # tanh window so only two ops remain after it
                nc.vector.tensor_scalar(out=d_sb[:, :],
                                        in0=rz_sb[:, FDT:2 * FDT],
                                        scalar1=-1.0, scalar2=1.0,
                                        op0=ALU.mult, op1=ALU.add)
                nc.vector.tensor_mul(e_sb[:, :], rz_sb[:, FDT:2 * FDT],
                                     hcur[:, :])
                nc.vector.tensor_mul(t1_sb[:, :], n_sb[:, :], d_sb[:, :])
                nc.vector.tensor_add(hcur[:, :], t1_sb[:, :], e_sb[:, :])
                if k + 2 < RND:
                    alloc_round(k + 2)
                    emit_wih(k + 2)
                if k + 1 < RND:
                    nc.scalar.activation(nx_sb[:, (k + 1) % 2, :],
                                         tiles[k + 1][1][:, 0:FDT], AF.Copy)
                if k >= W:
                    nc.gpsimd.tensor_copy(pv_f[:, :, :, s],
                                          hcur[:, OFF_PF:OFF_PF + FDP]
                                          .rearrange("p (c b) -> p c b", b=BC))
                    nc.gpsimd.tensor_copy(pv_b[:, :, :, S - 1 - s],
                                          hcur[:, OFF_PB:OFF_PB + FDP]
                                          .rearrange("p (c b) -> p c b", b=BC))
                    nc.gpsimd.tensor_copy(qv_f[:, :, :, s],
                                          hcur[:, OFF_QF:OFF_QF + FDQ]
                                          .rearrange("p (c b) -> p c b", b=BC))
                    nc.gpsimd.tensor_copy(qv_b[:, :, :, S - 1 - s],
                                          hcur[:, OFF_QB:OFF_QB + FDQ]
                                          .rearrange("p (c b) -> p c b", b=BC))

            # ---- attention ----
            # keep the PE busy across the scan->attention transition so the
            # p-state stays high; these writes land in the plg tile before
            # its first accumulation group starts, which discards them
            fill = psA.tile([128, 1024], F32, name="plgf", tag="t1")
            for _ in range(12):
                nc.tensor.matmul(fill[:, 0:256], ident_sb[:, :],
                                 epTp[:, 0, 0:256], start=False, stop=False,
                                 skip_group_check=True)

            # qenc transposes first: they only need qencFB
            for hc2 in range(2):
                ptq = psB.tile([128, 512], BF16, name="ptq", tag="t2")
                for i in range(4):
                    b, hc = (hc2 * 4 + i) // 2, (hc2 * 4 + i) % 2
                    nc.tensor.transpose(
                        ptq[0:64, i * 128:(i + 1) * 128],
                        qencFB[:, hc * NTQ + b * Q:hc * NTQ + (b + 1) * Q],
                        ident_sb[:, :])
                nc.vector.tensor_scalar_add(
                    qencT[:, hc2 * 512:(hc2 + 1) * 512], ptq[0:64, :], 0.0)

            # the w1.p and attn_b logit terms are constant across q, so they
            # cancel in the softmax and are never computed
            pqw = psB.tile([1, 512], F32, name="pqw", tag="t2")
            nc.tensor.matmul(pqw[0:1, 0:NTQ], sew_sb[:, 12:13],
                             qencFB[:, 0:NTQ], start=True, stop=False)
            nc.tensor.matmul(pqw[0:1, 0:NTQ], sew_sb[:, 13:14],
                             qencFB[:, NTQ:2 * NTQ], start=False, stop=True)
            nc.vector.tensor_scalar_mul(qenc3[:, 0:NTQ], qencFB[:, 0:NTQ],
                                        outw_sb[:, 4:5])
            nc.vector.tensor_scalar_mul(qenc3[:, NTQ:2 * NTQ],
                                        qencFB[:, NTQ:2 * NTQ],
                                        outw_sb[:, 5:6])
            nc.vector.scalar_tensor_tensor(
                qwm[0:1, :], mq_sb[0:1, :], NEG, pqw[0:1, 0:NTQ],
                op0=ALU.mult, op1=ALU.add)

            # logits for all 16 (b, tcn) blocks into one psum tile; the
            # qwm mask is -1e7 at padded q so exp underflows to exactly 0 --
            # no max-subtraction needed (logits are O(10) bounded).  Bank-1
            # blocks run first with the qwm matmul last (they don't wait on
            # qwm); bank-0 blocks open with the tiny qwm matmul instead so
            # the big penc matmuls never sit blocked at the PE queue head.
            plg = psA.tile([128, 1024], F32, name="plg", tag="t1")

            def logit_group(j, qwm_first):
                b, tcn = j // 4, j % 4
                t0 = b * P + tcn * 128
                o = j * 64
                mm = [(ones_sb[0:1, :], qwm[0:1, b * Q:(b + 1) * Q]),
                      (pencFB[:, t0:t0 + 128], qenc3[:, b * Q:(b + 1) * Q]),
                      (pencFB[:, NTP + t0:NTP + t0 + 128],
                       qenc3[:, NTQ + b * Q:NTQ + (b + 1) * Q])]
                if not qwm_first:
                    mm = mm[1:] + mm[:1]
                for i, (lhsT, rhs) in enumerate(mm):
                    nc.tensor.matmul(plg[:, o:o + 64], lhsT, rhs,
                                     start=(i == 0), stop=(i == 2))

            for j in range(8, 16):
                logit_group(j, qwm_first=False)
            for j in range(8):
                logit_group(j, qwm_first=True)
            exu = pt("exu", [128, 1024], BF16)
            sm16 = pt("sm16", [128, 16], F32)
            rs16 = pt("rs16", [128, 16], F32)
            for hf in (1, 0):
                nc.scalar.activation(exu[:, hf * 512:(hf + 1) * 512],
                                     plg[:, hf * 512:(hf + 1) * 512], AF.Exp)
                nc.vector.tensor_reduce(
                    sm16[:, hf * 8:(hf + 1) * 8],
                    exu[:, hf * 512:(hf + 1) * 512]
                    .rearrange("p (n q) -> p n q", n=8),
                    AX, ALU.add)
                nc.vector.reciprocal(rs16[:, hf * 8:(hf + 1) * 8],
                                     sm16[:, hf * 8:(hf + 1) * 8])
                for j in range(hf * 8, hf * 8 + 8):
                    nc.vector.tensor_scalar_mul(
                        probs[:, j * 64:(j + 1) * 64],
                        exu[:, j * 64:(j + 1) * 64], rs16[:, j:j + 1])

            for b in (2, 3, 0, 1):
                ptb = psB.tile([128, 512], BF16, name="ptb", tag="t2")
                for tcn in range(4):
                    nc.tensor.transpose(
                        ptb[0:64, tcn * 128:(tcn + 1) * 128],
                        probs[:, (b * 4 + tcn) * 64:(b * 4 + tcn + 1) * 64],
                        ident_sb[:, :])
                if b % 2 == 0:
                    nc.scalar.activation(probsT[:, b * P:(b + 1) * P],
                                         ptb[0:64, :], AF.Copy)
                else:
                    nc.vector.tensor_scalar_add(probsT[:, b * P:(b + 1) * P],
                                                ptb[0:64, :], 0.0)

            for b in (2, 3, 0, 1):
                for hc in range(2):
                    paw = psA.tile([128, 1024], F32, name="paw", tag="t1")
                    nc.tensor.matmul(
                        paw[:, 0:P],
                        qencT[0:64, (b * 2 + hc) * 128:(b * 2 + hc + 1) * 128],
                        probsT[0:64, b * P:(b + 1) * P], start=True, stop=True)
                    dst = attwFB[:, hc * NTP + b * P:hc * NTP + (b + 1) * P]
                    if (b + hc) % 2 == 0:
                        nc.scalar.activation(dst, paw[:, 0:P], AF.Copy)
                    else:
                        nc.vector.tensor_scalar_add(dst, paw[:, 0:P], 0.0)
            for b in (2, 3, 0, 1):
                for hc in range(2):
                    o = hc * NTP + b * P
                    nc.vector.tensor_mul(pawFB[:, o:o + P],
                                         pencFB[:, o:o + P],
                                         attwFB[:, o:o + P])

            for bp in (1, 0):
                pse = psB.tile([2, 1024], F32, name="pse", tag="t2")
                for bi in range(2):
                    b = bp * 2 + bi
                    rhss = (pencFB[:, b * P:(b + 1) * P],
                            pencFB[:, NTP + b * P:NTP + (b + 1) * P],
                            attwFB[:, b * P:(b + 1) * P],
                            attwFB[:, NTP + b * P:NTP + (b + 1) * P],
                            pawFB[:, b * P:(b + 1) * P],
                            pawFB[:, NTP + b * P:NTP + (b + 1) * P])
                    for j, rhs in enumerate(rhss):
                        nc.tensor.matmul(
                            pse[0:2, bi * P:(bi + 1) * P],
                            sew_sb[:, 2 * j:2 * j + 2],
                            rhs, start=(j == 0), stop=(j == 5))
                nc.scalar.activation(se_sb[0:2, bp * 2 * P:(bp * 2 + 2) * P],
                                     pse[0:2, :],
                                     AF.Identity, bias=outw_sb[0:2, 3:4])
                # fan this half out to se8 rows on two different DMA queues
                c0 = bp * 2 * P
                nc.gpsimd.dma_start(se8[bp * 2:bp * 2 + 2, :],
                                    se_sb[0:1, c0:c0 + 2 * P])
                nc.sync.dma_start(se8[BC + bp * 2:BC + bp * 2 + 2, :],
                                  se_sb[1:2, c0:c0 + 2 * P])
            nc.vector.copy_predicated(se8[:, :], m8_sb[:, :], neg_sb[:, :])

            # log-softmax without max-subtraction: valid entries are O(10),
            # -1e7 pads underflow exp to 0
            nc.scalar.activation(lse_sb[:, :], se8[:, :], AF.Exp)
            nc.vector.tensor_reduce(red_sb[:, 2:3], lse_sb[:, :], AX, ALU.add)
            nc.scalar.activation(red_sb[:, 3:4], red_sb[:, 2:3], AF.Ln)
            nc.vector.tensor_scalar(out=lsm_sb[:, :], in0=se8[:, :],
                                    scalar1=red_sb[:, 3:4], scalar2=None,
                                    op0=ALU.subtract)

            nc.sync.dma_start(out[0:2 * BC, :], se8[:, :])
            nc.sync.dma_start(out[2 * BC:4 * BC, :], lsm_sb[:, :])

    _split_multiwaits(nc)
    return nc, es


def _split_multiwaits(nc):
    """HW instruction encodings hold a single semaphore wait; move extra
    waits emitted by Tile onto same-engine NOPs inserted just before."""
    for b in nc.main_func.blocks:
        il = b.instructions
        newlist = []
        for inst in il:
            if type(inst).__name__ == "InstISA":
                # EVENT_SEMAPHORE_RANGE_CLEAR mis-encodes for this walrus
                # build; NRT clears semaphores per execution anyway.
                continue
            si = inst.sync_info
            if si is not None and len(si.on_wait) > 1:
                waits = list(si.on_wait)
                for wx in waits[:-1]:
                    nop = nc.engines[inst.engine].nop(hint="wsplit").ins
                    # remove from wherever nop() appended it
                    for bb in nc.main_func.blocks:
                        try:
                            bb.instructions.remove(nop)
                            break
                        except ValueError:
                            pass
                    nop.sync_info = mybir.SyncInfo(on_wait=[wx], on_update=[])
                    newlist.append(nop)
                inst.sync_info = mybir.SyncInfo(on_wait=[waits[-1]],
                                                on_update=list(si.on_update))
            newlist.append(inst)
        il[:] = newlist


def _perm_tokens(tok2d, nch, blk):
    """Token array (BC, T) -> s-major column order: col = rank(s)*blk + c*BC + b."""
    T = tok2d.shape[1]
    cols = np.empty(BC * T, np.int64)
    for rank in range(S):
        s = SORD[rank]
        blkv = tok2d[:, s::S]          # (BC, nch) tokens at pos s per chunk
        # col index rank*blk + c*BC + b
        cols[rank * blk:(rank + 1) * blk] = blkv.T.reshape(-1)
    return cols


def _prep_core(inputs, c):
    bs = slice(c * BC, (c + 1) * BC)
    ptok = np.asarray(inputs["passage"][bs]).astype(np.int64)
    qtok = np.asarray(inputs["question"][bs]).astype(np.int64)
    embp = inputs["_embp"]
    pcols = _perm_tokens(ptok, NCHP, FDP)
    qcols = _perm_tokens(qtok, NCHQ, FDQ)
    d = {}
    d["epTp_d"] = np.ascontiguousarray(
        embp[pcols].T.reshape(3, 128, NTP).transpose(1, 0, 2).reshape(128, -1))
    d["epTq_d"] = np.ascontiguousarray(
        embp[qcols].T.reshape(3, 128, NTQ).transpose(1, 0, 2).reshape(128, -1))
    qm0 = (qtok.reshape(-1) == 0).astype(np.float32)
    d["mq"] = np.ascontiguousarray(qm0[None, :])
    pm2 = (ptok.reshape(-1) == 0).reshape(BC, P).astype(np.uint8)
    d["m8"] = np.ascontiguousarray(np.concatenate([pm2, pm2], axis=0))
    return d


def _prep_shared(inputs):
    bf = ml_dtypes.bfloat16

    wihT = np.zeros((4, 3, 128, 3 * HH), bf)      # (d, kc, p, m)
    whhT = np.zeros((4, HH, 3 * HH), bf)          # (d, p, m)
    bhnr = np.zeros((4, HH), bf)
    for di, (pre, dd) in enumerate((("p", "f"), ("p", "b"),
                                    ("q", "f"), ("q", "b"))):
        wih = np.asarray(inputs[f"{pre}_wih_{dd}"], np.float32)
        whh = np.asarray(inputs[f"{pre}_whh_{dd}"], np.float32)
        bih = np.asarray(inputs[f"{pre}_bih_{dd}"], np.float32)
        bhh = np.asarray(inputs[f"{pre}_bhh_{dd}"], np.float32)
        wT = np.zeros((EPAD, 3 * HH), np.float32)
        wT[:E, :] = wih.T
        # row 300: pad-token indicator -> +BIGM on the z gate
        wT[300, HH:2 * HH] = BIGM
        # row 301: constant-1 -> gate biases (bih+bhh for r/z, bih for n)
        wT[301, 0:HH] = bih[0:HH] + bhh[0:HH]
        wT[301, HH:2 * HH] = bih[HH:2 * HH] + bhh[HH:2 * HH]
        wT[301, 2 * HH:] = bih[2 * HH:]
        wihT[di] = wT.astype(bf).reshape(3, 128, 3 * HH)
        whhT[di] = whh.T.astype(bf)
        bhnr[di] = bhh[2 * HH:].astype(bf)
    wihT = np.ascontiguousarray(
        wihT.transpose(2, 0, 1, 3).reshape(128, -1))      # (p,(d,kc,m))
    whhT = np.ascontiguousarray(
        whhT.transpose(1, 0, 2).reshape(128, -1))         # (p,(d,m))
    bhnr = np.ascontiguousarray(bhnr.reshape(1, -1))

    aw = np.asarray(inputs["attn_w"], np.float32)
    w1, w2, w3 = aw[:256], aw[256:512], aw[512:]
    outw = np.zeros((HH, 8), np.float32)
    outw[:, 4], outw[:, 5] = w3[:128], w3[128:]
    outw[0, 2] = float(np.asarray(inputs["attn_b"]))
    outw[0, 3] = float(np.asarray(inputs["start_b"]))
    outw[1, 3] = float(np.asarray(inputs["end_b"]))

    sw = np.asarray(inputs["start_w"], np.float32)
    ew = np.asarray(inputs["end_w"], np.float32)
    sew = np.zeros((HH, 14), bf)
    for j in range(6):
        sew[:, 2 * j] = sw[j * 128:(j + 1) * 128].astype(bf)
        sew[:, 2 * j + 1] = ew[j * 128:(j + 1) * 128].astype(bf)
    sew[:, 12] = w2[:128].astype(bf)
    sew[:, 13] = w2[128:].astype(bf)
    return {"wihT": wihT, "whhT": whhT, "bhnr": bhnr,
            "outw": outw, "sew": sew}


def kernel(**inputs):
    if "nc" not in _CACHE:
        _CACHE["nc"] = _build_nc()
    nc, _es = _CACHE["nc"]
    shared = _prep_shared(inputs)
    bf = ml_dtypes.bfloat16
    embp = np.zeros((VOCAB, EPAD), bf)
    embp[:, :E] = np.asarray(inputs["emb"], np.float32).astype(bf)
    embp[0, 300] = 1.0   # pad-token indicator row
    embp[:, 301] = 1.0   # constant-1 bias row
    inputs = dict(inputs)
    inputs["_embp"] = embp
    in_maps = []
    for c in range(NC):
        m = dict(shared)
        m.update(_prep_core(inputs, c))
        in_maps.append(m)
    res = run_bass_kernel_spmd(nc, in_maps, list(range(NC)))
    outs = [np.asarray(res.results[c]["out"]) for c in range(NC)]
    se = np.concatenate([o[0:2 * BC].reshape(2, BC, P) for o in outs], axis=1)
    lsm = np.concatenate([o[2 * BC:].reshape(2, BC, P) for o in outs], axis=1)
    return (np.ascontiguousarray(se[0]), np.ascontiguousarray(se[1]),
            np.ascontiguousarray(lsm[0]), np.ascontiguousarray(lsm[1]))


# revision 37
# speedup vs baseline: 2.8761x; 1.0579x over previous
"""AttentionRNN (BiDAF-style QA reader) Trainium2 kernel.

Per core (pure data-parallel over batch, 4 of 32 rows per core):
  1. Host gathers embeddings in an s-major permuted token order and pads two
     extra embedding rows: row 300 = pad-token indicator (drives a +BIGM into
     the z gate via the weight matrix, freezing h at padded steps), row 301 =
     constant 1.0 (injects the gate biases).  So each scan round's x-gate
     pre-activations are plain contiguous-slice matmuls.
  2. GRU scans as chunked-parallel recurrences: chunks of S=16 payload steps
     with W=12 warmup steps re-run from h=0 (the GRU contracts ~0.6/step).
     Chunks whose warmup would cross t=0 are frozen (z pinned via +BIGM)
     until their true start.  One round = one time step of 72 chains; the
     x-projection matmuls for round k+1 are issued ahead of round k's
     recurrent matmuls so the PE stays busy during the serial chain.
  3. Decomposed BiDAF attention, softmax over Q, start/end heads, log-softmax
     over P; padded positions forced to exactly -1e7 as in the reference.
"""

import contextlib

import numpy as np
import ml_dtypes

import concourse.bass as bass
import concourse.mybir as mybir
from concourse.masks import make_identity
from concourse.tile import TileContext
from concourse.bass_utils import run_bass_kernel_spmd

F32 = mybir.dt.float32
BF16 = mybir.dt.bfloat16
U8 = mybir.dt.uint8
AX = mybir.AxisListType.X
ALU = mybir.AluOpType
AF = mybir.ActivationFunctionType

B, P, Q, E, H, VOCAB = 32, 512, 64, 300, 256, 50000
HH = 128
EPAD = 384
NC = 8
BC = B // NC
NEG = -1e7
BIGM = 1.0e4

S, W = 16, 12
RND = S + W                   # 28 rounds
NCHP, NCHQ = P // S, Q // S   # 32, 4
FDP, FDQ = NCHP * BC, NCHQ * BC   # 128, 16
OFF_PF, OFF_PB, OFF_QF, OFF_QB = 0, FDP, 2 * FDP, 2 * FDP + FDQ
FDT = 2 * FDP + 2 * FDQ       # 288

NTP, NTQ = BC * P, BC * Q     # 2048, 256

_CACHE = {}


def _mk_rank():
    order, seen = [], set()
    for k in range(RND):
        s = (k - W) % S
        for v in (s, S - 1 - s):
            if v not in seen:
                seen.add(v)
                order.append(v)
    rank = [0] * S
    for i, s in enumerate(order):
        rank[s] = i
    return order, rank


SORD, SRANK = _mk_rank()


def _round_geom(k):
    e = k - W
    s = e % S
    coff = (e - s) // S                      # -1 | 0
    cmin = (W - k + S - 1) // S if k < W else 0
    return s, coff, cmin


def _build_nc():
    nc = bass.Bass()

    epTp_d = nc.declare_dram_parameter("epTp_d", [128, 3 * NTP], BF16,
                                       isOutput=False)
    epTq_d = nc.declare_dram_parameter("epTq_d", [128, 3 * NTQ], BF16,
                                       isOutput=False)
    mq_d = nc.declare_dram_parameter("mq", [1, NTQ], F32, isOutput=False)
    m8 = nc.declare_dram_parameter("m8", [2 * BC, P], U8, isOutput=False)
    wihT = nc.declare_dram_parameter("wihT", [128, 4 * 3 * 3 * HH], BF16,
                                     isOutput=False)
    whhT = nc.declare_dram_parameter("whhT", [128, 4 * 3 * HH], BF16,
                                     isOutput=False)
    bhnr_d = nc.declare_dram_parameter("bhnr", [1, 4 * HH], BF16,
                                       isOutput=False)
    outw = nc.declare_dram_parameter("outw", [HH, 8], F32, isOutput=False)
    seww = nc.declare_dram_parameter("sew", [HH, 14], BF16, isOutput=False)
    out = nc.declare_dram_parameter("out", [4 * BC, P], F32, isOutput=True)

    es = contextlib.ExitStack()

    # ---------- Tile phases ----------
    with TileContext(nc) as tc:
        with tc.tile_pool(name="psA", bufs=2, space="PSUM") as psA, \
             tc.tile_pool(name="psB", bufs=2, space="PSUM") as psB, \
             tc.tile_pool(name="sbp", bufs=2) as sbp, \
             tc.tile_pool(name="pst", bufs=1) as pst:

            def pt(name, shape, dtype):
                return pst.tile(shape, dtype, name=name, tag=name)

            neg_sb = pt("neg_sb", [2 * BC, P], F32)
            ones_sb = pt("ones_sb", [1, 128], BF16)
            bigm_sb = pt("bigm_sb", [1, 128], BF16)

            # input tiles (DMA-streamed)
            epTp = pt("epTp", [128, 3, NTP], BF16)
            epTq = pt("epTq", [128, 3, NTQ], BF16)
            wih_sb = pt("wih_sb", [128, 4 * 3 * 3 * HH], BF16)
            whh_sb = pt("whh_sb", [128, 4 * 3 * HH], BF16)
            bhnr_sb = pt("bhnr_sb", [1, 4 * HH], BF16)
            mq_sb = pt("mq_sb", [1, NTQ], F32)
            m8_sb = pt("m8_sb", [2 * BC, P], U8)
            outw_sb = pt("outw_sb", [128, 8], F32)
            sew_sb = pt("sew_sb", [128, 14], BF16)

            # scan state, one copy per stream (0 = forward, 1 = backward)
            FDH = FDP + FDQ   # 144: p chains at [0:128], q chains [128:144]
            pencFB = pt("pencFB", [128, 2 * NTP], BF16)
            qencFB = pt("qencFB", [128, 2 * NTQ], BF16)
            hcur = [pt(f"hcur{s}", [128, FDH], BF16) for s in range(2)]
            rz_sb = [pt(f"rz_sb{s}", [128, 2 * FDH], BF16) for s in range(2)]
            nh_sb = [pt(f"nh_sb{s}", [128, FDH], BF16) for s in range(2)]
            nx_sb = [pt(f"nx_sb{s}", [128, 2, FDH], BF16) for s in range(2)]
            t1_sb = [pt(f"t1_sb{s}", [128, FDH], BF16) for s in range(2)]
            t2_sb = [pt(f"t2_sb{s}", [128, FDH], BF16) for s in range(2)]
            n_sb = [pt(f"n_sb{s}", [128, FDH], BF16) for s in range(2)]
            d_sb = [pt(f"d_sb{s}", [128, FDH], BF16) for s in range(2)]
            e_sb = [pt(f"e_sb{s}", [128, FDH], BF16) for s in range(2)]
            ident_sb = pt("ident_sb", [128, 128], BF16)
            # attention tiles
            qenc3 = pt("qenc3", [128, 2 * NTQ], BF16)
            qwm = pt("qwm", [1, NTQ], BF16)
            qwt = pt("qwt", [1, NTQ], F32)
            probs = pt("probs", [128, 64 * 4 * BC], BF16)
            probsT = pt("probsT", [64, P * BC], BF16)
            qencT = pt("qencT", [64, 2 * HH * BC], BF16)
            attwFB = pt("attwFB", [128, 2 * NTP], BF16)
            pawFB = pt("pawFB", [128, 2 * NTP], BF16)
            se_sb = pt("se_sb", [2, BC * P], F32)
            se8 = pt("se8", [2 * BC, P], F32)
            lsm_sb = pt("lsm_sb", [2 * BC, P], F32)
            lse_sb = pt("lse_sb", [2 * BC, P], F32)
            red_sb = pt("red_sb", [2 * BC, 8], F32)

            # ---- input DMA (gpsimd queue: cheap issue), priority order ----
            g = nc.gpsimd
            HW12 = 4 * 3 * 3 * HH // 2
            g.dma_start(wih_sb[:, 0:HW12], wihT[:, 0:HW12])       # p dirs
            epv = epTp_d[:, :].rearrange("p (c t) -> p c t", c=3)
            g.dma_start(epTp[:, :, 0:4 * FDP], epv[:, :, 0:4 * FDP])
            g.dma_start(wih_sb[:, HW12:], wihT[:, HW12:])         # q dirs
            g.dma_start(epTq[:, :, :],
                        epTq_d[:, :].rearrange("p (c t) -> p c t", c=3))
            g.dma_start(whh_sb[:, :], whhT[:, :])
            g.dma_start(bhnr_sb[:, :], bhnr_d[:, :])
            for r0, r1 in ((4, 6), (6, 8), (8, 16)):
                g.dma_start(epTp[:, :, r0 * FDP:r1 * FDP],
                            epv[:, :, r0 * FDP:r1 * FDP])
            g.dma_start(mq_sb[:, :], mq_d[:, :])
            g.dma_start(m8_sb[:, :], m8[:, :])
            g.dma_start(outw_sb[0:HH, :], outw[:, :])
            g.dma_start(sew_sb[0:HH, :], seww[:, :])

            nc.vector.memset(ones_sb[:, :], 1.0)
            nc.vector.memset(bigm_sb[:, :], BIGM)
            nc.vector.memset(hcur[0][:, :], 0)
            nc.vector.memset(hcur[1][:, :], 0)
            nc.vector.memset(neg_sb[:, :], NEG)
            make_identity(nc, ident_sb[:, :])

            # Two independent half-width streams (0 = forward dirs, 1 =
            # backward dirs), self-staggered ~half a round apart so their
            # serial chains interleave on the engines.  Per stream-round two
            # single-bank psum tiles: T1 = r [0:144] | z [144:288] (one
            # accumulation group), T2 = nx [0:144] | nh [144:288] (one group).
            tiles = [[None] * RND, [None] * RND]
            pool_s = (psA, psB)

            def alloc_round(st, j):
                tiles[st][j] = (
                    pool_s[st].tile([128, 512], F32, name=f"T1{st}", tag="a"),
                    pool_s[st].tile([128, 512], F32, name=f"T2{st}", tag="b"))

            # per-stream x-projection geometry for round j: (epT, src col,
            # psum dst col, width) for the p part and the q part
            def xgeom(st, j):
                s, coff, cmin = _round_geom(j)
                res = []
                for (nch, epT, blk, qoff) in ((NCHP, epTp, FDP, 0),
                                              (NCHQ, epTq, FDQ, FDP)):
                    cnt = nch - cmin
                    if st == 0:
                        res.append((epT, SRANK[s] * blk + (cmin + coff) * BC,
                                    qoff + cmin * BC, cnt * BC))
                    else:
                        res.append((epT,
                                    SRANK[S - 1 - s] * blk + (-coff) * BC,
                                    qoff, cnt * BC))
                return res

            def emit_wih(st, j):
                T1, T2 = tiles[st][j]
                s, coff, cmin = _round_geom(j)
                geo = xgeom(st, j)
                dirs = (0, 2) if st == 0 else (1, 3)
                # T1 group: wih_r + wih_z (+ bigm), closed later by whh_r/z
                first = True
                for gate, goff in ((0, 0), (1, FDH)):
                    for gi, di in enumerate(dirs):
                        epT, c0, o0, wd = geo[gi]
                        for kc in range(3):
                            wcol = ((di * 3 + kc) * 3 + gate) * HH
                            nc.tensor.matmul(
                                T1[:, goff + o0:goff + o0 + wd],
                                wih_sb[:, wcol:wcol + HH],
                                epT[:, kc, c0:c0 + wd],
                                start=first, stop=False)
                            first = False
                if cmin > 0:
                    # freeze warmup-frozen chains: z += BIGM
                    fz = cmin * BC
                    los = ((FDH, FDH + FDP) if st == 0
                           else (FDH + FDP - fz, FDH + FDH - fz))
                    for lo in los:
                        nc.tensor.matmul(
                            T1[:, lo:lo + fz],
                            bigm_sb[0:1, :], ones_sb[0:1, 0:fz],
                            start=False, stop=False)
                # T2 group: wih_n + bhh_n rows, closed later by whh_n
                first = True
                for gi, di in enumerate(dirs):
                    epT, c0, o0, wd = geo[gi]
                    for kc in range(3):
                        wcol = ((di * 3 + kc) * 3 + 2) * HH
                        nc.tensor.matmul(
                            T2[:, o0:o0 + wd],
                            wih_sb[:, wcol:wcol + HH],
                            epT[:, kc, c0:c0 + wd],
                            start=first, stop=False)
                        first = False
                for gi, di in enumerate(dirs):
                    off, fd = (0, FDP) if gi == 0 else (FDP, FDQ)
                    nc.tensor.matmul(T2[:, FDH + off:FDH + off + fd],
                                     bhnr_sb[0:1, di * HH:(di + 1) * HH],
                                     ones_sb[0:1, 0:fd],
                                     start=False, stop=False)

            def emit_whh(st, j):
                T1, T2 = tiles[st][j]
                dirs = (0, 2) if st == 0 else (1, 3)
                for gi, goff, dst in ((0, 0, T1), (1, FDH, T1), (2, FDH, T2)):
                    for ii, di in enumerate(dirs):
                        off, fd = (0, FDP) if ii == 0 else (FDP, FDQ)
                        nc.tensor.matmul(
                            dst[:, goff + off:goff + off + fd],
                            whh_sb[:, (di * 3 + gi) * HH:(di * 3 + gi + 1) * HH],
                            hcur[st][:, off:off + fd],
                            start=False, stop=(gi >= 1 and ii == 1))

            # payload output views (b-major column layout: b*T + c*S + s)
            def view4(x, base, ntok, nch):
                return x[:, base:base + ntok].rearrange(
                    "p (b c s) -> p c b s", b=BC, c=nch, s=S)

            pv = (view4(pencFB, 0, NTP, NCHP), view4(pencFB, NTP, NTP, NCHP))
            qv = (view4(qencFB, 0, NTQ, NCHQ), view4(qencFB, NTQ, NTQ, NCHQ))

            # ---- the scan ----
            for st in range(2):
                alloc_round(st, 0)
                emit_wih(st, 0)
            for st in range(2):
                nc.vector.tensor_scalar_add(nx_sb[st][:, 0, :],
                                            tiles[st][0][1][:, 0:FDH], 0.0)
            for k in range(RND):
                s, coff, cmin = _round_geom(k)
                for st in range(2):
                    T1, T2 = tiles[st][k]
                    rz, nh, nx = rz_sb[st], nh_sb[st], nx_sb[st]
                    t1, t2, n_, d_, e_ = (t1_sb[st], t2_sb[st], n_sb[st],
                                          d_sb[st], e_sb[st])
                    hc = hcur[st]
                    emit_whh(st, k)
                    if k + 1 < RND:
                        alloc_round(st, k + 1)
                        emit_wih(st, k + 1)
                    # nh evacuation (psum -> sbuf bf16), overlaps sigmoid
                    nc.vector.tensor_scalar_add(nh[:, :],
                                                T2[:, FDH:2 * FDH], 0.0)
                    # r sigmoid on the critical path; z sigmoid off it
                    nc.scalar.activation(rz[:, 0:FDH], T1[:, 0:FDH],
                                         AF.Sigmoid)
                    nc.scalar.activation(rz[:, FDH:2 * FDH],
                                         T1[:, FDH:2 * FDH], AF.Sigmoid)
                    nc.vector.tensor_mul(t1[:, :], rz[:, 0:FDH], nh[:, :])
                    nc.vector.tensor_add(t2[:, :], t1[:, :],
                                         nx[:, k % 2, :])
                    nc.scalar.activation(n_[:, :], t2[:, :], AF.Tanh)
                    # h' = n*(1-z) + z*h: (1-z) on DVE and z*h on Pool run
                    # during the tanh window; two ops remain after it
                    nc.vector.tensor_scalar(out=d_[:, :],
                                            in0=rz[:, FDH:2 * FDH],
                                            scalar1=-1.0, scalar2=1.0,
                                            op0=ALU.mult, op1=ALU.add)
                    nc.gpsimd.tensor_mul(e_[:, :], rz[:, FDH:2 * FDH],
                                         hc[:, :])
                    nc.vector.tensor_mul(t1[:, :], n_[:, :], d_[:, :])
                    nc.vector.tensor_add(hc[:, :], t1[:, :], e_[:, :])
                    if k + 1 < RND:
                        nc.vector.tensor_scalar_add(
                            nx[:, (k + 1) % 2, :],
                            tiles[st][k + 1][1][:, 0:FDH], 0.0)
                    if k >= W:
                        sp = s if st == 0 else S - 1 - s
                        nc.gpsimd.tensor_copy(
                            pv[st][:, :, :, sp],
                            hc[:, 0:FDP]
                            .rearrange("p (c b) -> p c b", b=BC))
                        nc.gpsimd.tensor_copy(
                            qv[st][:, :, :, sp],
                            hc[:, FDP:FDH]
                            .rearrange("p (c b) -> p c b", b=BC))

            # ---- attention ----
            # keep the PE busy across the scan->attention transition so the
            # p-state stays high; these writes land in the plg tile before
            # its first accumulation group starts, which discards them
            fill = psA.tile([128, 512], F32, name="plgf", tag="a")
            for _ in range(12):
                nc.tensor.matmul(fill[:, 0:256], ident_sb[:, :],
                                 epTp[:, 0, 0:256], start=False, stop=False,
                                 skip_group_check=True)

            # qenc transposes first: they only need qencFB
            for hc2 in range(2):
                ptq = psB.tile([128, 512], BF16, name="ptq", tag="a")
                for i in range(4):
                    b, hc = (hc2 * 4 + i) // 2, (hc2 * 4 + i) % 2
                    nc.tensor.transpose(
                        ptq[0:64, i * 128:(i + 1) * 128],
                        qencFB[:, hc * NTQ + b * Q:hc * NTQ + (b + 1) * Q],
                        ident_sb[:, :])
                nc.vector.tensor_scalar_add(
                    qencT[:, hc2 * 512:(hc2 + 1) * 512], ptq[0:64, :], 0.0)

            # the w1.p and attn_b logit terms are constant across q, so they
            # cancel in the softmax and are never computed
            pqw = psB.tile([1, 512], F32, name="pqw", tag="b")
            nc.tensor.matmul(pqw[0:1, 0:NTQ], sew_sb[:, 12:13],
                             qencFB[:, 0:NTQ], start=True, stop=False)
            nc.tensor.matmul(pqw[0:1, 0:NTQ], sew_sb[:, 13:14],
                             qencFB[:, NTQ:2 * NTQ], start=False, stop=True)
            nc.vector.tensor_scalar_mul(qenc3[:, 0:NTQ], qencFB[:, 0:NTQ],
                                        outw_sb[:, 4:5])
            nc.vector.tensor_scalar_mul(qenc3[:, NTQ:2 * NTQ],
                                        qencFB[:, NTQ:2 * NTQ],
                                        outw_sb[:, 5:6])
            nc.vector.scalar_tensor_tensor(
                qwm[0:1, :], mq_sb[0:1, :], NEG, pqw[0:1, 0:NTQ],
                op0=ALU.mult, op1=ALU.add)

            # logits for all 16 (b, tcn) blocks into one psum tile; the
            # qwm mask is -1e7 at padded q so exp underflows to exactly 0 --
            # no max-subtraction needed (logits are O(10) bounded).  Bank-1
            # blocks run first with the qwm matmul last (they don't wait on
            # qwm); bank-0 blocks open with the tiny qwm matmul instead so
            # the big penc matmuls never sit blocked at the PE queue head.
            plgs = [psA.tile([128, 512], F32, name=f"plg{h}",
                             tag="a" if h == 0 else "b")
                    for h in range(2)]

            def logit_group(j, qwm_first):
                b, tcn = j // 4, j % 4
                t0 = b * P + tcn * 128
                o = (j % 8) * 64
                plg = plgs[j // 8]
                mm = [(ones_sb[0:1, :], qwm[0:1, b * Q:(b + 1) * Q]),
                      (pencFB[:, t0:t0 + 128], qenc3[:, b * Q:(b + 1) * Q]),
                      (pencFB[:, NTP + t0:NTP + t0 + 128],
                       qenc3[:, NTQ + b * Q:NTQ + (b + 1) * Q])]
                if not qwm_first:
                    mm = mm[1:] + mm[:1]
                for i, (lhsT, rhs) in enumerate(mm):
                    nc.tensor.matmul(plg[:, o:o + 64], lhsT, rhs,
                                     start=(i == 0), stop=(i == 2))

            for j in range(8, 16):
                logit_group(j, qwm_first=False)
            for j in range(8):
                logit_group(j, qwm_first=True)
            exu = pt("exu", [128, 1024], BF16)
            sm16 = pt("sm16", [128, 16], F32)
            rs16 = pt("rs16", [128, 16], F32)
            for hf in (1, 0):
                nc.scalar.activation(exu[:, hf * 512:(hf + 1) * 512],
                                     plgs[hf][:, 0:512], AF.Exp)
                nc.vector.tensor_reduce(
                    sm16[:, hf * 8:(hf + 1) * 8],
                    exu[:, hf * 512:(hf + 1) * 512]
                    .rearrange("p (n q) -> p n q", n=8),
                    AX, ALU.add)
                nc.vector.reciprocal(rs16[:, hf * 8:(hf + 1) * 8],
                                     sm16[:, hf * 8:(hf + 1) * 8])
                for j in range(hf * 8, hf * 8 + 8):
                    nc.vector.tensor_scalar_mul(
                        probs[:, j * 64:(j + 1) * 64],
                        exu[:, j * 64:(j + 1) * 64], rs16[:, j:j + 1])

            for b in (2, 3, 0, 1):
                ptb = psB.tile([128, 512], BF16, name="ptb", tag="a")
                for tcn in range(4):
                    nc.tensor.transpose(
                        ptb[0:64, tcn * 128:(tcn + 1) * 128],
                        probs[:, (b * 4 + tcn) * 64:(b * 4 + tcn + 1) * 64],
                        ident_sb[:, :])
                if b % 2 == 0:
                    nc.scalar.activation(probsT[:, b * P:(b + 1) * P],
                                         ptb[0:64, :], AF.Copy)
                else:
                    nc.vector.tensor_scalar_add(probsT[:, b * P:(b + 1) * P],
                                                ptb[0:64, :], 0.0)

            for b in (2, 3, 0, 1):
                for hc in range(2):
                    paw = psA.tile([128, 512], F32, name="paw",
                                   tag="a" if (b + hc) % 2 == 0 else "b")
                    nc.tensor.matmul(
                        paw[:, 0:P],
                        qencT[0:64, (b * 2 + hc) * 128:(b * 2 + hc + 1) * 128],
                        probsT[0:64, b * P:(b + 1) * P], start=True, stop=True)
                    dst = attwFB[:, hc * NTP + b * P:hc * NTP + (b + 1) * P]
                    if (b + hc) % 2 == 0:
                        nc.scalar.activation(dst, paw[:, 0:P], AF.Copy)
                    else:
                        nc.vector.tensor_scalar_add(dst, paw[:, 0:P], 0.0)
            for b in (2, 3, 0, 1):
                for hc in range(2):
                    o = hc * NTP + b * P
                    nc.vector.tensor_mul(pawFB[:, o:o + P],
                                         pencFB[:, o:o + P],
                                         attwFB[:, o:o + P])

            for bp in (1, 0):
                for bi in range(2):
                    b = bp * 2 + bi
                    pse = psB.tile([2, 512], F32, name="pse", tag="b")
                    rhss = (pencFB[:, b * P:(b + 1) * P],
                            pencFB[:, NTP + b * P:NTP + (b + 1) * P],
                            attwFB[:, b * P:(b + 1) * P],
                            attwFB[:, NTP + b * P:NTP + (b + 1) * P],
                            pawFB[:, b * P:(b + 1) * P],
                            pawFB[:, NTP + b * P:NTP + (b + 1) * P])
                    for j, rhs in enumerate(rhss):
                        nc.tensor.matmul(
                            pse[0:2, 0:P],
                            sew_sb[:, 2 * j:2 * j + 2],
                            rhs, start=(j == 0), stop=(j == 5))
                    nc.scalar.activation(se_sb[0:2, b * P:(b + 1) * P],
                                         pse[0:2, 0:P],
                                         AF.Identity, bias=outw_sb[0:2, 3:4])
                # fan this half out to se8 rows on two different DMA queues
                c0 = bp * 2 * P
                nc.gpsimd.dma_start(se8[bp * 2:bp * 2 + 2, :],
                                    se_sb[0:1, c0:c0 + 2 * P])
                nc.sync.dma_start(se8[BC + bp * 2:BC + bp * 2 + 2, :],
                                  se_sb[1:2, c0:c0 + 2 * P])
            nc.vector.copy_predicated(se8[:, :], m8_sb[:, :], neg_sb[:, :])

            # log-softmax without max-subtraction: valid entries are O(10),
            # -1e7 pads underflow exp to 0
            nc.scalar.activation(lse_sb[:, :], se8[:, :], AF.Exp)
            nc.vector.tensor_reduce(red_sb[:, 2:3], lse_sb[:, :], AX, ALU.add)
            nc.scalar.activation(red_sb[:, 3:4], red_sb[:, 2:3], AF.Ln)
            nc.vector.tensor_scalar(out=lsm_sb[:, :], in0=se8[:, :],
                                    scalar1=red_sb[:, 3:4], scalar2=None,
                                    op0=ALU.subtract)

            nc.sync.dma_start(out[0:2 * BC, :], se8[:, :])
            nc.sync.dma_start(out[2 * BC:4 * BC, :], lsm_sb[:, :])

    _split_multiwaits(nc)
    return nc, es


def _split_multiwaits(nc):
    """HW instruction encodings hold a single semaphore wait; move extra
    waits emitted by Tile onto same-engine NOPs inserted just before."""
    for b in nc.main_func.blocks:
        il = b.instructions
        newlist = []
        for inst in il:
            if type(inst).__name__ == "InstISA":
                # EVENT_SEMAPHORE_RANGE_CLEAR mis-encodes for this walrus
                # build; NRT clears semaphores per execution anyway.
                continue
            si = inst.sync_info
            if si is not None and len(si.on_wait) > 1:
                waits = list(si.on_wait)
                for wx in waits[:-1]:
                    nop = nc.engines[inst.engine].nop(hint="wsplit").ins
                    # remove from wherever nop() appended it
                    for bb in nc.main_func.blocks:
                        try:
                            bb.instructions.remove(nop)
                            break
                        except ValueError:
                            pass
                    nop.sync_info = mybir.SyncInfo(on_wait=[wx], on_update=[])
                    newlist.append(nop)
                inst.sync_info = mybir.SyncInfo(on_wait=[waits[-1]],
                                                on_update=list(si.on_update))
            newlist.append(inst)
        il[:] = newlist


def _perm_tokens(tok2d, nch, blk):
    """Token array (BC, T) -> s-major column order: col = rank(s)*blk + c*BC + b."""
    T = tok2d.shape[1]
    cols = np.empty(BC * T, np.int64)
    for rank in range(S):
        s = SORD[rank]
        blkv = tok2d[:, s::S]          # (BC, nch) tokens at pos s per chunk
        # col index rank*blk + c*BC + b
        cols[rank * blk:(rank + 1) * blk] = blkv.T.reshape(-1)
    return cols


def _prep_core(inputs, c):
    bs = slice(c * BC, (c + 1) * BC)
    ptok = np.asarray(inputs["passage"][bs]).astype(np.int64)
    qtok = np.asarray(inputs["question"][bs]).astype(np.int64)
    embp = inputs["_embp"]
    pcols = _perm_tokens(ptok, NCHP, FDP)
    qcols = _perm_tokens(qtok, NCHQ, FDQ)
    d = {}
    d["epTp_d"] = np.ascontiguousarray(
        embp[pcols].T.reshape(3, 128, NTP).transpose(1, 0, 2).reshape(128, -1))
    d["epTq_d"] = np.ascontiguousarray(
        embp[qcols].T.reshape(3, 128, NTQ).transpose(1, 0, 2).reshape(128, -1))
    qm0 = (qtok.reshape(-1) == 0).astype(np.float32)
    d["mq"] = np.ascontiguousarray(qm0[None, :])
    pm2 = (ptok.reshape(-1) == 0).reshape(BC, P).astype(np.uint8)
    d["m8"] = np.ascontiguousarray(np.concatenate([pm2, pm2], axis=0))
    return d


def _prep_shared(inputs):
    bf = ml_dtypes.bfloat16

    wihT = np.zeros((4, 3, 128, 3 * HH), bf)      # (d, kc, p, m)
    whhT = np.zeros((4, HH, 3 * HH), bf)          # (d, p, m)
    bhnr = np.zeros((4, HH), bf)
    for di, (pre, dd) in enumerate((("p", "f"), ("p", "b"),
                                    ("q", "f"), ("q", "b"))):
        wih = np.asarray(inputs[f"{pre}_wih_{dd}"], np.float32)
        whh = np.asarray(inputs[f"{pre}_whh_{dd}"], np.float32)
        bih = np.asarray(inputs[f"{pre}_bih_{dd}"], np.float32)
        bhh = np.asarray(inputs[f"{pre}_bhh_{dd}"], np.float32)
        wT = np.zeros((EPAD, 3 * HH), np.float32)
        wT[:E, :] = wih.T
        # row 300: pad-token indicator -> +BIGM on the z gate
        wT[300, HH:2 * HH] = BIGM
        # row 301: constant-1 -> gate biases (bih+bhh for r/z, bih for n)
        wT[301, 0:HH] = bih[0:HH] + bhh[0:HH]
        wT[301, HH:2 * HH] = bih[HH:2 * HH] + bhh[HH:2 * HH]
        wT[301, 2 * HH:] = bih[2 * HH:]
        wihT[di] = wT.astype(bf).reshape(3, 128, 3 * HH)
        whhT[di] = whh.T.astype(bf)
        bhnr[di] = bhh[2 * HH:].astype(bf)
    wihT = np.ascontiguousarray(
        wihT.transpose(2, 0, 1, 3).reshape(128, -1))      # (p,(d,kc,m))
    whhT = np.ascontiguousarray(
        whhT.transpose(1, 0, 2).reshape(128, -1))         # (p,(d,m))
    bhnr = np.ascontiguousarray(bhnr.reshape(1, -1))

    aw = np.asarray(inputs["attn_w"], np.float32)
    w1, w2, w3 = aw[:256], aw[256:512], aw[512:]
    outw = np.zeros((HH, 8), np.float32)
    outw[:, 4], outw[:, 5] = w3[:128], w3[128:]
    outw[0, 2] = float(np.asarray(inputs["attn_b"]))
    outw[0, 3] = float(np.asarray(inputs["start_b"]))
    outw[1, 3] = float(np.asarray(inputs["end_b"]))

    sw = np.asarray(inputs["start_w"], np.float32)
    ew = np.asarray(inputs["end_w"], np.float32)
    sew = np.zeros((HH, 14), bf)
    for j in range(6):
        sew[:, 2 * j] = sw[j * 128:(j + 1) * 128].astype(bf)
        sew[:, 2 * j + 1] = ew[j * 128:(j + 1) * 128].astype(bf)
    sew[:, 12] = w2[:128].astype(bf)
    sew[:, 13] = w2[128:].astype(bf)
    return {"wihT": wihT, "whhT": whhT, "bhnr": bhnr,
            "outw": outw, "sew": sew}


def kernel(**inputs):
    if "nc" not in _CACHE:
        _CACHE["nc"] = _build_nc()
    nc, _es = _CACHE["nc"]
    shared = _prep_shared(inputs)
    bf = ml_dtypes.bfloat16
    embp = np.zeros((VOCAB, EPAD), bf)
    embp[:, :E] = np.asarray(inputs["emb"], np.float32).astype(bf)
    embp[0, 300] = 1.0   # pad-token indicator row
    embp[:, 301] = 1.0   # constant-1 bias row
    inputs = dict(inputs)
    inputs["_embp"] = embp
    in_maps = []
    for c in range(NC):
        m = dict(shared)
        m.update(_prep_core(inputs, c))
        in_maps.append(m)
    res = run_bass_kernel_spmd(nc, in_maps, list(range(NC)))
    outs = [np.asarray(res.results[c]["out"]) for c in range(NC)]
    se = np.concatenate([o[0:2 * BC].reshape(2, BC, P) for o in outs], axis=1)
    lsm = np.concatenate([o[2 * BC:].reshape(2, BC, P) for o in outs], axis=1)
    return (np.ascontiguousarray(se[0]), np.ascontiguousarray(se[1]),
            np.ascontiguousarray(lsm[0]), np.ascontiguousarray(lsm[1]))
